# revision 13
# baseline (speedup 1.0000x reference)
"""GCN (2-layer, symmetric-normalized, self-loops) on 8 TRN2 NeuronCores.

Math (reference):
    A_hat = D^-1/2 (A + I) D^-1/2        (deg over dst incl. self-loops)
    h1    = relu(A_hat @ (x @ W1) + b1)
    out   = log_softmax(A_hat @ h1 @ W2 + b2)

Decomposition (nodes sharded by dst range across 8 cores, 3 launches):
    K1: ut   = dinv * (x @ W1)                       [per-core shard]
        writes ut rows (bf16) for the K2 gather table and utT (fp32)
        for the dense self-loop term.
    K2: htT  = dinv * relu(dinv * (A @ ut + ut_self) + b1)
        gt   = htT @ W2   (so layer 2 aggregates 16-wide)
        writes gt rows (bf16) for the K3 gather table and gtT (fp32).
    K3: out  = log_softmax(dinv * (A @ gt + dinv*gt_self) + b2)

Aggregation per core: edges (no self-loops) grouped by (dst-256-pair,
src chunk) and batched into large dma_gather calls (~11K descriptors)
to amortize the ~10.5us fixed cost per SWDGE gather call.  Scatter into
transposed PSUM accumulators [F, 256] via one bf16 matmul per 128-edge
block: lhsT = gathered rows (stationary), rhs = one-hot slot matrix.
Tables are bf16 with rows duplicated to 256B to satisfy the gather's
minimum element size.
"""

import os
import sys
import types

import numpy as np
import ml_dtypes

# ---------------------------------------------------------------- sizes
N = 100000
E = 1600000
F_IN = 256
H = 64
C = 16
NCORE = 8
P = 128
CHUNK = 25000            # int16-addressable source chunk
GPAIR = 10               # dst-pairs per gather call group
TRACE = bool(int(os.environ.get("BASS_GCN_TRACE", "0")))
SMALL = bool(int(os.environ.get("BASS_GCN_SMALL", "0")))
if SMALL:
    N, E, CHUNK, GPAIR = 12800, 96000, 3200, 3

LAST_EXEC_NS = []
_dbg = None


def _derived():
    ncn = N // NCORE
    padn = ((ncn + 255) // 256) * 256
    nwin = padn // P
    npair = nwin // 2
    nchunk = (N + CHUNK - 1) // CHUNK
    groups = [list(range(g, min(g + GPAIR, npair)))
              for g in range(0, npair, GPAIR)]
    return ncn, padn, nwin, npair, nchunk, groups


# ------------------------------------------------------- ntff shim (opt)
def _install_ntff_shim():
    try:
        if "antenv.axon_hooks" in sys.modules:
            return True
        sys.path.insert(0, "/root/.axon_site/trn_agent_boot")
        from trn_boot import _ntff_profile_via_ctypes  # type: ignore

        mod = types.ModuleType("antenv.axon_hooks")
        holder = [None]
        mod.set_axon_ntff_profile_hook = lambda h: holder.__setitem__(0, h)
        mod.get_axon_ntff_profile_hook = lambda: holder[0]
        sys.modules["antenv.axon_hooks"] = mod
        import antenv

        antenv.axon_hooks = mod
        mod.set_axon_ntff_profile_hook(
            _ntff_profile_via_ctypes("/opt/axon/libaxon_pjrt.so")
        )
        return True
    except Exception:
        return False


# ------------------------------------------------------------ host plan
def _build_plan(edge_index):
    """Edge index structures (functions of edge_index only).

    Per core: edges (no self-loops) with dst in the core's range are
    grouped by (pair = dst>>8, chunk = src//CHUNK) and laid out call by
    call: for each (group of GPAIR pairs, chunk), the member (pair,
    chunk) segments are padded to multiples of 128 descriptors and
    concatenated.  Segment sizes are made uniform across cores (max)
    so one SPMD program fits all.
    """
    ncn, padn, nwin, npair, nchunk, groups = _derived()
    nseg = npair * nchunk

    src_a = np.asarray(edge_index[0], np.int64)
    dst_a = np.asarray(edge_index[1], np.int64)
    deg = np.bincount(dst_a, minlength=N).astype(np.float64) + 1.0
    dinv = (1.0 / np.sqrt(deg)).astype(np.float32)

    per_core = []
    cnts = np.zeros((NCORE, nseg), np.int64)
    for c in range(NCORE):
        lo = c * ncn
        m = (dst_a >= lo) & (dst_a < lo + ncn)
        s = src_a[m]
        d = dst_a[m] - lo
        pair = d >> 8
        chunk = s // CHUNK
        segid = pair * nchunk + chunk
        order = np.argsort(segid, kind="stable")
        s, d, segid = s[order], d[order], segid[order]
        cnts[c] = np.bincount(segid, minlength=nseg)
        per_core.append((s, d, segid))

    S = 128 * ((cnts.max(axis=0) + 127) // 128)          # [nseg] uniform
    S = np.maximum(S, 128)
    # call layout: for each (group g, chunk ch): segments (p in g, ch)
    seg_order = []
    call_sizes = []
    for g in groups:
        for ch in range(nchunk):
            segs = [p * nchunk + ch for p in g]
            seg_order.extend(segs)
            call_sizes.append(int(sum(S[q] for q in segs)))
    assert max(call_sizes) <= 16000, call_sizes
    off = np.zeros(nseg + 1, np.int64)
    tot = 0
    seg_off = {}
    for q in seg_order:
        seg_off[q] = tot
        tot += int(S[q])
    total = tot

    idxw_l, slot_l = [], []
    for c in range(NCORE):
        s, d, segid = per_core[c]
        idx16 = np.zeros(total, np.int16)
        slot = np.full(total, 999.0, np.float32)
        seg_start = np.searchsorted(segid, np.arange(nseg))
        base = np.array([seg_off[q] for q in range(nseg)], np.int64)
        pos = base[segid] + (np.arange(len(s)) - seg_start[segid])
        idx16[pos] = (s % CHUNK).astype(np.int16)
        slot[pos] = (d & 255).astype(np.float32)
        idxw_l.append(np.ascontiguousarray(
            np.tile(idx16.reshape(-1, 16).T, (8, 1))))
        slot_l.append(np.ascontiguousarray(slot.reshape(-1, P).T))

    return {
        "S": S,
        "call_sizes": call_sizes,
        "idxw": idxw_l,
        "slot": slot_l,
        "dinv": dinv,
    }


# --------------------------------------------------------- bass builders
def _bass_mods():
    import concourse.bass as bass
    import concourse.bacc as bacc
    import concourse.tile as tile
    import concourse.mybir as mybir
    from concourse import library_config
    from concourse.masks import make_identity

    return bass, bacc, tile, mybir, library_config, make_identity


def _build_k1():
    """From xT (bf16) produce: ut rows [padn, H] bf16 (gather table
    shard), utT [H, padn] fp32 (self-loop term, already dinv-scaled)."""
    bass, bacc, tile, mybir, libcfg, make_identity = _bass_mods()
    ncn, padn, nwin, npair, nchunk, groups = _derived()
    f32, bf16 = mybir.dt.float32, mybir.dt.bfloat16

    nc = bacc.Bacc("TRN2", target_bir_lowering=False, debug=False,
                   num_devices=NCORE)
    xT = nc.dram_tensor("xT", [F_IN, padn], f32, kind="ExternalInput").ap()
    w1 = nc.dram_tensor("w1", [F_IN, H], f32, kind="ExternalInput").ap()
    dTd = nc.dram_tensor("dinvT", [H, padn], f32, kind="ExternalInput").ap()
    utd = nc.dram_tensor("ut", [padn, P], bf16, kind="ExternalOutput").ap()
    utTd = nc.dram_tensor("utT", [H, padn], f32, kind="ExternalOutput").ap()

    kf = F_IN // P          # 2
    SW = 4                  # windows per stripe (psum bank = 512 fp32)
    stripes = []
    w0 = 0
    while w0 < nwin:
        stripes.append((w0, min(SW, nwin - w0)))
        w0 += SW

    with tile.TileContext(nc) as tc:
        with (
            tc.tile_pool(name="const", bufs=1) as constp,
            tc.tile_pool(name="xin", bufs=3) as xp,
            tc.tile_pool(name="ps", bufs=2, space="PSUM") as psump,
            tc.tile_pool(name="wk", bufs=3) as wp,
        ):
            w1_s = constp.tile([P, kf * H], f32)
            for k in range(kf):
                nc.sync.dma_start(w1_s[:, k * H:(k + 1) * H],
                                  w1[k * P:(k + 1) * P, :])
            identH = constp.tile([H, H], bf16)
            make_identity(nc, identH[:])

            for (ws, sw) in stripes:
                c0 = ws * P
                SC = sw * P
                up = psump.tile([H, SW * P], f32, tag="up", bufs=2)
                for k in range(kf):
                    xt = xp.tile([P, SW * P], f32, tag="xt")
                    nc.sync.dma_start(xt[:, :SC], xT[k * P:(k + 1) * P,
                                                     c0:c0 + SC])
                    nc.tensor.matmul(up[:, :SC],
                                     lhsT=w1_s[:, k * H:(k + 1) * H],
                                     rhs=xt[:, :SC], start=(k == 0),
                                     stop=(k == kf - 1))
                dT = xp.tile([H, SW * P], f32, tag="dT")
                nc.sync.dma_start(dT[:, :SC], dTd[:, c0:c0 + SC])
                uT = wp.tile([H, SW * P], f32, tag="uT")
                nc.vector.tensor_tensor(uT[:, :SC], up[:, :SC], dT[:, :SC],
                                        op=mybir.AluOpType.mult)
                nc.sync.dma_start(utTd[:, c0:c0 + SC], uT[:, :SC])
                uTb = wp.tile([H, SW * P], bf16, tag="uTb")
                nc.vector.tensor_copy(uTb[:, :SC], uT[:, :SC])
                uTr = wp.tile([H, SW * P], f32, tag="uTr")
                nc.vector.tensor_tensor(uTr[:, :SC], uT[:, :SC],
                                        uTb[:, :SC],
                                        op=mybir.AluOpType.subtract)
                uTrb = wp.tile([H, SW * P], bf16, tag="uTrb")
                nc.vector.tensor_copy(uTrb[:, :SC], uTr[:, :SC])
                stage = wp.tile([P, SW, P], bf16, tag="stage")
                for w in range(sw):
                    tp = psump.tile([P, H], bf16, tag="tp", bufs=2)
                    nc.tensor.transpose(tp[:], uTb[:, w * P:(w + 1) * P],
                                        identH[:])
                    nc.vector.tensor_copy(stage[:, w, 0:H], tp[:])
                    tr = psump.tile([P, H], bf16, tag="tr", bufs=2)
                    nc.tensor.transpose(tr[:], uTrb[:, w * P:(w + 1) * P],
                                        identH[:])
                    nc.vector.tensor_copy(stage[:, w, H:P], tr[:])
                nc.sync.dma_start(
                    utd[c0:c0 + SC, :].rearrange("(b a) h -> a b h", b=sw),
                    stage[:, :sw, :])
    nc.compile()
    return nc


def _agg(nc, tc, mybir, pools, table, idx_s, slot_s, iota_s, S, call_sizes,
         feats, drain_fn, lh_slices=None):
    """Batched gather + transposed one-hot scatter.

    For each (group, chunk) call: one dma_gather of call_sizes[i]
    descriptors; per 128-desc block one bf16 matmul accumulating into
    the pair's PSUM tile [feats, 256].  drain_fn(p, ps) after a pair's
    last chunk."""
    f32, bf16 = mybir.dt.float32, mybir.dt.bfloat16
    ncn, padn, nwin, npair, nchunk, groups = _derived()
    if lh_slices is None:
        lh_slices = [(0, feats)]
    nsl = len(lh_slices)
    gatp, selp, psump = pools
    off16 = 0
    mmcol = 0
    ci = 0
    accp = psump.parent_pool if False else None
    for g in groups:
        acc = gatp.tile([feats, GPAIR, 2 * P], f32, tag="acc", bufs=1,
                        name="acc")
        for ch in range(nchunk):
            sz = call_sizes[ci]
            ci += 1
            gat = gatp.tile([P, sz // P, P], bf16, tag="gat", name="gat")
            nc.gpsimd.dma_gather(
                gat[:],
                table[ch * CHUNK:min(N, (ch + 1) * CHUNK), :],
                idx_s[:, off16:off16 + sz // 16],
                sz, sz, P, elem_step=P, single_packet=False,
            )
            off16 += sz // 16
            bb = 0
            for gi, p in enumerate(g):
                nb = int(S[p * nchunk + ch]) // P
                nmm = nb * nsl
                pseg = psump.tile([feats, 2 * P], f32, tag="pseg", bufs=4,
                                  name="pseg")
                done = 0
                for _ in range(nb):
                    sel = selp.tile([P, 2 * P], bf16, tag="sel", name="sel")
                    nc.vector.tensor_tensor(
                        out=sel[:],
                        in0=slot_s[:, mmcol:mmcol + 1].to_broadcast(
                            [P, 2 * P]),
                        in1=iota_s[:],
                        op=mybir.AluOpType.is_equal,
                    )
                    for (a, b) in lh_slices:
                        nc.tensor.matmul(
                            pseg[:], lhsT=gat[:, bb, a:b], rhs=sel[:],
                            start=(done == 0), stop=(done == nmm - 1),
                        )
                        done += 1
                    mmcol += 1
                    bb += 1
                if ch == 0:
                    nc.vector.tensor_copy(acc[:, gi, :], pseg[:])
                else:
                    nc.vector.tensor_tensor(acc[:, gi, :], acc[:, gi, :],
                                            pseg[:],
                                            op=mybir.AluOpType.add)
        for p_i, p in enumerate(g):
            drain_fn(p, acc[:, p_i, :])


def _build_k2(idx_cols, slot_cols, S, call_sizes):
    """Aggregate ut -> htT -> gt rows (bf16) + gtT (fp32)."""
    bass, bacc, tile, mybir, libcfg, make_identity = _bass_mods()
    ncn, padn, nwin, npair, nchunk, groups = _derived()
    f32, bf16 = mybir.dt.float32, mybir.dt.bfloat16

    nc = bacc.Bacc("TRN2", target_bir_lowering=False, debug=False,
                   num_devices=NCORE, dynamic_dma_scratch_size=49152)
    table = nc.dram_tensor("table", [N, P], bf16, kind="ExternalInput").ap()
    idxd = nc.dram_tensor("idx", [P, idx_cols], mybir.dt.int16,
                          kind="ExternalInput").ap()
    slotd = nc.dram_tensor("slot", [P, slot_cols], bf16,
                           kind="ExternalInput").ap()
    iotad = nc.dram_tensor("iota", [P, 2 * P], bf16,
                           kind="ExternalInput").ap()
    utTd = nc.dram_tensor("utT", [H, padn], f32, kind="ExternalInput").ap()
    dT64d = nc.dram_tensor("dinvT64", [H, padn], f32,
                           kind="ExternalInput").ap()
    b1d = nc.dram_tensor("b1col", [H, 1], f32, kind="ExternalInput").ap()
    w2d = nc.dram_tensor("w2", [H, C], bf16, kind="ExternalInput").ap()
    gtd = nc.dram_tensor("gt", [padn, C], bf16, kind="ExternalOutput").ap()
    gtTd = nc.dram_tensor("gtT", [C, padn], f32, kind="ExternalOutput").ap()

    with tile.TileContext(nc) as tc:
        with (
            tc.tile_pool(name="const", bufs=1) as constp,
            tc.tile_pool(name="gat", bufs=2) as gatp,
            tc.tile_pool(name="sel", bufs=4) as selp,
            tc.tile_pool(name="ps", bufs=1, space="PSUM") as psump,
            tc.tile_pool(name="dr", bufs=2) as drp,
            tc.tile_pool(name="st", bufs=2) as stp,
        ):
            with tc.tile_critical():
                nc.gpsimd.load_library(libcfg.mlp)
            idx_s = constp.tile([P, idx_cols], mybir.dt.int16)
            nc.sync.dma_start(idx_s[:], idxd[:, :])
            slot_s = constp.tile([P, slot_cols], bf16)
            nc.sync.dma_start(slot_s[:], slotd[:, :])
            iota_s = constp.tile([P, 2 * P], bf16)
            nc.sync.dma_start(iota_s[:], iotad[:, :])
            b1_s = constp.tile([H, 1], f32)
            nc.sync.dma_start(b1_s[:], b1d[:, :])
            w2_s = constp.tile([H, C], bf16)
            nc.sync.dma_start(w2_s[:], w2d[:, :])
            identC = constp.tile([C, C], bf16)
            make_identity(nc, identC[:])

            # per-pair drain: ps [H, 256] -> htT -> gt rows + gtT cols
            def drain(p, ps):
                c0 = p * 2 * P
                uT = drp.tile([H, 2 * P], f32, tag="uTsl", name="uTsl")
                nc.sync.dma_start(uT[:], utTd[:, c0:c0 + 2 * P])
                dTt = drp.tile([H, 2 * P], f32, tag="dTsl", name="dTsl")
                nc.sync.dma_start(dTt[:], dT64d[:, c0:c0 + 2 * P])
                dT = dTt[:]
                t1 = drp.tile([H, 2 * P], f32, tag="t1", name="t1")
                nc.vector.tensor_tensor(t1[:], ps, uT[:],
                                        op=mybir.AluOpType.add)
                nc.vector.tensor_tensor(t1[:], t1[:], dT,
                                        op=mybir.AluOpType.mult)
                nc.vector.tensor_scalar_add(t1[:], t1[:], b1_s[:])
                nc.vector.tensor_scalar_max(t1[:], t1[:], 0.0)
                hb = drp.tile([H, 2 * P], bf16, tag="hb", name="hb")
                nc.vector.tensor_tensor(hb[:], t1[:], dT,
                                        op=mybir.AluOpType.mult)
                gps = psump.tile([C, 2 * P], f32, tag="gps", bufs=1,
                                 name="gps")
                nc.tensor.matmul(gps[:], lhsT=w2_s[:], rhs=hb[:],
                                 start=True, stop=True)
                gT = drp.tile([C, 2 * P], f32, tag="gT", name="gT")
                nc.vector.tensor_copy(gT[:], gps[:])
                nc.sync.dma_start(gtTd[:, c0:c0 + 2 * P], gT[:])
                gTb = drp.tile([C, 2 * P], bf16, tag="gTb", name="gTb")
                nc.vector.tensor_copy(gTb[:], gT[:])
                stage = stp.tile([P, 2, C], bf16, tag="gstage", name="gstage")
                for w in range(2):
                    tp = psump.tile([P, C], bf16, tag="gtp", bufs=2,
                                    name="gtp")
                    nc.tensor.transpose(tp[:], gTb[:, w * P:(w + 1) * P],
                                        identC[:])
                    nc.vector.tensor_copy(stage[:, w, :], tp[:])
                nc.sync.dma_start(
                    gtd[c0:c0 + 2 * P, :].rearrange("(b a) h -> a b h", b=2),
                    stage[:])

            _agg(nc, tc, mybir, (gatp, selp, psump), table, idx_s, slot_s,
                 iota_s, S, call_sizes, H, drain,
                 lh_slices=[(0, H), (H, 2 * H)])
    nc.compile()
    return nc


def _build_k3(idx_cols, slot_cols, S, call_sizes):
    """Aggregate gt -> log_softmax out rows [padn, C] fp32."""
    bass, bacc, tile, mybir, libcfg, make_identity = _bass_mods()
    ncn, padn, nwin, npair, nchunk, groups = _derived()
    f32, bf16 = mybir.dt.float32, mybir.dt.bfloat16

    nc = bacc.Bacc("TRN2", target_bir_lowering=False, debug=False,
                   num_devices=NCORE, dynamic_dma_scratch_size=49152)
    table = nc.dram_tensor("table", [N, P], bf16, kind="ExternalInput").ap()
    idxd = nc.dram_tensor("idx", [P, idx_cols], mybir.dt.int16,
                          kind="ExternalInput").ap()
    slotd = nc.dram_tensor("slot", [P, slot_cols], bf16,
                           kind="ExternalInput").ap()
    iotad = nc.dram_tensor("iota", [P, 2 * P], bf16,
                           kind="ExternalInput").ap()
    gtTd = nc.dram_tensor("gtT", [C, padn], f32, kind="ExternalInput").ap()
    dT16d = nc.dram_tensor("dinvT16", [C, padn], f32,
                           kind="ExternalInput").ap()
    b2d = nc.dram_tensor("b2rep", [P, C], f32, kind="ExternalInput").ap()
    outd = nc.dram_tensor("out", [padn, C], f32, kind="ExternalOutput").ap()

    with tile.TileContext(nc) as tc:
        with (
            tc.tile_pool(name="const", bufs=1) as constp,
            tc.tile_pool(name="gat", bufs=2) as gatp,
            tc.tile_pool(name="sel", bufs=4) as selp,
            tc.tile_pool(name="ps", bufs=1, space="PSUM") as psump,
            tc.tile_pool(name="dr", bufs=2) as drp,
            tc.tile_pool(name="st", bufs=2) as stp,
        ):
            with tc.tile_critical():
                nc.gpsimd.load_library(libcfg.mlp)
            idx_s = constp.tile([P, idx_cols], mybir.dt.int16)
            nc.sync.dma_start(idx_s[:], idxd[:, :])
            slot_s = constp.tile([P, slot_cols], bf16)
            nc.sync.dma_start(slot_s[:], slotd[:, :])
            iota_s = constp.tile([P, 2 * P], bf16)
            nc.sync.dma_start(iota_s[:], iotad[:, :])
            b2_s = constp.tile([P, C], f32)
            nc.sync.dma_start(b2_s[:], b2d[:, :])
            identC = constp.tile([C, C], bf16)
            make_identity(nc, identC[:])

            def drain(p, ps):
                c0 = p * 2 * P
                gT = drp.tile([C, 2 * P], f32, tag="gTsl", name="gTsl")
                nc.sync.dma_start(gT[:], gtTd[:, c0:c0 + 2 * P])
                dTt = drp.tile([C, 2 * P], f32, tag="dTsl", name="dTsl")
                nc.sync.dma_start(dTt[:], dT16d[:, c0:c0 + 2 * P])
                dT = dTt[:]
                t0 = drp.tile([C, 2 * P], f32, tag="t0", name="t0")
                nc.vector.tensor_tensor(t0[:], ps, gT[:],
                                        op=mybir.AluOpType.add)
                ob = drp.tile([C, 2 * P], bf16, tag="ob", name="ob")
                nc.vector.tensor_tensor(ob[:], t0[:], dT,
                                        op=mybir.AluOpType.mult)
                stage = stp.tile([P, 2, C], f32, tag="ostage", name="ostage")
                for w in range(2):
                    tp = psump.tile([P, C], bf16, tag="otp", bufs=2,
                                    name="otp")
                    nc.tensor.transpose(tp[:], ob[:, w * P:(w + 1) * P],
                                        identC[:])
                    z = drp.tile([P, C], f32, tag="z", name="z")
                    nc.vector.tensor_tensor(z[:], tp[:], b2_s[:],
                                            op=mybir.AluOpType.add)
                    negm = drp.tile([P, 1], f32, tag="negm", name="negm")
                    nc.vector.tensor_reduce(
                        negm[:], z[:], axis=mybir.AxisListType.X,
                        op=mybir.AluOpType.max, negate=True)
                    e = drp.tile([P, C], f32, tag="e", name="e")
                    sa = drp.tile([P, 1], f32, tag="sa", name="sa")
                    nc.scalar.activation(
                        e[:], z[:], mybir.ActivationFunctionType.Exp,
                        bias=negm[:], accum_out=sa[:])
                    lns = drp.tile([P, 1], f32, tag="lns", name="lns")
                    nc.scalar.activation(
                        lns[:], sa[:], mybir.ActivationFunctionType.Ln)
                    nc.vector.tensor_scalar(
                        out=stage[:, w, :], in0=z[:], scalar1=negm[:],
                        scalar2=lns[:], op0=mybir.AluOpType.add,
                        op1=mybir.AluOpType.subtract)
                nc.sync.dma_start(
                    outd[c0:c0 + 2 * P, :].rearrange("(b a) h -> a b h", b=2),
                    stage[:])

            _agg(nc, tc, mybir, (gatp, selp, psump), table, idx_s, slot_s,
                 iota_s, S, call_sizes, C, drain)
    nc.compile()
    return nc


def _run(nc, in_maps):
    if os.environ.get("BASS_GCN_SIM"):
        from concourse.bass_interp import MultiCoreSim

        sim = MultiCoreSim(nc, num_cores=NCORE, trace=False)
        for c in range(NCORE):
            for k, v in in_maps[c].items():
                sim.cores[c].tensor(k)[:] = v
        sim.simulate()
        outs = []
        for c in range(NCORE):
            names = [
                a.memorylocations[0].name
                for a in nc.m.functions[0].allocations
                if getattr(a, "kind", None) == "ExternalOutput"
            ]
            outs.append({n: np.array(sim.cores[c].tensor(n)) for n in names})
        return outs

    from concourse.bass_utils import run_bass_kernel_spmd

    trace = TRACE and _install_ntff_shim()
    res = run_bass_kernel_spmd(nc, in_maps, core_ids=list(range(NCORE)),
                               trace=trace)
    if res.exec_time_ns:
        LAST_EXEC_NS.append(res.exec_time_ns)
    return res.results


# ---------------------------------------------------------------- kernel
def kernel(x, edge_index, W1, b1, W2, b2):
    ncn, padn, nwin, npair, nchunk, groups = _derived()
    LAST_EXEC_NS.clear()

    x = np.asarray(x, np.float32)
    edge_index = np.asarray(edge_index)
    W1 = np.asarray(W1, np.float32)
    b1 = np.asarray(b1, np.float32)
    W2 = np.asarray(W2, np.float32)
    b2 = np.asarray(b2, np.float32)

    plan = _build_plan(edge_index)
    S, call_sizes, dinv = plan["S"], plan["call_sizes"], plan["dinv"]
    idx_cols = plan["idxw"][0].shape[1]
    slot_cols = plan["slot"][0].shape[1]

    iota2 = np.tile(np.arange(2 * P, dtype=np.float32)[None, :], (P, 1))
    b2rep = np.tile(b2[None, :], (P, 1)).astype(np.float32)

    # ---- K1
    nc1 = _build_k1()
    in1 = []
    for c in range(NCORE):
        xc = np.zeros((padn, F_IN), np.float32)
        xc[:ncn] = x[c * ncn:(c + 1) * ncn]
        dv = np.zeros(padn, np.float32)
        dv[:ncn] = dinv[c * ncn:(c + 1) * ncn]
        in1.append({
            "xT": np.ascontiguousarray(xc.T),
            "w1": W1,
            "dinvT": np.ascontiguousarray(
                np.broadcast_to(dv[None, :], (H, padn))),
        })
    r1 = _run(nc1, in1)
    table1 = np.ascontiguousarray(np.concatenate(
        [r1[c]["ut"][:ncn] for c in range(NCORE)], axis=0))

    # ---- K2
    nc2 = _build_k2(idx_cols, slot_cols, S, call_sizes)
    in2 = []
    for c in range(NCORE):
        dv = np.zeros(padn, np.float32)
        dv[:ncn] = dinv[c * ncn:(c + 1) * ncn]
        in2.append({
            "table": table1,
            "idx": plan["idxw"][c],
            "slot": plan["slot"][c].astype(ml_dtypes.bfloat16),
            "iota": iota2.astype(ml_dtypes.bfloat16),
            "utT": r1[c]["utT"],
            "dinvT64": np.ascontiguousarray(
                np.broadcast_to(dv[None, :], (H, padn))),
            "b1col": b1[:, None],
            "w2": W2.astype(ml_dtypes.bfloat16),
        })
    r2 = _run(nc2, in2)
    gt_full = np.concatenate([r2[c]["gt"][:ncn] for c in range(NCORE)],
                             axis=0)
    table2 = np.zeros((N, P), ml_dtypes.bfloat16)
    for rep in range(P // C):
        table2[:, rep * C:(rep + 1) * C] = gt_full

    # ---- K3
    nc3 = _build_k3(idx_cols, slot_cols, S, call_sizes)
    in3 = []
    for c in range(NCORE):
        dv = np.zeros(padn, np.float32)
        dv[:ncn] = dinv[c * ncn:(c + 1) * ncn]
        in3.append({
            "table": table2,
            "idx": plan["idxw"][c],
            "slot": plan["slot"][c].astype(ml_dtypes.bfloat16),
            "iota": iota2.astype(ml_dtypes.bfloat16),
            "gtT": r2[c]["gtT"],
            "dinvT16": np.ascontiguousarray(
                np.broadcast_to(dv[None, :], (C, padn))),
            "b2rep": b2rep,
        })
    r3 = _run(nc3, in3)
    global _dbg
    _dbg = {"r1": r1, "r2": r2, "r3": r3}
    out = np.concatenate([r3[c]["out"][:ncn] for c in range(NCORE)], axis=0)
    return np.ascontiguousarray(out.astype(np.float32))


# revision 14
# speedup vs baseline: 1.3809x; 1.3809x over previous
"""GCN (2-layer, symmetric-normalized, self-loops) on 8 TRN2 NeuronCores.

Math (reference):
    A_hat = D^-1/2 (A + I) D^-1/2        (deg over dst incl. self-loops)
    h1    = relu(A_hat @ (x @ W1) + b1)
    out   = log_softmax(A_hat @ h1 @ W2 + b2)

Decomposition (nodes sharded by dst range across 8 cores, 3 launches):
    K1: ut   = dinv * (x @ W1)                       [per-core shard]
        writes ut rows (bf16) for the K2 gather table and utT (fp32)
        for the dense self-loop term.
    K2: htT  = dinv * relu(dinv * (A @ ut + ut_self) + b1)
        gt   = htT @ W2   (so layer 2 aggregates 16-wide)
        writes gt rows (bf16) for the K3 gather table and gtT (fp32).
    K3: out  = log_softmax(dinv * (A @ gt + dinv*gt_self) + b2)

Aggregation per core: edges (no self-loops) grouped by (dst-256-pair,
src chunk) and batched into large dma_gather calls (~11K descriptors)
to amortize the ~10.5us fixed cost per SWDGE gather call.  Scatter into
transposed PSUM accumulators [F, 256] via one bf16 matmul per 128-edge
block: lhsT = gathered rows (stationary), rhs = one-hot slot matrix.
Tables are bf16 with rows duplicated to 256B to satisfy the gather's
minimum element size.
"""

import os
import sys
import types

import numpy as np
import ml_dtypes

# ---------------------------------------------------------------- sizes
N = 100000
E = 1600000
F_IN = 256
H = 64
C = 16
NCORE = 8
P = 128
CHUNK = 25000            # int16-addressable source chunk
GPAIR = 4                # dst-pairs per gather call group
TRACE = bool(int(os.environ.get("BASS_GCN_TRACE", "0")))
SMALL = bool(int(os.environ.get("BASS_GCN_SMALL", "0")))
if SMALL:
    N, E, CHUNK, GPAIR = 12800, 96000, 3200, 3

LAST_EXEC_NS = []
_dbg = None


def _derived():
    ncn = N // NCORE
    padn = ((ncn + 255) // 256) * 256
    nwin = padn // P
    npair = nwin // 2
    nchunk = (N + CHUNK - 1) // CHUNK
    groups = [list(range(g, min(g + GPAIR, npair)))
              for g in range(0, npair, GPAIR)]
    return ncn, padn, nwin, npair, nchunk, groups


# ------------------------------------------------------- ntff shim (opt)
def _install_ntff_shim():
    try:
        if "antenv.axon_hooks" in sys.modules:
            return True
        sys.path.insert(0, "/root/.axon_site/trn_agent_boot")
        from trn_boot import _ntff_profile_via_ctypes  # type: ignore

        mod = types.ModuleType("antenv.axon_hooks")
        holder = [None]
        mod.set_axon_ntff_profile_hook = lambda h: holder.__setitem__(0, h)
        mod.get_axon_ntff_profile_hook = lambda: holder[0]
        sys.modules["antenv.axon_hooks"] = mod
        import antenv

        antenv.axon_hooks = mod
        mod.set_axon_ntff_profile_hook(
            _ntff_profile_via_ctypes("/opt/axon/libaxon_pjrt.so")
        )
        return True
    except Exception:
        return False


# ------------------------------------------------------------ host plan
def _build_plan(edge_index):
    """Edge index structures (functions of edge_index only).

    Per core: edges (no self-loops) with dst in the core's range are
    grouped by (pair = dst>>8, chunk = src//CHUNK) and laid out call by
    call: for each (group of GPAIR pairs, chunk), the member (pair,
    chunk) segments are padded to multiples of 128 descriptors and
    concatenated.  Segment sizes are made uniform across cores (max)
    so one SPMD program fits all.
    """
    ncn, padn, nwin, npair, nchunk, groups = _derived()
    nseg = npair * nchunk

    src_a = np.asarray(edge_index[0], np.int64)
    dst_a = np.asarray(edge_index[1], np.int64)
    deg = np.bincount(dst_a, minlength=N).astype(np.float64) + 1.0
    dinv = (1.0 / np.sqrt(deg)).astype(np.float32)

    per_core = []
    cnts = np.zeros((NCORE, nseg), np.int64)
    for c in range(NCORE):
        lo = c * ncn
        m = (dst_a >= lo) & (dst_a < lo + ncn)
        s = src_a[m]
        d = dst_a[m] - lo
        pair = d >> 8
        chunk = s // CHUNK
        segid = pair * nchunk + chunk
        order = np.argsort(segid, kind="stable")
        s, d, segid = s[order], d[order], segid[order]
        cnts[c] = np.bincount(segid, minlength=nseg)
        per_core.append((s, d, segid))

    S = 128 * ((cnts.max(axis=0) + 127) // 128)          # [nseg] uniform
    S = np.maximum(S, 128)
    # call layout: for each (group g, chunk ch): segments (p in g, ch)
    seg_order = []
    call_sizes = []
    for g in groups:
        for ch in range(nchunk):
            segs = [p * nchunk + ch for p in g]
            seg_order.extend(segs)
            call_sizes.append(int(sum(S[q] for q in segs)))
    assert max(call_sizes) <= 16000, call_sizes
    off = np.zeros(nseg + 1, np.int64)
    tot = 0
    seg_off = {}
    for q in seg_order:
        seg_off[q] = tot
        tot += int(S[q])
    total = tot

    idxw_l, slot_l = [], []
    for c in range(NCORE):
        s, d, segid = per_core[c]
        idx16 = np.zeros(total, np.int16)
        slot = np.full(total, 999.0, np.float32)
        seg_start = np.searchsorted(segid, np.arange(nseg))
        base = np.array([seg_off[q] for q in range(nseg)], np.int64)
        pos = base[segid] + (np.arange(len(s)) - seg_start[segid])
        idx16[pos] = (s % CHUNK).astype(np.int16)
        slot[pos] = (d & 255).astype(np.float32)
        idxw_l.append(np.ascontiguousarray(
            np.tile(idx16.reshape(-1, 16).T, (8, 1))))
        slot_l.append(np.ascontiguousarray(slot.reshape(-1, P).T))

    return {
        "S": S,
        "call_sizes": call_sizes,
        "idxw": idxw_l,
        "slot": slot_l,
        "dinv": dinv,
    }


# --------------------------------------------------------- bass builders
def _bass_mods():
    import concourse.bass as bass
    import concourse.bacc as bacc
    import concourse.tile as tile
    import concourse.mybir as mybir
    from concourse import library_config
    from concourse.masks import make_identity

    return bass, bacc, tile, mybir, library_config, make_identity


def _build_k1():
    """From xT (bf16) produce: ut rows [padn, H] bf16 (gather table
    shard), utT [H, padn] fp32 (self-loop term, already dinv-scaled)."""
    bass, bacc, tile, mybir, libcfg, make_identity = _bass_mods()
    ncn, padn, nwin, npair, nchunk, groups = _derived()
    f32, bf16 = mybir.dt.float32, mybir.dt.bfloat16

    nc = bacc.Bacc("TRN2", target_bir_lowering=False, debug=False,
                   num_devices=NCORE)
    xT = nc.dram_tensor("xT", [F_IN, padn], f32, kind="ExternalInput").ap()
    w1 = nc.dram_tensor("w1", [F_IN, H], f32, kind="ExternalInput").ap()
    dTd = nc.dram_tensor("dinvT", [H, padn], f32, kind="ExternalInput").ap()
    utd = nc.dram_tensor("ut", [padn, P], bf16, kind="ExternalOutput").ap()
    utTd = nc.dram_tensor("utT", [H, padn], f32, kind="ExternalOutput").ap()

    kf = F_IN // P          # 2
    SW = 4                  # windows per stripe (psum bank = 512 fp32)
    stripes = []
    w0 = 0
    while w0 < nwin:
        stripes.append((w0, min(SW, nwin - w0)))
        w0 += SW

    with tile.TileContext(nc) as tc:
        with (
            tc.tile_pool(name="const", bufs=1) as constp,
            tc.tile_pool(name="xin", bufs=3) as xp,
            tc.tile_pool(name="ps", bufs=2, space="PSUM") as psump,
            tc.tile_pool(name="wk", bufs=3) as wp,
        ):
            w1_s = constp.tile([P, kf * H], f32)
            for k in range(kf):
                nc.sync.dma_start(w1_s[:, k * H:(k + 1) * H],
                                  w1[k * P:(k + 1) * P, :])
            identH = constp.tile([H, H], bf16)
            make_identity(nc, identH[:])

            for (ws, sw) in stripes:
                c0 = ws * P
                SC = sw * P
                up = psump.tile([H, SW * P], f32, tag="up", bufs=2)
                for k in range(kf):
                    xt = xp.tile([P, SW * P], f32, tag="xt")
                    nc.sync.dma_start(xt[:, :SC], xT[k * P:(k + 1) * P,
                                                     c0:c0 + SC])
                    nc.tensor.matmul(up[:, :SC],
                                     lhsT=w1_s[:, k * H:(k + 1) * H],
                                     rhs=xt[:, :SC], start=(k == 0),
                                     stop=(k == kf - 1))
                dT = xp.tile([H, SW * P], f32, tag="dT")
                nc.sync.dma_start(dT[:, :SC], dTd[:, c0:c0 + SC])
                uT = wp.tile([H, SW * P], f32, tag="uT")
                nc.vector.tensor_tensor(uT[:, :SC], up[:, :SC], dT[:, :SC],
                                        op=mybir.AluOpType.mult)
                nc.sync.dma_start(utTd[:, c0:c0 + SC], uT[:, :SC])
                uTb = wp.tile([H, SW * P], bf16, tag="uTb")
                nc.vector.tensor_copy(uTb[:, :SC], uT[:, :SC])
                uTr = wp.tile([H, SW * P], f32, tag="uTr")
                nc.vector.tensor_tensor(uTr[:, :SC], uT[:, :SC],
                                        uTb[:, :SC],
                                        op=mybir.AluOpType.subtract)
                uTrb = wp.tile([H, SW * P], bf16, tag="uTrb")
                nc.vector.tensor_copy(uTrb[:, :SC], uTr[:, :SC])
                stage = wp.tile([P, SW, P], bf16, tag="stage")
                for w in range(sw):
                    tp = psump.tile([P, H], bf16, tag="tp", bufs=2)
                    nc.tensor.transpose(tp[:], uTb[:, w * P:(w + 1) * P],
                                        identH[:])
                    nc.vector.tensor_copy(stage[:, w, 0:H], tp[:])
                    tr = psump.tile([P, H], bf16, tag="tr", bufs=2)
                    nc.tensor.transpose(tr[:], uTrb[:, w * P:(w + 1) * P],
                                        identH[:])
                    nc.vector.tensor_copy(stage[:, w, H:P], tr[:])
                nc.sync.dma_start(
                    utd[c0:c0 + SC, :].rearrange("(b a) h -> a b h", b=sw),
                    stage[:, :sw, :])
    nc.compile()
    return nc


def _agg(nc, tc, mybir, pools, table, idx_s, slot_s, iota_s, S, call_sizes,
         feats, drain_fn, lh_slices=None):
    """Batched gather + transposed one-hot scatter.

    For each (group, chunk) call: one dma_gather of call_sizes[i]
    descriptors; per 128-desc block one bf16 matmul accumulating into
    the pair's PSUM tile [feats, 256].  drain_fn(p, ps) after a pair's
    last chunk."""
    f32, bf16 = mybir.dt.float32, mybir.dt.bfloat16
    ncn, padn, nwin, npair, nchunk, groups = _derived()
    if lh_slices is None:
        lh_slices = [(0, feats)]
    nsl = len(lh_slices)
    gatp, selp, psump = pools
    off16 = 0
    mmcol = 0
    ci = 0
    accp = psump.parent_pool if False else None
    for g in groups:
        acc = gatp.tile([feats, GPAIR, 2 * P], f32, tag="acc", bufs=1,
                        name="acc")
        for ch in range(nchunk):
            sz = call_sizes[ci]
            ci += 1
            gat = gatp.tile([P, sz // P, P], bf16, tag="gat", name="gat")
            nc.gpsimd.dma_gather(
                gat[:],
                table[ch * CHUNK:min(N, (ch + 1) * CHUNK), :],
                idx_s[:, off16:off16 + sz // 16],
                sz, sz, P, elem_step=P, single_packet=False,
            )
            off16 += sz // 16
            bb = 0
            for gi, p in enumerate(g):
                nb = int(S[p * nchunk + ch]) // P
                nmm = nb * nsl
                pseg = psump.tile([feats, 2 * P], f32, tag="pseg", bufs=4,
                                  name="pseg")
                done = 0
                for _ in range(nb):
                    sel = selp.tile([P, 2 * P], bf16, tag="sel", name="sel")
                    nc.vector.tensor_tensor(
                        out=sel[:],
                        in0=slot_s[:, mmcol:mmcol + 1].to_broadcast(
                            [P, 2 * P]),
                        in1=iota_s[:],
                        op=mybir.AluOpType.is_equal,
                    )
                    for (a, b) in lh_slices:
                        nc.tensor.matmul(
                            pseg[:], lhsT=gat[:, bb, a:b], rhs=sel[:],
                            start=(done == 0), stop=(done == nmm - 1),
                        )
                        done += 1
                    mmcol += 1
                    bb += 1
                if ch == 0:
                    nc.vector.tensor_copy(acc[:, gi, :], pseg[:])
                else:
                    nc.vector.tensor_tensor(acc[:, gi, :], acc[:, gi, :],
                                            pseg[:],
                                            op=mybir.AluOpType.add)
        for p_i, p in enumerate(g):
            drain_fn(p, acc[:, p_i, :])


def _build_k2(idx_cols, slot_cols, S, call_sizes):
    """Aggregate ut -> htT -> gt rows (bf16) + gtT (fp32)."""
    bass, bacc, tile, mybir, libcfg, make_identity = _bass_mods()
    ncn, padn, nwin, npair, nchunk, groups = _derived()
    f32, bf16 = mybir.dt.float32, mybir.dt.bfloat16

    nc = bacc.Bacc("TRN2", target_bir_lowering=False, debug=False,
                   num_devices=NCORE, dynamic_dma_scratch_size=49152)
    table = nc.dram_tensor("table", [N, P], bf16, kind="ExternalInput").ap()
    idxd = nc.dram_tensor("idx", [P, idx_cols], mybir.dt.int16,
                          kind="ExternalInput").ap()
    slotd = nc.dram_tensor("slot", [P, slot_cols], bf16,
                           kind="ExternalInput").ap()
    iotad = nc.dram_tensor("iota", [P, 2 * P], bf16,
                           kind="ExternalInput").ap()
    utTd = nc.dram_tensor("utT", [H, padn], f32, kind="ExternalInput").ap()
    dT64d = nc.dram_tensor("dinvT64", [H, padn], f32,
                           kind="ExternalInput").ap()
    b1d = nc.dram_tensor("b1col", [H, 1], f32, kind="ExternalInput").ap()
    w2d = nc.dram_tensor("w2", [H, C], bf16, kind="ExternalInput").ap()
    gtd = nc.dram_tensor("gt", [padn, C], bf16, kind="ExternalOutput").ap()
    gtTd = nc.dram_tensor("gtT", [C, padn], f32, kind="ExternalOutput").ap()

    with tile.TileContext(nc) as tc:
        with (
            tc.tile_pool(name="const", bufs=1) as constp,
            tc.tile_pool(name="gat", bufs=2) as gatp,
            tc.tile_pool(name="sel", bufs=4) as selp,
            tc.tile_pool(name="ps", bufs=1, space="PSUM") as psump,
            tc.tile_pool(name="dr", bufs=2) as drp,
            tc.tile_pool(name="st", bufs=2) as stp,
        ):
            with tc.tile_critical():
                nc.gpsimd.load_library(libcfg.mlp)
            idx_s = constp.tile([P, idx_cols], mybir.dt.int16)
            nc.sync.dma_start(idx_s[:], idxd[:, :])
            slot_s = constp.tile([P, slot_cols], bf16)
            nc.sync.dma_start(slot_s[:], slotd[:, :])
            iota_s = constp.tile([P, 2 * P], bf16)
            nc.sync.dma_start(iota_s[:], iotad[:, :])
            b1_s = constp.tile([H, 1], f32)
            nc.sync.dma_start(b1_s[:], b1d[:, :])
            w2_s = constp.tile([H, C], bf16)
            nc.sync.dma_start(w2_s[:], w2d[:, :])
            identC = constp.tile([C, C], bf16)
            make_identity(nc, identC[:])

            # per-pair drain: ps [H, 256] -> htT -> gt rows + gtT cols
            def drain(p, ps):
                c0 = p * 2 * P
                uT = drp.tile([H, 2 * P], f32, tag="uTsl", name="uTsl")
                nc.sync.dma_start(uT[:], utTd[:, c0:c0 + 2 * P])
                dTt = drp.tile([H, 2 * P], f32, tag="dTsl", name="dTsl")
                nc.sync.dma_start(dTt[:], dT64d[:, c0:c0 + 2 * P])
                dT = dTt[:]
                t1 = drp.tile([H, 2 * P], f32, tag="t1", name="t1")
                nc.vector.tensor_tensor(t1[:], ps, uT[:],
                                        op=mybir.AluOpType.add)
                nc.vector.tensor_tensor(t1[:], t1[:], dT,
                                        op=mybir.AluOpType.mult)
                nc.vector.tensor_scalar_add(t1[:], t1[:], b1_s[:])
                nc.vector.tensor_scalar_max(t1[:], t1[:], 0.0)
                hb = drp.tile([H, 2 * P], bf16, tag="hb", name="hb")
                nc.vector.tensor_tensor(hb[:], t1[:], dT,
                                        op=mybir.AluOpType.mult)
                gps = psump.tile([C, 2 * P], f32, tag="gps", bufs=1,
                                 name="gps")
                nc.tensor.matmul(gps[:], lhsT=w2_s[:], rhs=hb[:],
                                 start=True, stop=True)
                gT = drp.tile([C, 2 * P], f32, tag="gT", name="gT")
                nc.vector.tensor_copy(gT[:], gps[:])
                nc.sync.dma_start(gtTd[:, c0:c0 + 2 * P], gT[:])
                gTb = drp.tile([C, 2 * P], bf16, tag="gTb", name="gTb")
                nc.vector.tensor_copy(gTb[:], gT[:])
                stage = stp.tile([P, 2, C], bf16, tag="gstage", name="gstage")
                for w in range(2):
                    tp = psump.tile([P, C], bf16, tag="gtp", bufs=2,
                                    name="gtp")
                    nc.tensor.transpose(tp[:], gTb[:, w * P:(w + 1) * P],
                                        identC[:])
                    nc.vector.tensor_copy(stage[:, w, :], tp[:])
                nc.sync.dma_start(
                    gtd[c0:c0 + 2 * P, :].rearrange("(b a) h -> a b h", b=2),
                    stage[:])

            _agg(nc, tc, mybir, (gatp, selp, psump), table, idx_s, slot_s,
                 iota_s, S, call_sizes, H, drain,
                 lh_slices=[(0, H), (H, 2 * H)])
    nc.compile()
    return nc


def _build_k3(idx_cols, slot_cols, S, call_sizes):
    """Aggregate gt -> log_softmax out rows [padn, C] fp32."""
    bass, bacc, tile, mybir, libcfg, make_identity = _bass_mods()
    ncn, padn, nwin, npair, nchunk, groups = _derived()
    f32, bf16 = mybir.dt.float32, mybir.dt.bfloat16

    nc = bacc.Bacc("TRN2", target_bir_lowering=False, debug=False,
                   num_devices=NCORE, dynamic_dma_scratch_size=49152)
    table = nc.dram_tensor("table", [N, P], bf16, kind="ExternalInput").ap()
    idxd = nc.dram_tensor("idx", [P, idx_cols], mybir.dt.int16,
                          kind="ExternalInput").ap()
    slotd = nc.dram_tensor("slot", [P, slot_cols], bf16,
                           kind="ExternalInput").ap()
    iotad = nc.dram_tensor("iota", [P, 2 * P], bf16,
                           kind="ExternalInput").ap()
    gtTd = nc.dram_tensor("gtT", [C, padn], f32, kind="ExternalInput").ap()
    dT16d = nc.dram_tensor("dinvT16", [C, padn], f32,
                           kind="ExternalInput").ap()
    b2d = nc.dram_tensor("b2rep", [P, C], f32, kind="ExternalInput").ap()
    outd = nc.dram_tensor("out", [padn, C], f32, kind="ExternalOutput").ap()

    with tile.TileContext(nc) as tc:
        with (
            tc.tile_pool(name="const", bufs=1) as constp,
            tc.tile_pool(name="gat", bufs=2) as gatp,
            tc.tile_pool(name="sel", bufs=4) as selp,
            tc.tile_pool(name="ps", bufs=1, space="PSUM") as psump,
            tc.tile_pool(name="dr", bufs=2) as drp,
            tc.tile_pool(name="st", bufs=2) as stp,
        ):
            with tc.tile_critical():
                nc.gpsimd.load_library(libcfg.mlp)
            idx_s = constp.tile([P, idx_cols], mybir.dt.int16)
            nc.sync.dma_start(idx_s[:], idxd[:, :])
            slot_s = constp.tile([P, slot_cols], bf16)
            nc.sync.dma_start(slot_s[:], slotd[:, :])
            iota_s = constp.tile([P, 2 * P], bf16)
            nc.sync.dma_start(iota_s[:], iotad[:, :])
            b2_s = constp.tile([P, C], f32)
            nc.sync.dma_start(b2_s[:], b2d[:, :])
            identC = constp.tile([C, C], bf16)
            make_identity(nc, identC[:])

            def drain(p, ps):
                c0 = p * 2 * P
                gT = drp.tile([C, 2 * P], f32, tag="gTsl", name="gTsl")
                nc.sync.dma_start(gT[:], gtTd[:, c0:c0 + 2 * P])
                dTt = drp.tile([C, 2 * P], f32, tag="dTsl", name="dTsl")
                nc.sync.dma_start(dTt[:], dT16d[:, c0:c0 + 2 * P])
                dT = dTt[:]
                t0 = drp.tile([C, 2 * P], f32, tag="t0", name="t0")
                nc.vector.tensor_tensor(t0[:], ps, gT[:],
                                        op=mybir.AluOpType.add)
                ob = drp.tile([C, 2 * P], bf16, tag="ob", name="ob")
                nc.vector.tensor_tensor(ob[:], t0[:], dT,
                                        op=mybir.AluOpType.mult)
                stage = stp.tile([P, 2, C], f32, tag="ostage", name="ostage")
                for w in range(2):
                    tp = psump.tile([P, C], bf16, tag="otp", bufs=2,
                                    name="otp")
                    nc.tensor.transpose(tp[:], ob[:, w * P:(w + 1) * P],
                                        identC[:])
                    z = drp.tile([P, C], f32, tag="z", name="z")
                    nc.vector.tensor_tensor(z[:], tp[:], b2_s[:],
                                            op=mybir.AluOpType.add)
                    negm = drp.tile([P, 1], f32, tag="negm", name="negm")
                    nc.vector.tensor_reduce(
                        negm[:], z[:], axis=mybir.AxisListType.X,
                        op=mybir.AluOpType.max, negate=True)
                    e = drp.tile([P, C], f32, tag="e", name="e")
                    sa = drp.tile([P, 1], f32, tag="sa", name="sa")
                    nc.scalar.activation(
                        e[:], z[:], mybir.ActivationFunctionType.Exp,
                        bias=negm[:], accum_out=sa[:])
                    lns = drp.tile([P, 1], f32, tag="lns", name="lns")
                    nc.scalar.activation(
                        lns[:], sa[:], mybir.ActivationFunctionType.Ln)
                    nc.vector.tensor_scalar(
                        out=stage[:, w, :], in0=z[:], scalar1=negm[:],
                        scalar2=lns[:], op0=mybir.AluOpType.add,
                        op1=mybir.AluOpType.subtract)
                nc.sync.dma_start(
                    outd[c0:c0 + 2 * P, :].rearrange("(b a) h -> a b h", b=2),
                    stage[:])

            _agg(nc, tc, mybir, (gatp, selp, psump), table, idx_s, slot_s,
                 iota_s, S, call_sizes, C, drain)
    nc.compile()
    return nc


def _run(nc, in_maps):
    if os.environ.get("BASS_GCN_SIM"):
        from concourse.bass_interp import MultiCoreSim

        sim = MultiCoreSim(nc, num_cores=NCORE, trace=False)
        for c in range(NCORE):
            for k, v in in_maps[c].items():
                sim.cores[c].tensor(k)[:] = v
        sim.simulate()
        outs = []
        for c in range(NCORE):
            names = [
                a.memorylocations[0].name
                for a in nc.m.functions[0].allocations
                if getattr(a, "kind", None) == "ExternalOutput"
            ]
            outs.append({n: np.array(sim.cores[c].tensor(n)) for n in names})
        return outs

    from concourse.bass_utils import run_bass_kernel_spmd

    trace = TRACE and _install_ntff_shim()
    res = run_bass_kernel_spmd(nc, in_maps, core_ids=list(range(NCORE)),
                               trace=trace)
    if res.exec_time_ns:
        LAST_EXEC_NS.append(res.exec_time_ns)
    return res.results


# ---------------------------------------------------------------- kernel
def kernel(x, edge_index, W1, b1, W2, b2):
    ncn, padn, nwin, npair, nchunk, groups = _derived()
    LAST_EXEC_NS.clear()

    x = np.asarray(x, np.float32)
    edge_index = np.asarray(edge_index)
    W1 = np.asarray(W1, np.float32)
    b1 = np.asarray(b1, np.float32)
    W2 = np.asarray(W2, np.float32)
    b2 = np.asarray(b2, np.float32)

    plan = _build_plan(edge_index)
    S, call_sizes, dinv = plan["S"], plan["call_sizes"], plan["dinv"]
    idx_cols = plan["idxw"][0].shape[1]
    slot_cols = plan["slot"][0].shape[1]

    iota2 = np.tile(np.arange(2 * P, dtype=np.float32)[None, :], (P, 1))
    b2rep = np.tile(b2[None, :], (P, 1)).astype(np.float32)

    # ---- K1
    nc1 = _build_k1()
    in1 = []
    for c in range(NCORE):
        xc = np.zeros((padn, F_IN), np.float32)
        xc[:ncn] = x[c * ncn:(c + 1) * ncn]
        dv = np.zeros(padn, np.float32)
        dv[:ncn] = dinv[c * ncn:(c + 1) * ncn]
        in1.append({
            "xT": np.ascontiguousarray(xc.T),
            "w1": W1,
            "dinvT": np.ascontiguousarray(
                np.broadcast_to(dv[None, :], (H, padn))),
        })
    r1 = _run(nc1, in1)
    table1 = np.ascontiguousarray(np.concatenate(
        [r1[c]["ut"][:ncn] for c in range(NCORE)], axis=0))

    # ---- K2
    nc2 = _build_k2(idx_cols, slot_cols, S, call_sizes)
    in2 = []
    for c in range(NCORE):
        dv = np.zeros(padn, np.float32)
        dv[:ncn] = dinv[c * ncn:(c + 1) * ncn]
        in2.append({
            "table": table1,
            "idx": plan["idxw"][c],
            "slot": plan["slot"][c].astype(ml_dtypes.bfloat16),
            "iota": iota2.astype(ml_dtypes.bfloat16),
            "utT": r1[c]["utT"],
            "dinvT64": np.ascontiguousarray(
                np.broadcast_to(dv[None, :], (H, padn))),
            "b1col": b1[:, None],
            "w2": W2.astype(ml_dtypes.bfloat16),
        })
    r2 = _run(nc2, in2)
    gt_full = np.concatenate([r2[c]["gt"][:ncn] for c in range(NCORE)],
                             axis=0)
    table2 = np.zeros((N, P), ml_dtypes.bfloat16)
    for rep in range(P // C):
        table2[:, rep * C:(rep + 1) * C] = gt_full

    # ---- K3
    nc3 = _build_k3(idx_cols, slot_cols, S, call_sizes)
    in3 = []
    for c in range(NCORE):
        dv = np.zeros(padn, np.float32)
        dv[:ncn] = dinv[c * ncn:(c + 1) * ncn]
        in3.append({
            "table": table2,
            "idx": plan["idxw"][c],
            "slot": plan["slot"][c].astype(ml_dtypes.bfloat16),
            "iota": iota2.astype(ml_dtypes.bfloat16),
            "gtT": r2[c]["gtT"],
            "dinvT16": np.ascontiguousarray(
                np.broadcast_to(dv[None, :], (C, padn))),
            "b2rep": b2rep,
        })
    r3 = _run(nc3, in3)
    global _dbg
    _dbg = {"r1": r1, "r2": r2, "r3": r3}
    out = np.concatenate([r3[c]["out"][:ncn] for c in range(NCORE)], axis=0)
    return np.ascontiguousarray(out.astype(np.float32))


# revision 15
# speedup vs baseline: 1.5951x; 1.1551x over previous
"""GCN (2-layer, symmetric-normalized, self-loops) on 8 TRN2 NeuronCores.

Math (reference):
    A_hat = D^-1/2 (A + I) D^-1/2        (deg over dst incl. self-loops)
    h1    = relu(A_hat @ (x @ W1) + b1)
    out   = log_softmax(A_hat @ h1 @ W2 + b2)

Decomposition (nodes sharded by dst range across 8 cores, 3 launches):
    K1: ut   = dinv * (x @ W1)                       [per-core shard]
        writes ut rows (bf16) for the K2 gather table and utT (fp32)
        for the dense self-loop term.
    K2: htT  = dinv * relu(dinv * (A @ ut + ut_self) + b1)
        gt   = htT @ W2   (so layer 2 aggregates 16-wide)
        writes gt rows (bf16) for the K3 gather table and gtT (fp32).
    K3: out  = log_softmax(dinv * (A @ gt + dinv*gt_self) + b2)

Aggregation per core: edges (no self-loops) grouped by (dst-256-pair,
src chunk) and batched into large dma_gather calls (~11K descriptors)
to amortize the ~10.5us fixed cost per SWDGE gather call.  Scatter into
transposed PSUM accumulators [F, 256] via one bf16 matmul per 128-edge
block: lhsT = gathered rows (stationary), rhs = one-hot slot matrix.
Tables are bf16 with rows duplicated to 256B to satisfy the gather's
minimum element size.
"""

import os
import sys
import types

import numpy as np
import ml_dtypes

# ---------------------------------------------------------------- sizes
N = 100000
E = 1600000
F_IN = 256
H = 64
C = 16
NCORE = 8
P = 128
CHUNK = 25000            # int16-addressable source chunk
GPAIR = 4                # dst-pairs per gather call group
TRACE = bool(int(os.environ.get("BASS_GCN_TRACE", "0")))
SMALL = bool(int(os.environ.get("BASS_GCN_SMALL", "0")))
if SMALL:
    N, E, CHUNK, GPAIR = 12800, 96000, 3200, 3

LAST_EXEC_NS = []
_dbg = None


def _derived():
    ncn = N // NCORE
    padn = ((ncn + 255) // 256) * 256
    nwin = padn // P
    npair = nwin // 2
    nchunk = (N + CHUNK - 1) // CHUNK
    groups = [list(range(g, min(g + GPAIR, npair)))
              for g in range(0, npair, GPAIR)]
    return ncn, padn, nwin, npair, nchunk, groups


# ------------------------------------------------------- ntff shim (opt)
def _install_ntff_shim():
    try:
        if "antenv.axon_hooks" in sys.modules:
            return True
        sys.path.insert(0, "/root/.axon_site/trn_agent_boot")
        from trn_boot import _ntff_profile_via_ctypes  # type: ignore

        mod = types.ModuleType("antenv.axon_hooks")
        holder = [None]
        mod.set_axon_ntff_profile_hook = lambda h: holder.__setitem__(0, h)
        mod.get_axon_ntff_profile_hook = lambda: holder[0]
        sys.modules["antenv.axon_hooks"] = mod
        import antenv

        antenv.axon_hooks = mod
        mod.set_axon_ntff_profile_hook(
            _ntff_profile_via_ctypes("/opt/axon/libaxon_pjrt.so")
        )
        return True
    except Exception:
        return False


# ------------------------------------------------------------ host plan
def _build_plan(edge_index):
    """Edge index structures (functions of edge_index only).

    Per core: edges (no self-loops) with dst in the core's range are
    grouped by (pair = dst>>8, chunk = src//CHUNK) and laid out call by
    call: for each (group of GPAIR pairs, chunk), the member (pair,
    chunk) segments are padded to multiples of 128 descriptors and
    concatenated.  Segment sizes are made uniform across cores (max)
    so one SPMD program fits all.
    """
    ncn, padn, nwin, npair, nchunk, groups = _derived()
    nseg = npair * nchunk

    src_a = np.asarray(edge_index[0], np.int64)
    dst_a = np.asarray(edge_index[1], np.int64)
    deg = np.bincount(dst_a, minlength=N).astype(np.float64) + 1.0
    dinv = (1.0 / np.sqrt(deg)).astype(np.float32)

    per_core = []
    cnts = np.zeros((NCORE, nseg), np.int64)
    for c in range(NCORE):
        lo = c * ncn
        m = (dst_a >= lo) & (dst_a < lo + ncn)
        s = src_a[m]
        d = dst_a[m] - lo
        pair = d >> 8
        chunk = s // CHUNK
        segid = pair * nchunk + chunk
        order = np.argsort(segid, kind="stable")
        s, d, segid = s[order], d[order], segid[order]
        cnts[c] = np.bincount(segid, minlength=nseg)
        per_core.append((s, d, segid))

    S = 128 * ((cnts.max(axis=0) + 127) // 128)          # [nseg] uniform
    S = np.maximum(S, 128)
    # call layout: for each (group g, chunk ch): segments (p in g, ch)
    seg_order = []
    call_sizes = []
    for g in groups:
        for ch in range(nchunk):
            segs = [p * nchunk + ch for p in g]
            seg_order.extend(segs)
            call_sizes.append(int(sum(S[q] for q in segs)))
    assert max(call_sizes) <= 16000, call_sizes
    off = np.zeros(nseg + 1, np.int64)
    tot = 0
    seg_off = {}
    for q in seg_order:
        seg_off[q] = tot
        tot += int(S[q])
    total = tot

    idxw_l, slot_l = [], []
    for c in range(NCORE):
        s, d, segid = per_core[c]
        idx16 = np.zeros(total, np.int16)
        slot = np.full(total, 999.0, np.float32)
        seg_start = np.searchsorted(segid, np.arange(nseg))
        base = np.array([seg_off[q] for q in range(nseg)], np.int64)
        pos = base[segid] + (np.arange(len(s)) - seg_start[segid])
        idx16[pos] = (s % CHUNK).astype(np.int16)
        slot[pos] = (d & 255).astype(np.float32)
        idxw_l.append(np.ascontiguousarray(
            np.tile(idx16.reshape(-1, 16).T, (8, 1))))
        slot_l.append(np.ascontiguousarray(slot.reshape(-1, P).T))

    return {
        "S": S,
        "call_sizes": call_sizes,
        "idxw": idxw_l,
        "slot": slot_l,
        "dinv": dinv,
    }


# --------------------------------------------------------- bass builders
def _bass_mods():
    import concourse.bass as bass
    import concourse.bacc as bacc
    import concourse.tile as tile
    import concourse.mybir as mybir
    from concourse import library_config
    from concourse.masks import make_identity

    return bass, bacc, tile, mybir, library_config, make_identity


def _build_k1():
    """From xT (bf16) produce: ut rows [padn, H] bf16 (gather table
    shard), utT [H, padn] fp32 (self-loop term, already dinv-scaled)."""
    bass, bacc, tile, mybir, libcfg, make_identity = _bass_mods()
    ncn, padn, nwin, npair, nchunk, groups = _derived()
    f32, bf16 = mybir.dt.float32, mybir.dt.bfloat16

    nc = bacc.Bacc("TRN2", target_bir_lowering=False, debug=False,
                   num_devices=NCORE)
    xT = nc.dram_tensor("xT", [F_IN, padn], f32, kind="ExternalInput").ap()
    w1 = nc.dram_tensor("w1", [F_IN, H], f32, kind="ExternalInput").ap()
    dTd = nc.dram_tensor("dinvT", [H, padn], f32, kind="ExternalInput").ap()
    utd = nc.dram_tensor("ut", [padn, P], bf16, kind="ExternalOutput").ap()
    utTd = nc.dram_tensor("utT", [H, padn], f32, kind="ExternalOutput").ap()

    kf = F_IN // P          # 2
    SW = 4                  # windows per stripe (psum bank = 512 fp32)
    stripes = []
    w0 = 0
    while w0 < nwin:
        stripes.append((w0, min(SW, nwin - w0)))
        w0 += SW

    with tile.TileContext(nc) as tc:
        with (
            tc.tile_pool(name="const", bufs=1) as constp,
            tc.tile_pool(name="xin", bufs=3) as xp,
            tc.tile_pool(name="ps", bufs=2, space="PSUM") as psump,
            tc.tile_pool(name="wk", bufs=3) as wp,
        ):
            w1_s = constp.tile([P, kf * H], f32)
            for k in range(kf):
                nc.sync.dma_start(w1_s[:, k * H:(k + 1) * H],
                                  w1[k * P:(k + 1) * P, :])
            identH = constp.tile([H, H], bf16)
            make_identity(nc, identH[:])

            for (ws, sw) in stripes:
                c0 = ws * P
                SC = sw * P
                up = psump.tile([H, SW * P], f32, tag="up", bufs=2)
                for k in range(kf):
                    xt = xp.tile([P, SW * P], f32, tag="xt")
                    nc.sync.dma_start(xt[:, :SC], xT[k * P:(k + 1) * P,
                                                     c0:c0 + SC])
                    nc.tensor.matmul(up[:, :SC],
                                     lhsT=w1_s[:, k * H:(k + 1) * H],
                                     rhs=xt[:, :SC], start=(k == 0),
                                     stop=(k == kf - 1))
                dT = xp.tile([H, SW * P], f32, tag="dT")
                nc.sync.dma_start(dT[:, :SC], dTd[:, c0:c0 + SC])
                uT = wp.tile([H, SW * P], f32, tag="uT")
                nc.vector.tensor_tensor(uT[:, :SC], up[:, :SC], dT[:, :SC],
                                        op=mybir.AluOpType.mult)
                nc.sync.dma_start(utTd[:, c0:c0 + SC], uT[:, :SC])
                uTb = wp.tile([H, SW * P], bf16, tag="uTb")
                nc.vector.tensor_copy(uTb[:, :SC], uT[:, :SC])
                uTr = wp.tile([H, SW * P], f32, tag="uTr")
                nc.vector.tensor_tensor(uTr[:, :SC], uT[:, :SC],
                                        uTb[:, :SC],
                                        op=mybir.AluOpType.subtract)
                uTrb = wp.tile([H, SW * P], bf16, tag="uTrb")
                nc.vector.tensor_copy(uTrb[:, :SC], uTr[:, :SC])
                stage = wp.tile([P, SW, P], bf16, tag="stage")
                for w in range(sw):
                    tp = psump.tile([P, H], bf16, tag="tp", bufs=2)
                    nc.tensor.transpose(tp[:], uTb[:, w * P:(w + 1) * P],
                                        identH[:])
                    nc.vector.tensor_copy(stage[:, w, 0:H], tp[:])
                    tr = psump.tile([P, H], bf16, tag="tr", bufs=2)
                    nc.tensor.transpose(tr[:], uTrb[:, w * P:(w + 1) * P],
                                        identH[:])
                    nc.vector.tensor_copy(stage[:, w, H:P], tr[:])
                nc.sync.dma_start(
                    utd[c0:c0 + SC, :].rearrange("(b a) h -> a b h", b=sw),
                    stage[:, :sw, :])
    nc.compile()
    return nc


def _agg(nc, tc, mybir, pools, table, idx_s, slot_s, iota_s, S, call_sizes,
         feats, drain_fn, lh_slices=None):
    """Batched gather + transposed one-hot scatter.

    For each (group, chunk) call: one dma_gather of call_sizes[i]
    descriptors; per 128-desc block one bf16 matmul accumulating into
    the pair's PSUM tile [feats, 256].  drain_fn(p, ps) after a pair's
    last chunk."""
    f32, bf16 = mybir.dt.float32, mybir.dt.bfloat16
    ncn, padn, nwin, npair, nchunk, groups = _derived()
    if lh_slices is None:
        lh_slices = [(0, feats)]
    nsl = len(lh_slices)
    gatp, selp, psump = pools
    off16 = 0
    mmcol = 0
    ci = 0
    accp = psump.parent_pool if False else None
    for g in groups:
        acc = gatp.tile([feats, GPAIR, 2 * P], f32, tag="acc", bufs=1,
                        name="acc")
        for ch in range(nchunk):
            sz = call_sizes[ci]
            ci += 1
            gat = gatp.tile([P, sz // P, P], bf16, tag="gat", name="gat")
            nc.gpsimd.dma_gather(
                gat[:],
                table[ch * CHUNK:min(N, (ch + 1) * CHUNK), :],
                idx_s[:, off16:off16 + sz // 16],
                sz, sz, P, elem_step=P, single_packet=False,
                queue_num=ci % 2,
            )
            off16 += sz // 16
            bb = 0
            for gi, p in enumerate(g):
                nb = int(S[p * nchunk + ch]) // P
                nmm = nb * nsl
                pseg = psump.tile([feats, 2 * P], f32, tag="pseg", bufs=4,
                                  name="pseg")
                done = 0
                for _ in range(nb):
                    sel = selp.tile([P, 2 * P], bf16, tag="sel", name="sel")
                    nc.vector.tensor_tensor(
                        out=sel[:],
                        in0=slot_s[:, mmcol:mmcol + 1].to_broadcast(
                            [P, 2 * P]),
                        in1=iota_s[:],
                        op=mybir.AluOpType.is_equal,
                    )
                    for (a, b) in lh_slices:
                        nc.tensor.matmul(
                            pseg[:], lhsT=gat[:, bb, a:b], rhs=sel[:],
                            start=(done == 0), stop=(done == nmm - 1),
                        )
                        done += 1
                    mmcol += 1
                    bb += 1
                if ch == 0:
                    nc.vector.tensor_copy(acc[:, gi, :], pseg[:])
                else:
                    nc.vector.tensor_tensor(acc[:, gi, :], acc[:, gi, :],
                                            pseg[:],
                                            op=mybir.AluOpType.add)
        for p_i, p in enumerate(g):
            drain_fn(p, acc[:, p_i, :])


def _build_k2(idx_cols, slot_cols, S, call_sizes):
    """Aggregate ut -> htT -> gt rows (bf16) + gtT (fp32)."""
    bass, bacc, tile, mybir, libcfg, make_identity = _bass_mods()
    ncn, padn, nwin, npair, nchunk, groups = _derived()
    f32, bf16 = mybir.dt.float32, mybir.dt.bfloat16

    nc = bacc.Bacc("TRN2", target_bir_lowering=False, debug=False,
                   num_devices=NCORE, dynamic_dma_scratch_size=49152,
                   num_swdge_queues=2)
    table = nc.dram_tensor("table", [N, P], bf16, kind="ExternalInput").ap()
    idxd = nc.dram_tensor("idx", [P, idx_cols], mybir.dt.int16,
                          kind="ExternalInput").ap()
    slotd = nc.dram_tensor("slot", [P, slot_cols], bf16,
                           kind="ExternalInput").ap()
    iotad = nc.dram_tensor("iota", [P, 2 * P], bf16,
                           kind="ExternalInput").ap()
    utTd = nc.dram_tensor("utT", [H, padn], f32, kind="ExternalInput").ap()
    dT64d = nc.dram_tensor("dinvT64", [H, padn], f32,
                           kind="ExternalInput").ap()
    b1d = nc.dram_tensor("b1col", [H, 1], f32, kind="ExternalInput").ap()
    w2d = nc.dram_tensor("w2", [H, C], bf16, kind="ExternalInput").ap()
    gtd = nc.dram_tensor("gt", [padn, C], bf16, kind="ExternalOutput").ap()
    gtTd = nc.dram_tensor("gtT", [C, padn], f32, kind="ExternalOutput").ap()

    with tile.TileContext(nc) as tc:
        with (
            tc.tile_pool(name="const", bufs=1) as constp,
            tc.tile_pool(name="gat", bufs=2) as gatp,
            tc.tile_pool(name="sel", bufs=4) as selp,
            tc.tile_pool(name="ps", bufs=1, space="PSUM") as psump,
            tc.tile_pool(name="dr", bufs=2) as drp,
            tc.tile_pool(name="st", bufs=2) as stp,
        ):
            with tc.tile_critical():
                nc.gpsimd.load_library(libcfg.mlp)
            idx_s = constp.tile([P, idx_cols], mybir.dt.int16)
            nc.sync.dma_start(idx_s[:], idxd[:, :])
            slot_s = constp.tile([P, slot_cols], bf16)
            nc.sync.dma_start(slot_s[:], slotd[:, :])
            iota_s = constp.tile([P, 2 * P], bf16)
            nc.sync.dma_start(iota_s[:], iotad[:, :])
            b1_s = constp.tile([H, 1], f32)
            nc.sync.dma_start(b1_s[:], b1d[:, :])
            w2_s = constp.tile([H, C], bf16)
            nc.sync.dma_start(w2_s[:], w2d[:, :])
            identC = constp.tile([C, C], bf16)
            make_identity(nc, identC[:])

            # per-pair drain: ps [H, 256] -> htT -> gt rows + gtT cols
            def drain(p, ps):
                c0 = p * 2 * P
                uT = drp.tile([H, 2 * P], f32, tag="uTsl", name="uTsl")
                nc.sync.dma_start(uT[:], utTd[:, c0:c0 + 2 * P])
                dTt = drp.tile([H, 2 * P], f32, tag="dTsl", name="dTsl")
                nc.sync.dma_start(dTt[:], dT64d[:, c0:c0 + 2 * P])
                dT = dTt[:]
                t1 = drp.tile([H, 2 * P], f32, tag="t1", name="t1")
                nc.vector.tensor_tensor(t1[:], ps, uT[:],
                                        op=mybir.AluOpType.add)
                nc.vector.tensor_tensor(t1[:], t1[:], dT,
                                        op=mybir.AluOpType.mult)
                nc.vector.tensor_scalar_add(t1[:], t1[:], b1_s[:])
                nc.vector.tensor_scalar_max(t1[:], t1[:], 0.0)
                hb = drp.tile([H, 2 * P], bf16, tag="hb", name="hb")
                nc.vector.tensor_tensor(hb[:], t1[:], dT,
                                        op=mybir.AluOpType.mult)
                gps = psump.tile([C, 2 * P], f32, tag="gps", bufs=1,
                                 name="gps")
                nc.tensor.matmul(gps[:], lhsT=w2_s[:], rhs=hb[:],
                                 start=True, stop=True)
                gT = drp.tile([C, 2 * P], f32, tag="gT", name="gT")
                nc.vector.tensor_copy(gT[:], gps[:])
                nc.sync.dma_start(gtTd[:, c0:c0 + 2 * P], gT[:])
                gTb = drp.tile([C, 2 * P], bf16, tag="gTb", name="gTb")
                nc.vector.tensor_copy(gTb[:], gT[:])
                stage = stp.tile([P, 2, C], bf16, tag="gstage", name="gstage")
                for w in range(2):
                    tp = psump.tile([P, C], bf16, tag="gtp", bufs=2,
                                    name="gtp")
                    nc.tensor.transpose(tp[:], gTb[:, w * P:(w + 1) * P],
                                        identC[:])
                    nc.vector.tensor_copy(stage[:, w, :], tp[:])
                nc.sync.dma_start(
                    gtd[c0:c0 + 2 * P, :].rearrange("(b a) h -> a b h", b=2),
                    stage[:])

            _agg(nc, tc, mybir, (gatp, selp, psump), table, idx_s, slot_s,
                 iota_s, S, call_sizes, H, drain,
                 lh_slices=[(0, H)])
    nc.compile()
    return nc


def _build_k3(idx_cols, slot_cols, S, call_sizes):
    """Aggregate gt -> log_softmax out rows [padn, C] fp32."""
    bass, bacc, tile, mybir, libcfg, make_identity = _bass_mods()
    ncn, padn, nwin, npair, nchunk, groups = _derived()
    f32, bf16 = mybir.dt.float32, mybir.dt.bfloat16

    nc = bacc.Bacc("TRN2", target_bir_lowering=False, debug=False,
                   num_devices=NCORE, dynamic_dma_scratch_size=49152,
                   num_swdge_queues=2)
    table = nc.dram_tensor("table", [N, P], bf16, kind="ExternalInput").ap()
    idxd = nc.dram_tensor("idx", [P, idx_cols], mybir.dt.int16,
                          kind="ExternalInput").ap()
    slotd = nc.dram_tensor("slot", [P, slot_cols], bf16,
                           kind="ExternalInput").ap()
    iotad = nc.dram_tensor("iota", [P, 2 * P], bf16,
                           kind="ExternalInput").ap()
    gtTd = nc.dram_tensor("gtT", [C, padn], f32, kind="ExternalInput").ap()
    dT16d = nc.dram_tensor("dinvT16", [C, padn], f32,
                           kind="ExternalInput").ap()
    b2d = nc.dram_tensor("b2rep", [P, C], f32, kind="ExternalInput").ap()
    outd = nc.dram_tensor("out", [padn, C], f32, kind="ExternalOutput").ap()

    with tile.TileContext(nc) as tc:
        with (
            tc.tile_pool(name="const", bufs=1) as constp,
            tc.tile_pool(name="gat", bufs=2) as gatp,
            tc.tile_pool(name="sel", bufs=4) as selp,
            tc.tile_pool(name="ps", bufs=1, space="PSUM") as psump,
            tc.tile_pool(name="dr", bufs=2) as drp,
            tc.tile_pool(name="st", bufs=2) as stp,
        ):
            with tc.tile_critical():
                nc.gpsimd.load_library(libcfg.mlp)
            idx_s = constp.tile([P, idx_cols], mybir.dt.int16)
            nc.sync.dma_start(idx_s[:], idxd[:, :])
            slot_s = constp.tile([P, slot_cols], bf16)
            nc.sync.dma_start(slot_s[:], slotd[:, :])
            iota_s = constp.tile([P, 2 * P], bf16)
            nc.sync.dma_start(iota_s[:], iotad[:, :])
            b2_s = constp.tile([P, C], f32)
            nc.sync.dma_start(b2_s[:], b2d[:, :])
            identC = constp.tile([C, C], bf16)
            make_identity(nc, identC[:])

            def drain(p, ps):
                c0 = p * 2 * P
                gT = drp.tile([C, 2 * P], f32, tag="gTsl", name="gTsl")
                nc.sync.dma_start(gT[:], gtTd[:, c0:c0 + 2 * P])
                dTt = drp.tile([C, 2 * P], f32, tag="dTsl", name="dTsl")
                nc.sync.dma_start(dTt[:], dT16d[:, c0:c0 + 2 * P])
                dT = dTt[:]
                t0 = drp.tile([C, 2 * P], f32, tag="t0", name="t0")
                nc.vector.tensor_tensor(t0[:], ps, gT[:],
                                        op=mybir.AluOpType.add)
                ob = drp.tile([C, 2 * P], bf16, tag="ob", name="ob")
                nc.vector.tensor_tensor(ob[:], t0[:], dT,
                                        op=mybir.AluOpType.mult)
                stage = stp.tile([P, 2, C], f32, tag="ostage", name="ostage")
                for w in range(2):
                    tp = psump.tile([P, C], bf16, tag="otp", bufs=2,
                                    name="otp")
                    nc.tensor.transpose(tp[:], ob[:, w * P:(w + 1) * P],
                                        identC[:])
                    z = drp.tile([P, C], f32, tag="z", name="z")
                    nc.vector.tensor_tensor(z[:], tp[:], b2_s[:],
                                            op=mybir.AluOpType.add)
                    negm = drp.tile([P, 1], f32, tag="negm", name="negm")
                    nc.vector.tensor_reduce(
                        negm[:], z[:], axis=mybir.AxisListType.X,
                        op=mybir.AluOpType.max, negate=True)
                    e = drp.tile([P, C], f32, tag="e", name="e")
                    sa = drp.tile([P, 1], f32, tag="sa", name="sa")
                    nc.scalar.activation(
                        e[:], z[:], mybir.ActivationFunctionType.Exp,
                        bias=negm[:], accum_out=sa[:])
                    lns = drp.tile([P, 1], f32, tag="lns", name="lns")
                    nc.scalar.activation(
                        lns[:], sa[:], mybir.ActivationFunctionType.Ln)
                    nc.vector.tensor_scalar(
                        out=stage[:, w, :], in0=z[:], scalar1=negm[:],
                        scalar2=lns[:], op0=mybir.AluOpType.add,
                        op1=mybir.AluOpType.subtract)
                nc.sync.dma_start(
                    outd[c0:c0 + 2 * P, :].rearrange("(b a) h -> a b h", b=2),
                    stage[:])

            _agg(nc, tc, mybir, (gatp, selp, psump), table, idx_s, slot_s,
                 iota_s, S, call_sizes, C, drain)
    nc.compile()
    return nc


def _run(nc, in_maps):
    if os.environ.get("BASS_GCN_SIM"):
        from concourse.bass_interp import MultiCoreSim

        sim = MultiCoreSim(nc, num_cores=NCORE, trace=False)
        for c in range(NCORE):
            for k, v in in_maps[c].items():
                sim.cores[c].tensor(k)[:] = v
        sim.simulate()
        outs = []
        for c in range(NCORE):
            names = [
                a.memorylocations[0].name
                for a in nc.m.functions[0].allocations
                if getattr(a, "kind", None) == "ExternalOutput"
            ]
            outs.append({n: np.array(sim.cores[c].tensor(n)) for n in names})
        return outs

    from concourse.bass_utils import run_bass_kernel_spmd

    trace = TRACE and _install_ntff_shim()
    res = run_bass_kernel_spmd(nc, in_maps, core_ids=list(range(NCORE)),
                               trace=trace)
    if res.exec_time_ns:
        LAST_EXEC_NS.append(res.exec_time_ns)
    return res.results


# ---------------------------------------------------------------- kernel
def kernel(x, edge_index, W1, b1, W2, b2):
    ncn, padn, nwin, npair, nchunk, groups = _derived()
    LAST_EXEC_NS.clear()

    x = np.asarray(x, np.float32)
    edge_index = np.asarray(edge_index)
    W1 = np.asarray(W1, np.float32)
    b1 = np.asarray(b1, np.float32)
    W2 = np.asarray(W2, np.float32)
    b2 = np.asarray(b2, np.float32)

    plan = _build_plan(edge_index)
    S, call_sizes, dinv = plan["S"], plan["call_sizes"], plan["dinv"]
    idx_cols = plan["idxw"][0].shape[1]
    slot_cols = plan["slot"][0].shape[1]

    iota2 = np.tile(np.arange(2 * P, dtype=np.float32)[None, :], (P, 1))
    b2rep = np.tile(b2[None, :], (P, 1)).astype(np.float32)

    # ---- K1
    nc1 = _build_k1()
    in1 = []
    for c in range(NCORE):
        xc = np.zeros((padn, F_IN), np.float32)
        xc[:ncn] = x[c * ncn:(c + 1) * ncn]
        dv = np.zeros(padn, np.float32)
        dv[:ncn] = dinv[c * ncn:(c + 1) * ncn]
        in1.append({
            "xT": np.ascontiguousarray(xc.T),
            "w1": W1,
            "dinvT": np.ascontiguousarray(
                np.broadcast_to(dv[None, :], (H, padn))),
        })
    r1 = _run(nc1, in1)
    table1 = np.ascontiguousarray(np.concatenate(
        [r1[c]["ut"][:ncn] for c in range(NCORE)], axis=0))

    # ---- K2
    nc2 = _build_k2(idx_cols, slot_cols, S, call_sizes)
    in2 = []
    for c in range(NCORE):
        dv = np.zeros(padn, np.float32)
        dv[:ncn] = dinv[c * ncn:(c + 1) * ncn]
        in2.append({
            "table": table1,
            "idx": plan["idxw"][c],
            "slot": plan["slot"][c].astype(ml_dtypes.bfloat16),
            "iota": iota2.astype(ml_dtypes.bfloat16),
            "utT": r1[c]["utT"],
            "dinvT64": np.ascontiguousarray(
                np.broadcast_to(dv[None, :], (H, padn))),
            "b1col": b1[:, None],
            "w2": W2.astype(ml_dtypes.bfloat16),
        })
    r2 = _run(nc2, in2)
    gt_full = np.concatenate([r2[c]["gt"][:ncn] for c in range(NCORE)],
                             axis=0)
    table2 = np.zeros((N, P), ml_dtypes.bfloat16)
    for rep in range(P // C):
        table2[:, rep * C:(rep + 1) * C] = gt_full

    # ---- K3
    nc3 = _build_k3(idx_cols, slot_cols, S, call_sizes)
    in3 = []
    for c in range(NCORE):
        dv = np.zeros(padn, np.float32)
        dv[:ncn] = dinv[c * ncn:(c + 1) * ncn]
        in3.append({
            "table": table2,
            "idx": plan["idxw"][c],
            "slot": plan["slot"][c].astype(ml_dtypes.bfloat16),
            "iota": iota2.astype(ml_dtypes.bfloat16),
            "gtT": r2[c]["gtT"],
            "dinvT16": np.ascontiguousarray(
                np.broadcast_to(dv[None, :], (C, padn))),
            "b2rep": b2rep,
        })
    r3 = _run(nc3, in3)
    global _dbg
    _dbg = {"r1": r1, "r2": r2, "r3": r3}
    out = np.concatenate([r3[c]["out"][:ncn] for c in range(NCORE)], axis=0)
    return np.ascontiguousarray(out.astype(np.float32))


# revision 16
# speedup vs baseline: 1.8081x; 1.1335x over previous
"""GCN (2-layer, symmetric-normalized, self-loops) on 8 TRN2 NeuronCores.

Math (reference):
    A_hat = D^-1/2 (A + I) D^-1/2        (deg over dst incl. self-loops)
    h1    = relu(A_hat @ (x @ W1) + b1)
    out   = log_softmax(A_hat @ h1 @ W2 + b2)

Decomposition (nodes sharded by dst range across 8 cores, 3 launches):
    K1: ut   = dinv * (x @ W1)                       [per-core shard]
        writes ut rows (bf16) for the K2 gather table and utT (fp32)
        for the dense self-loop term.
    K2: htT  = dinv * relu(dinv * (A @ ut + ut_self) + b1)
        gt   = htT @ W2   (so layer 2 aggregates 16-wide)
        writes gt rows (bf16) for the K3 gather table and gtT (fp32).
    K3: out  = log_softmax(dinv * (A @ gt + dinv*gt_self) + b2)

Aggregation per core: edges (no self-loops) grouped by (dst-256-pair,
src chunk) and batched into large dma_gather calls (~11K descriptors)
to amortize the ~10.5us fixed cost per SWDGE gather call.  Scatter into
transposed PSUM accumulators [F, 256] via one bf16 matmul per 128-edge
block: lhsT = gathered rows (stationary), rhs = one-hot slot matrix.
Tables are bf16 with rows duplicated to 256B to satisfy the gather's
minimum element size.
"""

import os
import sys
import types

import numpy as np
import ml_dtypes

# ---------------------------------------------------------------- sizes
N = 100000
E = 1600000
F_IN = 256
H = 64
C = 16
NCORE = 8
P = 128
CHUNK = 25000            # int16-addressable source chunk
GPAIR = 4                # dst-pairs per gather call group
TRACE = bool(int(os.environ.get("BASS_GCN_TRACE", "0")))
SMALL = bool(int(os.environ.get("BASS_GCN_SMALL", "0")))
if SMALL:
    N, E, CHUNK, GPAIR = 12800, 96000, 3200, 3

LAST_EXEC_NS = []
_dbg = None


def _derived():
    ncn = N // NCORE
    padn = ((ncn + 255) // 256) * 256
    nwin = padn // P
    npair = nwin // 2
    nchunk = (N + CHUNK - 1) // CHUNK
    groups = [list(range(g, min(g + GPAIR, npair)))
              for g in range(0, npair, GPAIR)]
    return ncn, padn, nwin, npair, nchunk, groups


# ------------------------------------------------------- ntff shim (opt)
def _install_ntff_shim():
    try:
        if "antenv.axon_hooks" in sys.modules:
            return True
        sys.path.insert(0, "/root/.axon_site/trn_agent_boot")
        from trn_boot import _ntff_profile_via_ctypes  # type: ignore

        mod = types.ModuleType("antenv.axon_hooks")
        holder = [None]
        mod.set_axon_ntff_profile_hook = lambda h: holder.__setitem__(0, h)
        mod.get_axon_ntff_profile_hook = lambda: holder[0]
        sys.modules["antenv.axon_hooks"] = mod
        import antenv

        antenv.axon_hooks = mod
        mod.set_axon_ntff_profile_hook(
            _ntff_profile_via_ctypes("/opt/axon/libaxon_pjrt.so")
        )
        return True
    except Exception:
        return False


# ------------------------------------------------------------ host plan
def _build_plan(edge_index):
    """Edge index structures (functions of edge_index only).

    Per core: edges (no self-loops) with dst in the core's range are
    grouped by (pair = dst>>8, chunk = src//CHUNK) and laid out call by
    call: for each (group of GPAIR pairs, chunk), the member (pair,
    chunk) segments are padded to multiples of 128 descriptors and
    concatenated.  Segment sizes are made uniform across cores (max)
    so one SPMD program fits all.
    """
    ncn, padn, nwin, npair, nchunk, groups = _derived()
    nseg = npair * nchunk

    src_a = np.asarray(edge_index[0], np.int64)
    dst_a = np.asarray(edge_index[1], np.int64)
    deg = np.bincount(dst_a, minlength=N).astype(np.float64) + 1.0
    dinv = (1.0 / np.sqrt(deg)).astype(np.float32)

    per_core = []
    cnts = np.zeros((NCORE, nseg), np.int64)
    for c in range(NCORE):
        lo = c * ncn
        m = (dst_a >= lo) & (dst_a < lo + ncn)
        s = src_a[m]
        d = dst_a[m] - lo
        pair = d >> 8
        chunk = s // CHUNK
        segid = pair * nchunk + chunk
        order = np.argsort(segid, kind="stable")
        s, d, segid = s[order], d[order], segid[order]
        cnts[c] = np.bincount(segid, minlength=nseg)
        per_core.append((s, d, segid))

    S = 128 * ((cnts.max(axis=0) + 127) // 128)          # [nseg] uniform
    S = np.maximum(S, 128)
    # call layout: for each (group g, chunk ch): segments (p in g, ch)
    seg_order = []
    call_sizes = []
    for g in groups:
        for ch in range(nchunk):
            segs = [p * nchunk + ch for p in g]
            seg_order.extend(segs)
            call_sizes.append(int(sum(S[q] for q in segs)))
    assert max(call_sizes) <= 16000, call_sizes
    off = np.zeros(nseg + 1, np.int64)
    tot = 0
    seg_off = {}
    for q in seg_order:
        seg_off[q] = tot
        tot += int(S[q])
    total = tot

    idxw_l, slot_l = [], []
    for c in range(NCORE):
        s, d, segid = per_core[c]
        idx16 = np.zeros(total, np.int16)
        slot = np.full(total, 999.0, np.float32)
        seg_start = np.searchsorted(segid, np.arange(nseg))
        base = np.array([seg_off[q] for q in range(nseg)], np.int64)
        pos = base[segid] + (np.arange(len(s)) - seg_start[segid])
        idx16[pos] = (s % CHUNK).astype(np.int16)
        slot[pos] = (d & 255).astype(np.float32)
        idxw_l.append(np.ascontiguousarray(
            np.tile(idx16.reshape(-1, 16).T, (8, 1))))
        slot_l.append(np.ascontiguousarray(slot.reshape(-1, P).T))

    return {
        "S": S,
        "call_sizes": call_sizes,
        "idxw": idxw_l,
        "slot": slot_l,
        "dinv": dinv,
    }


# --------------------------------------------------------- bass builders
def _bass_mods():
    import concourse.bass as bass
    import concourse.bacc as bacc
    import concourse.tile as tile
    import concourse.mybir as mybir
    from concourse import library_config
    from concourse.masks import make_identity

    return bass, bacc, tile, mybir, library_config, make_identity


def _build_k1():
    """From xT (bf16) produce: ut rows [padn, H] bf16 (gather table
    shard), utT [H, padn] fp32 (self-loop term, already dinv-scaled)."""
    bass, bacc, tile, mybir, libcfg, make_identity = _bass_mods()
    ncn, padn, nwin, npair, nchunk, groups = _derived()
    f32, bf16 = mybir.dt.float32, mybir.dt.bfloat16

    nc = bacc.Bacc("TRN2", target_bir_lowering=False, debug=False,
                   num_devices=NCORE)
    xT = nc.dram_tensor("xT", [F_IN, padn], f32, kind="ExternalInput").ap()
    w1 = nc.dram_tensor("w1", [F_IN, H], f32, kind="ExternalInput").ap()
    dTd = nc.dram_tensor("dinvT", [H, padn], f32, kind="ExternalInput").ap()
    utd = nc.dram_tensor("ut", [padn, P], bf16, kind="ExternalOutput").ap()
    utTd = nc.dram_tensor("utT", [H, padn], f32, kind="ExternalOutput").ap()

    kf = F_IN // P          # 2
    SW = 4                  # windows per stripe (psum bank = 512 fp32)
    stripes = []
    w0 = 0
    while w0 < nwin:
        stripes.append((w0, min(SW, nwin - w0)))
        w0 += SW

    with tile.TileContext(nc) as tc:
        with (
            tc.tile_pool(name="const", bufs=1) as constp,
            tc.tile_pool(name="xin", bufs=3) as xp,
            tc.tile_pool(name="ps", bufs=2, space="PSUM") as psump,
            tc.tile_pool(name="wk", bufs=3) as wp,
        ):
            w1_s = constp.tile([P, kf * H], f32)
            for k in range(kf):
                nc.sync.dma_start(w1_s[:, k * H:(k + 1) * H],
                                  w1[k * P:(k + 1) * P, :])
            identH = constp.tile([H, H], bf16)
            make_identity(nc, identH[:])

            for (ws, sw) in stripes:
                c0 = ws * P
                SC = sw * P
                up = psump.tile([H, SW * P], f32, tag="up", bufs=2)
                for k in range(kf):
                    xt = xp.tile([P, SW * P], f32, tag="xt")
                    nc.sync.dma_start(xt[:, :SC], xT[k * P:(k + 1) * P,
                                                     c0:c0 + SC])
                    nc.tensor.matmul(up[:, :SC],
                                     lhsT=w1_s[:, k * H:(k + 1) * H],
                                     rhs=xt[:, :SC], start=(k == 0),
                                     stop=(k == kf - 1))
                dT = xp.tile([H, SW * P], f32, tag="dT")
                nc.sync.dma_start(dT[:, :SC], dTd[:, c0:c0 + SC])
                uT = wp.tile([H, SW * P], f32, tag="uT")
                nc.vector.tensor_tensor(uT[:, :SC], up[:, :SC], dT[:, :SC],
                                        op=mybir.AluOpType.mult)
                nc.sync.dma_start(utTd[:, c0:c0 + SC], uT[:, :SC])
                uTb = wp.tile([H, SW * P], bf16, tag="uTb")
                nc.vector.tensor_copy(uTb[:, :SC], uT[:, :SC])
                uTr = wp.tile([H, SW * P], f32, tag="uTr")
                nc.vector.tensor_tensor(uTr[:, :SC], uT[:, :SC],
                                        uTb[:, :SC],
                                        op=mybir.AluOpType.subtract)
                uTrb = wp.tile([H, SW * P], bf16, tag="uTrb")
                nc.vector.tensor_copy(uTrb[:, :SC], uTr[:, :SC])
                stage = wp.tile([P, SW, P], bf16, tag="stage")
                for w in range(sw):
                    tp = psump.tile([P, H], bf16, tag="tp", bufs=2)
                    nc.tensor.transpose(tp[:], uTb[:, w * P:(w + 1) * P],
                                        identH[:])
                    nc.vector.tensor_copy(stage[:, w, 0:H], tp[:])
                    tr = psump.tile([P, H], bf16, tag="tr", bufs=2)
                    nc.tensor.transpose(tr[:], uTrb[:, w * P:(w + 1) * P],
                                        identH[:])
                    nc.vector.tensor_copy(stage[:, w, H:P], tr[:])
                nc.sync.dma_start(
                    utd[c0:c0 + SC, :].rearrange("(b a) h -> a b h", b=sw),
                    stage[:, :sw, :])
    nc.compile()
    return nc


def _agg(nc, tc, mybir, pools, table, idx_s, slot_s, iota_s, S, call_sizes,
         feats, drain_fn, lh_slices=None):
    """Batched gather + transposed one-hot scatter.

    For each (group, chunk) call: one dma_gather of call_sizes[i]
    descriptors; per 128-desc block one bf16 matmul accumulating into
    the pair's PSUM tile [feats, 256].  drain_fn(p, ps) after a pair's
    last chunk."""
    f32, bf16 = mybir.dt.float32, mybir.dt.bfloat16
    ncn, padn, nwin, npair, nchunk, groups = _derived()
    if lh_slices is None:
        lh_slices = [(0, feats)]
    nsl = len(lh_slices)
    gatp, selp, psump = pools
    off16 = 0
    mmcol = 0
    ci = 0
    accp = psump.parent_pool if False else None
    for g in groups:
        acc = gatp.tile([feats, GPAIR, 2 * P], f32, tag="acc", bufs=1,
                        name="acc")
        for ch in range(nchunk):
            sz = call_sizes[ci]
            ci += 1
            gat = gatp.tile([P, sz // P, P], bf16, tag="gat", name="gat")
            nc.gpsimd.dma_gather(
                gat[:],
                table[ch * CHUNK:min(N, (ch + 1) * CHUNK), :],
                idx_s[:, off16:off16 + sz // 16],
                sz, sz, P, elem_step=P, single_packet=False,
                queue_num=ci % 4,
            )
            off16 += sz // 16
            bb = 0
            for gi, p in enumerate(g):
                nb = int(S[p * nchunk + ch]) // P
                nmm = nb * nsl
                pseg = psump.tile([feats, 2 * P], f32, tag="pseg", bufs=4,
                                  name="pseg")
                done = 0
                for _ in range(nb):
                    sel = selp.tile([P, 2 * P], bf16, tag="sel", name="sel")
                    nc.vector.tensor_tensor(
                        out=sel[:],
                        in0=slot_s[:, mmcol:mmcol + 1].to_broadcast(
                            [P, 2 * P]),
                        in1=iota_s[:],
                        op=mybir.AluOpType.is_equal,
                    )
                    for (a, b) in lh_slices:
                        nc.tensor.matmul(
                            pseg[:], lhsT=gat[:, bb, a:b], rhs=sel[:],
                            start=(done == 0), stop=(done == nmm - 1),
                        )
                        done += 1
                    mmcol += 1
                    bb += 1
                if ch == 0:
                    nc.vector.tensor_copy(acc[:, gi, :], pseg[:])
                else:
                    nc.vector.tensor_tensor(acc[:, gi, :], acc[:, gi, :],
                                            pseg[:],
                                            op=mybir.AluOpType.add)
        for p_i, p in enumerate(g):
            drain_fn(p, acc[:, p_i, :])


def _build_k2(idx_cols, slot_cols, S, call_sizes):
    """Aggregate ut -> htT -> gt rows (bf16) + gtT (fp32)."""
    bass, bacc, tile, mybir, libcfg, make_identity = _bass_mods()
    ncn, padn, nwin, npair, nchunk, groups = _derived()
    f32, bf16 = mybir.dt.float32, mybir.dt.bfloat16

    nc = bacc.Bacc("TRN2", target_bir_lowering=False, debug=False,
                   num_devices=NCORE, dynamic_dma_scratch_size=49152,
                   num_swdge_queues=4)
    table = nc.dram_tensor("table", [N, P], bf16, kind="ExternalInput").ap()
    idxd = nc.dram_tensor("idx", [P, idx_cols], mybir.dt.int16,
                          kind="ExternalInput").ap()
    slotd = nc.dram_tensor("slot", [P, slot_cols], bf16,
                           kind="ExternalInput").ap()
    iotad = nc.dram_tensor("iota", [P, 2 * P], bf16,
                           kind="ExternalInput").ap()
    utTd = nc.dram_tensor("utT", [H, padn], f32, kind="ExternalInput").ap()
    dT64d = nc.dram_tensor("dinvT64", [H, padn], f32,
                           kind="ExternalInput").ap()
    b1d = nc.dram_tensor("b1col", [H, 1], f32, kind="ExternalInput").ap()
    w2d = nc.dram_tensor("w2", [H, C], bf16, kind="ExternalInput").ap()
    gtd = nc.dram_tensor("gt", [padn, C], bf16, kind="ExternalOutput").ap()
    gtTd = nc.dram_tensor("gtT", [C, padn], f32, kind="ExternalOutput").ap()

    with tile.TileContext(nc) as tc:
        with (
            tc.tile_pool(name="const", bufs=1) as constp,
            tc.tile_pool(name="gat", bufs=3) as gatp,
            tc.tile_pool(name="sel", bufs=4) as selp,
            tc.tile_pool(name="ps", bufs=1, space="PSUM") as psump,
            tc.tile_pool(name="dr", bufs=2) as drp,
            tc.tile_pool(name="st", bufs=2) as stp,
        ):
            with tc.tile_critical():
                nc.gpsimd.load_library(libcfg.mlp)
            idx_s = constp.tile([P, idx_cols], mybir.dt.int16)
            nc.sync.dma_start(idx_s[:], idxd[:, :])
            slot_s = constp.tile([P, slot_cols], bf16)
            nc.sync.dma_start(slot_s[:], slotd[:, :])
            iota_s = constp.tile([P, 2 * P], bf16)
            nc.sync.dma_start(iota_s[:], iotad[:, :])
            b1_s = constp.tile([H, 1], f32)
            nc.sync.dma_start(b1_s[:], b1d[:, :])
            w2_s = constp.tile([H, C], bf16)
            nc.sync.dma_start(w2_s[:], w2d[:, :])
            identC = constp.tile([C, C], bf16)
            make_identity(nc, identC[:])

            # per-pair drain: ps [H, 256] -> htT -> gt rows + gtT cols
            def drain(p, ps):
                c0 = p * 2 * P
                uT = drp.tile([H, 2 * P], f32, tag="uTsl", name="uTsl")
                nc.sync.dma_start(uT[:], utTd[:, c0:c0 + 2 * P])
                dTt = drp.tile([H, 2 * P], f32, tag="dTsl", name="dTsl")
                nc.sync.dma_start(dTt[:], dT64d[:, c0:c0 + 2 * P])
                dT = dTt[:]
                t1 = drp.tile([H, 2 * P], f32, tag="t1", name="t1")
                nc.vector.tensor_tensor(t1[:], ps, uT[:],
                                        op=mybir.AluOpType.add)
                nc.vector.tensor_tensor(t1[:], t1[:], dT,
                                        op=mybir.AluOpType.mult)
                nc.vector.tensor_scalar_add(t1[:], t1[:], b1_s[:])
                nc.vector.tensor_scalar_max(t1[:], t1[:], 0.0)
                hb = drp.tile([H, 2 * P], bf16, tag="hb", name="hb")
                nc.vector.tensor_tensor(hb[:], t1[:], dT,
                                        op=mybir.AluOpType.mult)
                gps = psump.tile([C, 2 * P], f32, tag="gps", bufs=1,
                                 name="gps")
                nc.tensor.matmul(gps[:], lhsT=w2_s[:], rhs=hb[:],
                                 start=True, stop=True)
                gT = drp.tile([C, 2 * P], f32, tag="gT", name="gT")
                nc.vector.tensor_copy(gT[:], gps[:])
                nc.sync.dma_start(gtTd[:, c0:c0 + 2 * P], gT[:])
                gTb = drp.tile([C, 2 * P], bf16, tag="gTb", name="gTb")
                nc.vector.tensor_copy(gTb[:], gT[:])
                stage = stp.tile([P, 2, C], bf16, tag="gstage", name="gstage")
                for w in range(2):
                    tp = psump.tile([P, C], bf16, tag="gtp", bufs=2,
                                    name="gtp")
                    nc.tensor.transpose(tp[:], gTb[:, w * P:(w + 1) * P],
                                        identC[:])
                    nc.vector.tensor_copy(stage[:, w, :], tp[:])
                nc.sync.dma_start(
                    gtd[c0:c0 + 2 * P, :].rearrange("(b a) h -> a b h", b=2),
                    stage[:])

            _agg(nc, tc, mybir, (gatp, selp, psump), table, idx_s, slot_s,
                 iota_s, S, call_sizes, H, drain,
                 lh_slices=[(0, H)])
    nc.compile()
    return nc


def _build_k3(idx_cols, slot_cols, S, call_sizes):
    """Aggregate gt -> log_softmax out rows [padn, C] fp32."""
    bass, bacc, tile, mybir, libcfg, make_identity = _bass_mods()
    ncn, padn, nwin, npair, nchunk, groups = _derived()
    f32, bf16 = mybir.dt.float32, mybir.dt.bfloat16

    nc = bacc.Bacc("TRN2", target_bir_lowering=False, debug=False,
                   num_devices=NCORE, dynamic_dma_scratch_size=49152,
                   num_swdge_queues=4)
    table = nc.dram_tensor("table", [N, P], bf16, kind="ExternalInput").ap()
    idxd = nc.dram_tensor("idx", [P, idx_cols], mybir.dt.int16,
                          kind="ExternalInput").ap()
    slotd = nc.dram_tensor("slot", [P, slot_cols], bf16,
                           kind="ExternalInput").ap()
    iotad = nc.dram_tensor("iota", [P, 2 * P], bf16,
                           kind="ExternalInput").ap()
    gtTd = nc.dram_tensor("gtT", [C, padn], f32, kind="ExternalInput").ap()
    dT16d = nc.dram_tensor("dinvT16", [C, padn], f32,
                           kind="ExternalInput").ap()
    b2d = nc.dram_tensor("b2rep", [P, C], f32, kind="ExternalInput").ap()
    outd = nc.dram_tensor("out", [padn, C], f32, kind="ExternalOutput").ap()

    with tile.TileContext(nc) as tc:
        with (
            tc.tile_pool(name="const", bufs=1) as constp,
            tc.tile_pool(name="gat", bufs=3) as gatp,
            tc.tile_pool(name="sel", bufs=4) as selp,
            tc.tile_pool(name="ps", bufs=1, space="PSUM") as psump,
            tc.tile_pool(name="dr", bufs=2) as drp,
            tc.tile_pool(name="st", bufs=2) as stp,
        ):
            with tc.tile_critical():
                nc.gpsimd.load_library(libcfg.mlp)
            idx_s = constp.tile([P, idx_cols], mybir.dt.int16)
            nc.sync.dma_start(idx_s[:], idxd[:, :])
            slot_s = constp.tile([P, slot_cols], bf16)
            nc.sync.dma_start(slot_s[:], slotd[:, :])
            iota_s = constp.tile([P, 2 * P], bf16)
            nc.sync.dma_start(iota_s[:], iotad[:, :])
            b2_s = constp.tile([P, C], f32)
            nc.sync.dma_start(b2_s[:], b2d[:, :])
            identC = constp.tile([C, C], bf16)
            make_identity(nc, identC[:])

            def drain(p, ps):
                c0 = p * 2 * P
                gT = drp.tile([C, 2 * P], f32, tag="gTsl", name="gTsl")
                nc.sync.dma_start(gT[:], gtTd[:, c0:c0 + 2 * P])
                dTt = drp.tile([C, 2 * P], f32, tag="dTsl", name="dTsl")
                nc.sync.dma_start(dTt[:], dT16d[:, c0:c0 + 2 * P])
                dT = dTt[:]
                t0 = drp.tile([C, 2 * P], f32, tag="t0", name="t0")
                nc.vector.tensor_tensor(t0[:], ps, gT[:],
                                        op=mybir.AluOpType.add)
                ob = drp.tile([C, 2 * P], bf16, tag="ob", name="ob")
                nc.vector.tensor_tensor(ob[:], t0[:], dT,
                                        op=mybir.AluOpType.mult)
                stage = stp.tile([P, 2, C], f32, tag="ostage", name="ostage")
                for w in range(2):
                    tp = psump.tile([P, C], bf16, tag="otp", bufs=2,
                                    name="otp")
                    nc.tensor.transpose(tp[:], ob[:, w * P:(w + 1) * P],
                                        identC[:])
                    z = drp.tile([P, C], f32, tag="z", name="z")
                    nc.vector.tensor_tensor(z[:], tp[:], b2_s[:],
                                            op=mybir.AluOpType.add)
                    negm = drp.tile([P, 1], f32, tag="negm", name="negm")
                    nc.vector.tensor_reduce(
                        negm[:], z[:], axis=mybir.AxisListType.X,
                        op=mybir.AluOpType.max, negate=True)
                    e = drp.tile([P, C], f32, tag="e", name="e")
                    sa = drp.tile([P, 1], f32, tag="sa", name="sa")
                    nc.scalar.activation(
                        e[:], z[:], mybir.ActivationFunctionType.Exp,
                        bias=negm[:], accum_out=sa[:])
                    lns = drp.tile([P, 1], f32, tag="lns", name="lns")
                    nc.scalar.activation(
                        lns[:], sa[:], mybir.ActivationFunctionType.Ln)
                    nc.vector.tensor_scalar(
                        out=stage[:, w, :], in0=z[:], scalar1=negm[:],
                        scalar2=lns[:], op0=mybir.AluOpType.add,
                        op1=mybir.AluOpType.subtract)
                nc.sync.dma_start(
                    outd[c0:c0 + 2 * P, :].rearrange("(b a) h -> a b h", b=2),
                    stage[:])

            _agg(nc, tc, mybir, (gatp, selp, psump), table, idx_s, slot_s,
                 iota_s, S, call_sizes, C, drain)
    nc.compile()
    return nc


def _run(nc, in_maps):
    if os.environ.get("BASS_GCN_SIM"):
        from concourse.bass_interp import MultiCoreSim

        sim = MultiCoreSim(nc, num_cores=NCORE, trace=False)
        for c in range(NCORE):
            for k, v in in_maps[c].items():
                sim.cores[c].tensor(k)[:] = v
        sim.simulate()
        outs = []
        for c in range(NCORE):
            names = [
                a.memorylocations[0].name
                for a in nc.m.functions[0].allocations
                if getattr(a, "kind", None) == "ExternalOutput"
            ]
            outs.append({n: np.array(sim.cores[c].tensor(n)) for n in names})
        return outs

    from concourse.bass_utils import run_bass_kernel_spmd

    trace = TRACE and _install_ntff_shim()
    res = run_bass_kernel_spmd(nc, in_maps, core_ids=list(range(NCORE)),
                               trace=trace)
    if res.exec_time_ns:
        LAST_EXEC_NS.append(res.exec_time_ns)
    return res.results


# ---------------------------------------------------------------- kernel
def kernel(x, edge_index, W1, b1, W2, b2):
    ncn, padn, nwin, npair, nchunk, groups = _derived()
    LAST_EXEC_NS.clear()

    x = np.asarray(x, np.float32)
    edge_index = np.asarray(edge_index)
    W1 = np.asarray(W1, np.float32)
    b1 = np.asarray(b1, np.float32)
    W2 = np.asarray(W2, np.float32)
    b2 = np.asarray(b2, np.float32)

    plan = _build_plan(edge_index)
    S, call_sizes, dinv = plan["S"], plan["call_sizes"], plan["dinv"]
    idx_cols = plan["idxw"][0].shape[1]
    slot_cols = plan["slot"][0].shape[1]

    iota2 = np.tile(np.arange(2 * P, dtype=np.float32)[None, :], (P, 1))
    b2rep = np.tile(b2[None, :], (P, 1)).astype(np.float32)

    # ---- K1
    nc1 = _build_k1()
    in1 = []
    for c in range(NCORE):
        xc = np.zeros((padn, F_IN), np.float32)
        xc[:ncn] = x[c * ncn:(c + 1) * ncn]
        dv = np.zeros(padn, np.float32)
        dv[:ncn] = dinv[c * ncn:(c + 1) * ncn]
        in1.append({
            "xT": np.ascontiguousarray(xc.T),
            "w1": W1,
            "dinvT": np.ascontiguousarray(
                np.broadcast_to(dv[None, :], (H, padn))),
        })
    r1 = _run(nc1, in1)
    table1 = np.ascontiguousarray(np.concatenate(
        [r1[c]["ut"][:ncn] for c in range(NCORE)], axis=0))

    # ---- K2
    nc2 = _build_k2(idx_cols, slot_cols, S, call_sizes)
    in2 = []
    for c in range(NCORE):
        dv = np.zeros(padn, np.float32)
        dv[:ncn] = dinv[c * ncn:(c + 1) * ncn]
        in2.append({
            "table": table1,
            "idx": plan["idxw"][c],
            "slot": plan["slot"][c].astype(ml_dtypes.bfloat16),
            "iota": iota2.astype(ml_dtypes.bfloat16),
            "utT": r1[c]["utT"],
            "dinvT64": np.ascontiguousarray(
                np.broadcast_to(dv[None, :], (H, padn))),
            "b1col": b1[:, None],
            "w2": W2.astype(ml_dtypes.bfloat16),
        })
    r2 = _run(nc2, in2)
    gt_full = np.concatenate([r2[c]["gt"][:ncn] for c in range(NCORE)],
                             axis=0)
    table2 = np.zeros((N, P), ml_dtypes.bfloat16)
    for rep in range(P // C):
        table2[:, rep * C:(rep + 1) * C] = gt_full

    # ---- K3
    nc3 = _build_k3(idx_cols, slot_cols, S, call_sizes)
    in3 = []
    for c in range(NCORE):
        dv = np.zeros(padn, np.float32)
        dv[:ncn] = dinv[c * ncn:(c + 1) * ncn]
        in3.append({
            "table": table2,
            "idx": plan["idxw"][c],
            "slot": plan["slot"][c].astype(ml_dtypes.bfloat16),
            "iota": iota2.astype(ml_dtypes.bfloat16),
            "gtT": r2[c]["gtT"],
            "dinvT16": np.ascontiguousarray(
                np.broadcast_to(dv[None, :], (C, padn))),
            "b2rep": b2rep,
        })
    r3 = _run(nc3, in3)
    global _dbg
    _dbg = {"r1": r1, "r2": r2, "r3": r3}
    out = np.concatenate([r3[c]["out"][:ncn] for c in range(NCORE)], axis=0)
    return np.ascontiguousarray(out.astype(np.float32))


# revision 18
# speedup vs baseline: 1.8938x; 1.0474x over previous
"""GCN (2-layer, symmetric-normalized, self-loops) on 8 TRN2 NeuronCores.

Math (reference):
    A_hat = D^-1/2 (A + I) D^-1/2        (deg over dst incl. self-loops)
    h1    = relu(A_hat @ (x @ W1) + b1)
    out   = log_softmax(A_hat @ h1 @ W2 + b2)

Decomposition (nodes sharded by dst range across 8 cores, 3 launches):
    K1: ut   = dinv * (x @ W1)                       [per-core shard]
        writes ut rows (bf16) for the K2 gather table and utT (fp32)
        for the dense self-loop term.
    K2: htT  = dinv * relu(dinv * (A @ ut + ut_self) + b1)
        gt   = htT @ W2   (so layer 2 aggregates 16-wide)
        writes gt rows (bf16) for the K3 gather table and gtT (fp32).
    K3: out  = log_softmax(dinv * (A @ gt + dinv*gt_self) + b2)

Aggregation per core: edges (no self-loops) grouped by (dst-256-pair,
src chunk) and batched into large dma_gather calls (~11K descriptors)
to amortize the ~10.5us fixed cost per SWDGE gather call.  Scatter into
transposed PSUM accumulators [F, 256] via one bf16 matmul per 128-edge
block: lhsT = gathered rows (stationary), rhs = one-hot slot matrix.
Tables are bf16 with rows duplicated to 256B to satisfy the gather's
minimum element size.
"""

import os
import sys
import types

import numpy as np
import ml_dtypes

# ---------------------------------------------------------------- sizes
N = 100000
E = 1600000
F_IN = 256
H = 64
C = 16
NCORE = 8
P = 128
CHUNK = 25000            # int16-addressable source chunk
GPAIR = 4                # dst-pairs per gather call group
TRACE = bool(int(os.environ.get("BASS_GCN_TRACE", "0")))
SMALL = bool(int(os.environ.get("BASS_GCN_SMALL", "0")))
if SMALL:
    N, E, CHUNK, GPAIR = 12800, 96000, 3200, 3

LAST_EXEC_NS = []
_dbg = None


def _derived():
    ncn = N // NCORE
    padn = ((ncn + 255) // 256) * 256
    nwin = padn // P
    npair = nwin // 2
    nchunk = (N + CHUNK - 1) // CHUNK
    groups = [list(range(g, min(g + GPAIR, npair)))
              for g in range(0, npair, GPAIR)]
    return ncn, padn, nwin, npair, nchunk, groups


# ------------------------------------------------------- ntff shim (opt)
def _install_ntff_shim():
    try:
        if "antenv.axon_hooks" in sys.modules:
            return True
        sys.path.insert(0, "/root/.axon_site/trn_agent_boot")
        from trn_boot import _ntff_profile_via_ctypes  # type: ignore

        mod = types.ModuleType("antenv.axon_hooks")
        holder = [None]
        mod.set_axon_ntff_profile_hook = lambda h: holder.__setitem__(0, h)
        mod.get_axon_ntff_profile_hook = lambda: holder[0]
        sys.modules["antenv.axon_hooks"] = mod
        import antenv

        antenv.axon_hooks = mod
        mod.set_axon_ntff_profile_hook(
            _ntff_profile_via_ctypes("/opt/axon/libaxon_pjrt.so")
        )
        return True
    except Exception:
        return False


# ------------------------------------------------------------ host plan
def _build_plan(edge_index):
    """Edge index structures (functions of edge_index only).

    Per core: edges (no self-loops) with dst in the core's range are
    grouped by (pair = dst>>8, chunk = src//CHUNK) and laid out call by
    call: for each (group of GPAIR pairs, chunk), the member (pair,
    chunk) segments are padded to multiples of 128 descriptors and
    concatenated.  Segment sizes are made uniform across cores (max)
    so one SPMD program fits all.
    """
    ncn, padn, nwin, npair, nchunk, groups = _derived()
    nseg = npair * nchunk

    src_a = np.asarray(edge_index[0], np.int64)
    dst_a = np.asarray(edge_index[1], np.int64)
    deg = np.bincount(dst_a, minlength=N).astype(np.float64) + 1.0
    dinv = (1.0 / np.sqrt(deg)).astype(np.float32)

    per_core = []
    cnts = np.zeros((NCORE, nseg), np.int64)
    for c in range(NCORE):
        lo = c * ncn
        m = (dst_a >= lo) & (dst_a < lo + ncn)
        s = src_a[m]
        d = dst_a[m] - lo
        pair = d >> 8
        chunk = s // CHUNK
        segid = pair * nchunk + chunk
        order = np.argsort(segid, kind="stable")
        s, d, segid = s[order], d[order], segid[order]
        cnts[c] = np.bincount(segid, minlength=nseg)
        per_core.append((s, d, segid))

    S = 128 * ((cnts.max(axis=0) + 127) // 128)          # [nseg] uniform
    S = np.maximum(S, 128)
    # call layout: for each (group g, chunk ch): segments (p in g, ch)
    seg_order = []
    call_sizes = []
    for g in groups:
        for ch in range(nchunk):
            segs = [p * nchunk + ch for p in g]
            seg_order.extend(segs)
            call_sizes.append(int(sum(S[q] for q in segs)))
    assert max(call_sizes) <= 16000, call_sizes
    off = np.zeros(nseg + 1, np.int64)
    tot = 0
    seg_off = {}
    for q in seg_order:
        seg_off[q] = tot
        tot += int(S[q])
    total = tot

    idxw_l, slot_l = [], []
    for c in range(NCORE):
        s, d, segid = per_core[c]
        idx16 = np.zeros(total, np.int16)
        slot = np.full(total, 999.0, np.float32)
        seg_start = np.searchsorted(segid, np.arange(nseg))
        base = np.array([seg_off[q] for q in range(nseg)], np.int64)
        pos = base[segid] + (np.arange(len(s)) - seg_start[segid])
        idx16[pos] = (s % CHUNK).astype(np.int16)
        slot[pos] = (d & 255).astype(np.float32)
        idxw_l.append(np.ascontiguousarray(
            np.tile(idx16.reshape(-1, 16).T, (8, 1))))
        slot_l.append(np.ascontiguousarray(slot.reshape(-1, P).T))

    return {
        "S": S,
        "call_sizes": call_sizes,
        "idxw": idxw_l,
        "slot": slot_l,
        "dinv": dinv,
    }


# --------------------------------------------------------- bass builders
def _bass_mods():
    import concourse.bass as bass
    import concourse.bacc as bacc
    import concourse.tile as tile
    import concourse.mybir as mybir
    from concourse import library_config
    from concourse.masks import make_identity

    return bass, bacc, tile, mybir, library_config, make_identity


def _build_k1():
    """From xT (bf16) produce: ut rows [padn, H] bf16 (gather table
    shard), utT [H, padn] fp32 (self-loop term, already dinv-scaled)."""
    bass, bacc, tile, mybir, libcfg, make_identity = _bass_mods()
    ncn, padn, nwin, npair, nchunk, groups = _derived()
    f32, bf16 = mybir.dt.float32, mybir.dt.bfloat16

    nc = bacc.Bacc("TRN2", target_bir_lowering=False, debug=False,
                   num_devices=NCORE)
    xT = nc.dram_tensor("xT", [F_IN, padn], f32, kind="ExternalInput").ap()
    w1 = nc.dram_tensor("w1", [F_IN, H], f32, kind="ExternalInput").ap()
    dTd = nc.dram_tensor("dinvT", [H, padn], f32, kind="ExternalInput").ap()
    utd = nc.dram_tensor("ut", [padn, P], bf16, kind="ExternalOutput").ap()
    utTd = nc.dram_tensor("utT", [H, padn], f32, kind="ExternalOutput").ap()

    kf = F_IN // P          # 2
    SW = 4                  # windows per stripe (psum bank = 512 fp32)
    stripes = []
    w0 = 0
    while w0 < nwin:
        stripes.append((w0, min(SW, nwin - w0)))
        w0 += SW

    with tile.TileContext(nc) as tc:
        with (
            tc.tile_pool(name="const", bufs=1) as constp,
            tc.tile_pool(name="xin", bufs=3) as xp,
            tc.tile_pool(name="ps", bufs=2, space="PSUM") as psump,
            tc.tile_pool(name="wk", bufs=3) as wp,
        ):
            w1_s = constp.tile([P, kf * H], f32)
            for k in range(kf):
                nc.sync.dma_start(w1_s[:, k * H:(k + 1) * H],
                                  w1[k * P:(k + 1) * P, :])
            identH = constp.tile([H, H], bf16)
            make_identity(nc, identH[:])

            for (ws, sw) in stripes:
                c0 = ws * P
                SC = sw * P
                up = psump.tile([H, SW * P], f32, tag="up", bufs=2)
                for k in range(kf):
                    xt = xp.tile([P, SW * P], f32, tag="xt")
                    nc.sync.dma_start(xt[:, :SC], xT[k * P:(k + 1) * P,
                                                     c0:c0 + SC])
                    nc.tensor.matmul(up[:, :SC],
                                     lhsT=w1_s[:, k * H:(k + 1) * H],
                                     rhs=xt[:, :SC], start=(k == 0),
                                     stop=(k == kf - 1))
                dT = xp.tile([H, SW * P], f32, tag="dT")
                nc.sync.dma_start(dT[:, :SC], dTd[:, c0:c0 + SC])
                uT = wp.tile([H, SW * P], f32, tag="uT")
                nc.vector.tensor_tensor(uT[:, :SC], up[:, :SC], dT[:, :SC],
                                        op=mybir.AluOpType.mult)
                nc.sync.dma_start(utTd[:, c0:c0 + SC], uT[:, :SC])
                uTb = wp.tile([H, SW * P], bf16, tag="uTb")
                nc.vector.tensor_copy(uTb[:, :SC], uT[:, :SC])
                uTr = wp.tile([H, SW * P], f32, tag="uTr")
                nc.vector.tensor_tensor(uTr[:, :SC], uT[:, :SC],
                                        uTb[:, :SC],
                                        op=mybir.AluOpType.subtract)
                uTrb = wp.tile([H, SW * P], bf16, tag="uTrb")
                nc.vector.tensor_copy(uTrb[:, :SC], uTr[:, :SC])
                stage = wp.tile([P, SW, P], bf16, tag="stage")
                for w in range(sw):
                    tp = psump.tile([P, H], bf16, tag="tp", bufs=2)
                    nc.tensor.transpose(tp[:], uTb[:, w * P:(w + 1) * P],
                                        identH[:])
                    nc.vector.tensor_copy(stage[:, w, 0:H], tp[:])
                    tr = psump.tile([P, H], bf16, tag="tr", bufs=2)
                    nc.tensor.transpose(tr[:], uTrb[:, w * P:(w + 1) * P],
                                        identH[:])
                    nc.vector.tensor_copy(stage[:, w, H:P], tr[:])
                nc.sync.dma_start(
                    utd[c0:c0 + SC, :].rearrange("(b a) h -> a b h", b=sw),
                    stage[:, :sw, :])
    nc.compile()
    return nc


def _agg(nc, tc, mybir, pools, table, idx_s, slot_s, iota_s, S, call_sizes,
         feats, drain_fn, lh_slices=None):
    """Batched gather + transposed one-hot scatter.

    For each (group, chunk) call: one dma_gather of call_sizes[i]
    descriptors; per 128-desc block one bf16 matmul accumulating into
    the pair's PSUM tile [feats, 256].  drain_fn(p, ps) after a pair's
    last chunk."""
    f32, bf16 = mybir.dt.float32, mybir.dt.bfloat16
    ncn, padn, nwin, npair, nchunk, groups = _derived()
    if lh_slices is None:
        lh_slices = [(0, feats)]
    nsl = len(lh_slices)
    gatp, selp, psump = pools
    off16 = 0
    mmcol = 0
    ci = 0
    accp = psump.parent_pool if False else None
    for g in groups:
        acc = gatp.tile([feats, GPAIR, 2 * P], f32, tag="acc", bufs=1,
                        name="acc")
        for ch in range(nchunk):
            sz = call_sizes[ci]
            ci += 1
            gat = gatp.tile([P, sz // P, P], bf16, tag="gat", name="gat")
            nc.gpsimd.dma_gather(
                gat[:],
                table[ch * CHUNK:min(N, (ch + 1) * CHUNK), :],
                idx_s[:, off16:off16 + sz // 16],
                sz, sz, P, elem_step=P, single_packet=False,
                queue_num=ci % 4,
            )
            off16 += sz // 16
            bb = 0
            for gi, p in enumerate(g):
                nb = int(S[p * nchunk + ch]) // P
                nmm = nb * nsl
                pseg = psump.tile([feats, 2 * P], f32, tag="pseg", bufs=4,
                                  name="pseg")
                done = 0
                for _ in range(nb):
                    sel = selp.tile([P, 2 * P], bf16, tag="sel", name="sel")
                    nc.vector.tensor_tensor(
                        out=sel[:],
                        in0=slot_s[:, mmcol:mmcol + 1].to_broadcast(
                            [P, 2 * P]),
                        in1=iota_s[:],
                        op=mybir.AluOpType.is_equal,
                    )
                    for (a, b) in lh_slices:
                        nc.tensor.matmul(
                            pseg[:], lhsT=gat[:, bb, a:b], rhs=sel[:],
                            start=(done == 0), stop=(done == nmm - 1),
                        )
                        done += 1
                    mmcol += 1
                    bb += 1
                if ch == 0:
                    nc.vector.tensor_copy(acc[:, gi, :], pseg[:])
                else:
                    nc.vector.tensor_tensor(acc[:, gi, :], acc[:, gi, :],
                                            pseg[:],
                                            op=mybir.AluOpType.add)
        drain_fn(g, acc)


def _build_k2(idx_cols, slot_cols, S, call_sizes):
    """Aggregate ut -> htT -> gt rows (bf16) + gtT (fp32)."""
    bass, bacc, tile, mybir, libcfg, make_identity = _bass_mods()
    ncn, padn, nwin, npair, nchunk, groups = _derived()
    f32, bf16 = mybir.dt.float32, mybir.dt.bfloat16

    nc = bacc.Bacc("TRN2", target_bir_lowering=False, debug=False,
                   num_devices=NCORE, dynamic_dma_scratch_size=49152,
                   num_swdge_queues=4)
    table = nc.dram_tensor("table", [N, P], bf16, kind="ExternalInput").ap()
    idxd = nc.dram_tensor("idx", [P, idx_cols], mybir.dt.int16,
                          kind="ExternalInput").ap()
    slotd = nc.dram_tensor("slot", [P, slot_cols], bf16,
                           kind="ExternalInput").ap()
    iotad = nc.dram_tensor("iota", [P, 2 * P], bf16,
                           kind="ExternalInput").ap()
    utTd = nc.dram_tensor("utT", [H, padn], f32, kind="ExternalInput").ap()
    dT64d = nc.dram_tensor("dinvT64", [H, padn], f32,
                           kind="ExternalInput").ap()
    b1d = nc.dram_tensor("b1col", [H, 1], f32, kind="ExternalInput").ap()
    w2d = nc.dram_tensor("w2", [H, C], bf16, kind="ExternalInput").ap()
    gtd = nc.dram_tensor("gt", [padn, C], bf16, kind="ExternalOutput").ap()
    gtTd = nc.dram_tensor("gtT", [C, padn], f32, kind="ExternalOutput").ap()

    with tile.TileContext(nc) as tc:
        with (
            tc.tile_pool(name="const", bufs=1) as constp,
            tc.tile_pool(name="gat", bufs=3) as gatp,
            tc.tile_pool(name="sel", bufs=4) as selp,
            tc.tile_pool(name="ps", bufs=1, space="PSUM") as psump,
            tc.tile_pool(name="dr", bufs=2) as drp,
            tc.tile_pool(name="st", bufs=2) as stp,
        ):
            with tc.tile_critical():
                nc.gpsimd.load_library(libcfg.mlp)
            idx_s = constp.tile([P, idx_cols], mybir.dt.int16)
            nc.sync.dma_start(idx_s[:], idxd[:, :])
            slot_s = constp.tile([P, slot_cols], bf16)
            nc.sync.dma_start(slot_s[:], slotd[:, :])
            iota_s = constp.tile([P, 2 * P], bf16)
            nc.sync.dma_start(iota_s[:], iotad[:, :])
            b1_s = constp.tile([H, 1], f32)
            nc.sync.dma_start(b1_s[:], b1d[:, :])
            w2_s = constp.tile([H, C], bf16)
            nc.sync.dma_start(w2_s[:], w2d[:, :])
            identC = constp.tile([C, C], bf16)
            make_identity(nc, identC[:])

            # per-group drain: acc [H, GPAIR, 256] -> gt rows + gtT cols
            def drain(g, acc):
                ng = len(g)
                W = ng * 2 * P
                c0 = g[0] * 2 * P
                av = acc[:, 0:ng, :].rearrange("f g c -> f (g c)")
                uT = drp.tile([H, GPAIR * 2 * P], f32, tag="uTsl",
                              name="uTsl")
                nc.sync.dma_start(uT[:, :W], utTd[:, c0:c0 + W])
                dTt = drp.tile([H, GPAIR * 2 * P], f32, tag="dTsl",
                               name="dTsl")
                nc.sync.dma_start(dTt[:, :W], dT64d[:, c0:c0 + W])
                dT = dTt[:, :W]
                t1 = drp.tile([H, GPAIR * 2 * P], f32, tag="t1", name="t1")
                nc.vector.tensor_tensor(t1[:, :W], av, uT[:, :W],
                                        op=mybir.AluOpType.add)
                nc.vector.tensor_tensor(t1[:, :W], t1[:, :W], dT,
                                        op=mybir.AluOpType.mult)
                nc.vector.tensor_scalar_add(t1[:, :W], t1[:, :W], b1_s[:])
                nc.vector.tensor_scalar_max(t1[:, :W], t1[:, :W], 0.0)
                hb = drp.tile([H, GPAIR * 2 * P], bf16, tag="hb", name="hb")
                nc.vector.tensor_tensor(hb[:, :W], t1[:, :W], dT,
                                        op=mybir.AluOpType.mult)
                gps = psump.tile([C, GPAIR * 2 * P], f32, tag="gps", bufs=1,
                                 name="gps")
                for j in range(0, W, 512):
                    je = min(j + 512, W)
                    nc.tensor.matmul(gps[:, j:je], lhsT=w2_s[:],
                                     rhs=hb[:, j:je], start=True, stop=True)
                gT = drp.tile([C, GPAIR * 2 * P], f32, tag="gT", name="gT")
                nc.vector.tensor_copy(gT[:, :W], gps[:, :W])
                nc.sync.dma_start(gtTd[:, c0:c0 + W], gT[:, :W])
                gTb = drp.tile([C, GPAIR * 2 * P], bf16, tag="gTb",
                               name="gTb")
                nc.vector.tensor_copy(gTb[:, :W], gT[:, :W])
                stage = stp.tile([P, 2 * GPAIR, C], bf16, tag="gstage",
                                 name="gstage")
                for w in range(2 * ng):
                    tp = psump.tile([P, C], bf16, tag="gtp", bufs=2,
                                    name="gtp")
                    nc.tensor.transpose(tp[:], gTb[:, w * P:(w + 1) * P],
                                        identC[:])
                    nc.vector.tensor_copy(stage[:, w, :], tp[:])
                nc.sync.dma_start(
                    gtd[c0:c0 + W, :].rearrange("(b a) h -> a b h",
                                                b=2 * ng),
                    stage[:, :2 * ng, :])

            _agg(nc, tc, mybir, (gatp, selp, psump), table, idx_s, slot_s,
                 iota_s, S, call_sizes, H, drain,
                 lh_slices=[(0, H)])
    nc.compile()
    return nc


def _build_k3(idx_cols, slot_cols, S, call_sizes):
    """Aggregate gt -> log_softmax out rows [padn, C] fp32."""
    bass, bacc, tile, mybir, libcfg, make_identity = _bass_mods()
    ncn, padn, nwin, npair, nchunk, groups = _derived()
    f32, bf16 = mybir.dt.float32, mybir.dt.bfloat16

    nc = bacc.Bacc("TRN2", target_bir_lowering=False, debug=False,
                   num_devices=NCORE, dynamic_dma_scratch_size=49152,
                   num_swdge_queues=4)
    table = nc.dram_tensor("table", [N, P], bf16, kind="ExternalInput").ap()
    idxd = nc.dram_tensor("idx", [P, idx_cols], mybir.dt.int16,
                          kind="ExternalInput").ap()
    slotd = nc.dram_tensor("slot", [P, slot_cols], bf16,
                           kind="ExternalInput").ap()
    iotad = nc.dram_tensor("iota", [P, 2 * P], bf16,
                           kind="ExternalInput").ap()
    gtTd = nc.dram_tensor("gtT", [C, padn], f32, kind="ExternalInput").ap()
    dT16d = nc.dram_tensor("dinvT16", [C, padn], f32,
                           kind="ExternalInput").ap()
    b2d = nc.dram_tensor("b2rep", [P, C], f32, kind="ExternalInput").ap()
    outd = nc.dram_tensor("out", [padn, C], f32, kind="ExternalOutput").ap()

    with tile.TileContext(nc) as tc:
        with (
            tc.tile_pool(name="const", bufs=1) as constp,
            tc.tile_pool(name="gat", bufs=3) as gatp,
            tc.tile_pool(name="sel", bufs=4) as selp,
            tc.tile_pool(name="ps", bufs=1, space="PSUM") as psump,
            tc.tile_pool(name="dr", bufs=2) as drp,
            tc.tile_pool(name="st", bufs=2) as stp,
        ):
            with tc.tile_critical():
                nc.gpsimd.load_library(libcfg.mlp)
            idx_s = constp.tile([P, idx_cols], mybir.dt.int16)
            nc.sync.dma_start(idx_s[:], idxd[:, :])
            slot_s = constp.tile([P, slot_cols], bf16)
            nc.sync.dma_start(slot_s[:], slotd[:, :])
            iota_s = constp.tile([P, 2 * P], bf16)
            nc.sync.dma_start(iota_s[:], iotad[:, :])
            b2_s = constp.tile([P, C], f32)
            nc.sync.dma_start(b2_s[:], b2d[:, :])
            identC = constp.tile([C, C], bf16)
            make_identity(nc, identC[:])

            def drain(g, acc):
                ng = len(g)
                W = ng * 2 * P
                c0 = g[0] * 2 * P
                av = acc[:, 0:ng, :].rearrange("f g c -> f (g c)")
                gT = drp.tile([C, GPAIR * 2 * P], f32, tag="gTsl",
                              name="gTsl")
                nc.sync.dma_start(gT[:, :W], gtTd[:, c0:c0 + W])
                dTt = drp.tile([C, GPAIR * 2 * P], f32, tag="dTsl",
                               name="dTsl")
                nc.sync.dma_start(dTt[:, :W], dT16d[:, c0:c0 + W])
                dT = dTt[:, :W]
                t0 = drp.tile([C, GPAIR * 2 * P], f32, tag="t0", name="t0")
                nc.vector.tensor_tensor(t0[:, :W], av, gT[:, :W],
                                        op=mybir.AluOpType.add)
                ob = drp.tile([C, GPAIR * 2 * P], bf16, tag="ob", name="ob")
                nc.vector.tensor_tensor(ob[:, :W], t0[:, :W], dT,
                                        op=mybir.AluOpType.mult)
                stage = stp.tile([P, 2 * GPAIR, C], f32, tag="ostage",
                                 name="ostage")
                for w in range(2 * ng):
                    tp = psump.tile([P, C], bf16, tag="otp", bufs=2,
                                    name="otp")
                    nc.tensor.transpose(tp[:], ob[:, w * P:(w + 1) * P],
                                        identC[:])
                    z = drp.tile([P, C], f32, tag="z", name="z")
                    nc.vector.tensor_tensor(z[:], tp[:], b2_s[:],
                                            op=mybir.AluOpType.add)
                    negm = drp.tile([P, 1], f32, tag="negm", name="negm")
                    nc.vector.tensor_reduce(
                        negm[:], z[:], axis=mybir.AxisListType.X,
                        op=mybir.AluOpType.max, negate=True)
                    e = drp.tile([P, C], f32, tag="e", name="e")
                    sa = drp.tile([P, 1], f32, tag="sa", name="sa")
                    nc.scalar.activation(
                        e[:], z[:], mybir.ActivationFunctionType.Exp,
                        bias=negm[:], accum_out=sa[:])
                    lns = drp.tile([P, 1], f32, tag="lns", name="lns")
                    nc.scalar.activation(
                        lns[:], sa[:], mybir.ActivationFunctionType.Ln)
                    nc.vector.tensor_scalar(
                        out=stage[:, w, :], in0=z[:], scalar1=negm[:],
                        scalar2=lns[:], op0=mybir.AluOpType.add,
                        op1=mybir.AluOpType.subtract)
                nc.sync.dma_start(
                    outd[c0:c0 + W, :].rearrange("(b a) h -> a b h",
                                                 b=2 * ng),
                    stage[:, :2 * ng, :])

            _agg(nc, tc, mybir, (gatp, selp, psump), table, idx_s, slot_s,
                 iota_s, S, call_sizes, C, drain)
    nc.compile()
    return nc


def _run(nc, in_maps):
    if os.environ.get("BASS_GCN_SIM"):
        from concourse.bass_interp import MultiCoreSim

        sim = MultiCoreSim(nc, num_cores=NCORE, trace=False)
        for c in range(NCORE):
            for k, v in in_maps[c].items():
                sim.cores[c].tensor(k)[:] = v
        sim.simulate()
        outs = []
        for c in range(NCORE):
            names = [
                a.memorylocations[0].name
                for a in nc.m.functions[0].allocations
                if getattr(a, "kind", None) == "ExternalOutput"
            ]
            outs.append({n: np.array(sim.cores[c].tensor(n)) for n in names})
        return outs

    from concourse.bass_utils import run_bass_kernel_spmd

    trace = TRACE and _install_ntff_shim()
    res = run_bass_kernel_spmd(nc, in_maps, core_ids=list(range(NCORE)),
                               trace=trace)
    if res.exec_time_ns:
        LAST_EXEC_NS.append(res.exec_time_ns)
    return res.results


# ---------------------------------------------------------------- kernel
def kernel(x, edge_index, W1, b1, W2, b2):
    ncn, padn, nwin, npair, nchunk, groups = _derived()
    LAST_EXEC_NS.clear()

    x = np.asarray(x, np.float32)
    edge_index = np.asarray(edge_index)
    W1 = np.asarray(W1, np.float32)
    b1 = np.asarray(b1, np.float32)
    W2 = np.asarray(W2, np.float32)
    b2 = np.asarray(b2, np.float32)

    plan = _build_plan(edge_index)
    S, call_sizes, dinv = plan["S"], plan["call_sizes"], plan["dinv"]
    idx_cols = plan["idxw"][0].shape[1]
    slot_cols = plan["slot"][0].shape[1]

    iota2 = np.tile(np.arange(2 * P, dtype=np.float32)[None, :], (P, 1))
    b2rep = np.tile(b2[None, :], (P, 1)).astype(np.float32)

    # ---- K1
    nc1 = _build_k1()
    in1 = []
    for c in range(NCORE):
        xc = np.zeros((padn, F_IN), np.float32)
        xc[:ncn] = x[c * ncn:(c + 1) * ncn]
        dv = np.zeros(padn, np.float32)
        dv[:ncn] = dinv[c * ncn:(c + 1) * ncn]
        in1.append({
            "xT": np.ascontiguousarray(xc.T),
            "w1": W1,
            "dinvT": np.ascontiguousarray(
                np.broadcast_to(dv[None, :], (H, padn))),
        })
    r1 = _run(nc1, in1)
    table1 = np.ascontiguousarray(np.concatenate(
        [r1[c]["ut"][:ncn] for c in range(NCORE)], axis=0))

    # ---- K2
    nc2 = _build_k2(idx_cols, slot_cols, S, call_sizes)
    in2 = []
    for c in range(NCORE):
        dv = np.zeros(padn, np.float32)
        dv[:ncn] = dinv[c * ncn:(c + 1) * ncn]
        in2.append({
            "table": table1,
            "idx": plan["idxw"][c],
            "slot": plan["slot"][c].astype(ml_dtypes.bfloat16),
            "iota": iota2.astype(ml_dtypes.bfloat16),
            "utT": r1[c]["utT"],
            "dinvT64": np.ascontiguousarray(
                np.broadcast_to(dv[None, :], (H, padn))),
            "b1col": b1[:, None],
            "w2": W2.astype(ml_dtypes.bfloat16),
        })
    r2 = _run(nc2, in2)
    gt_full = np.concatenate([r2[c]["gt"][:ncn] for c in range(NCORE)],
                             axis=0)
    table2 = np.zeros((N, P), ml_dtypes.bfloat16)
    for rep in range(P // C):
        table2[:, rep * C:(rep + 1) * C] = gt_full

    # ---- K3
    nc3 = _build_k3(idx_cols, slot_cols, S, call_sizes)
    in3 = []
    for c in range(NCORE):
        dv = np.zeros(padn, np.float32)
        dv[:ncn] = dinv[c * ncn:(c + 1) * ncn]
        in3.append({
            "table": table2,
            "idx": plan["idxw"][c],
            "slot": plan["slot"][c].astype(ml_dtypes.bfloat16),
            "iota": iota2.astype(ml_dtypes.bfloat16),
            "gtT": r2[c]["gtT"],
            "dinvT16": np.ascontiguousarray(
                np.broadcast_to(dv[None, :], (C, padn))),
            "b2rep": b2rep,
        })
    r3 = _run(nc3, in3)
    global _dbg
    _dbg = {"r1": r1, "r2": r2, "r3": r3}
    out = np.concatenate([r3[c]["out"][:ncn] for c in range(NCORE)], axis=0)
    return np.ascontiguousarray(out.astype(np.float32))


# revision 19
# speedup vs baseline: 2.0628x; 1.0892x over previous
"""GCN (2-layer, symmetric-normalized, self-loops) on 8 TRN2 NeuronCores.

Math (reference):
    A_hat = D^-1/2 (A + I) D^-1/2        (deg over dst incl. self-loops)
    h1    = relu(A_hat @ (x @ W1) + b1)
    out   = log_softmax(A_hat @ h1 @ W2 + b2)

Decomposition (nodes sharded by dst range across 8 cores, 3 launches):
    K1: ut   = dinv * (x @ W1)                       [per-core shard]
        writes ut rows (bf16) for the K2 gather table and utT (fp32)
        for the dense self-loop term.
    K2: htT  = dinv * relu(dinv * (A @ ut + ut_self) + b1)
        gt   = htT @ W2   (so layer 2 aggregates 16-wide)
        writes gt rows (bf16) for the K3 gather table and gtT (fp32).
    K3: out  = log_softmax(dinv * (A @ gt + dinv*gt_self) + b2)

Aggregation per core: edges (no self-loops) grouped by (dst-256-pair,
src chunk) and batched into large dma_gather calls (~11K descriptors)
to amortize the ~10.5us fixed cost per SWDGE gather call.  Scatter into
transposed PSUM accumulators [F, 256] via one bf16 matmul per 128-edge
block: lhsT = gathered rows (stationary), rhs = one-hot slot matrix.
Tables are bf16 with rows duplicated to 256B to satisfy the gather's
minimum element size.
"""

import os
import sys
import types

import numpy as np
import ml_dtypes

# ---------------------------------------------------------------- sizes
N = 100000
E = 1600000
F_IN = 256
H = 64
C = 16
NCORE = 8
P = 128
CHUNK = 25000            # int16-addressable source chunk
GPAIR = 4                # dst-pairs per gather call group
TRACE = bool(int(os.environ.get("BASS_GCN_TRACE", "0")))
SMALL = bool(int(os.environ.get("BASS_GCN_SMALL", "0")))
if SMALL:
    N, E, CHUNK, GPAIR = 12800, 96000, 3200, 3

LAST_EXEC_NS = []
_dbg = None


def _derived():
    ncn = N // NCORE
    padn = ((ncn + 255) // 256) * 256
    nwin = padn // P
    npair = nwin // 2
    nchunk = (N + CHUNK - 1) // CHUNK
    groups = [list(range(g, min(g + GPAIR, npair)))
              for g in range(0, npair, GPAIR)]
    return ncn, padn, nwin, npair, nchunk, groups


# ------------------------------------------------------- ntff shim (opt)
def _install_ntff_shim():
    try:
        if "antenv.axon_hooks" in sys.modules:
            return True
        sys.path.insert(0, "/root/.axon_site/trn_agent_boot")
        from trn_boot import _ntff_profile_via_ctypes  # type: ignore

        mod = types.ModuleType("antenv.axon_hooks")
        holder = [None]
        mod.set_axon_ntff_profile_hook = lambda h: holder.__setitem__(0, h)
        mod.get_axon_ntff_profile_hook = lambda: holder[0]
        sys.modules["antenv.axon_hooks"] = mod
        import antenv

        antenv.axon_hooks = mod
        mod.set_axon_ntff_profile_hook(
            _ntff_profile_via_ctypes("/opt/axon/libaxon_pjrt.so")
        )
        return True
    except Exception:
        return False


# ------------------------------------------------------------ host plan
def _build_plan(edge_index):
    """Edge index structures (functions of edge_index only).

    Per core: edges (no self-loops) with dst in the core's range are
    grouped by (pair = dst>>8, chunk = src//CHUNK) and laid out call by
    call: for each (group of GPAIR pairs, chunk), the member (pair,
    chunk) segments are padded to multiples of 128 descriptors and
    concatenated.  Segment sizes are made uniform across cores (max)
    so one SPMD program fits all.
    """
    ncn, padn, nwin, npair, nchunk, groups = _derived()
    nseg = npair * nchunk

    src_a = np.asarray(edge_index[0], np.int64)
    dst_a = np.asarray(edge_index[1], np.int64)
    deg = np.bincount(dst_a, minlength=N).astype(np.float64) + 1.0
    dinv = (1.0 / np.sqrt(deg)).astype(np.float32)

    per_core = []
    cnts = np.zeros((NCORE, nseg), np.int64)
    for c in range(NCORE):
        lo = c * ncn
        m = (dst_a >= lo) & (dst_a < lo + ncn)
        s = src_a[m]
        d = dst_a[m] - lo
        pair = d >> 8
        chunk = s // CHUNK
        segid = pair * nchunk + chunk
        order = np.argsort(segid, kind="stable")
        s, d, segid = s[order], d[order], segid[order]
        cnts[c] = np.bincount(segid, minlength=nseg)
        per_core.append((s, d, segid))

    S = 128 * ((cnts.max(axis=0) + 127) // 128)          # [nseg] uniform
    S = np.maximum(S, 128)
    # call layout: for each (group g, chunk ch): segments (p in g, ch)
    seg_order = []
    call_sizes = []
    for g in groups:
        for ch in range(nchunk):
            segs = [p * nchunk + ch for p in g]
            seg_order.extend(segs)
            call_sizes.append(int(sum(S[q] for q in segs)))
    assert max(call_sizes) <= 16000, call_sizes
    off = np.zeros(nseg + 1, np.int64)
    tot = 0
    seg_off = {}
    for q in seg_order:
        seg_off[q] = tot
        tot += int(S[q])
    total = tot

    idxw_l, slot_l = [], []
    for c in range(NCORE):
        s, d, segid = per_core[c]
        idx16 = np.zeros(total, np.int16)
        slot = np.full(total, 999.0, np.float32)
        seg_start = np.searchsorted(segid, np.arange(nseg))
        base = np.array([seg_off[q] for q in range(nseg)], np.int64)
        pos = base[segid] + (np.arange(len(s)) - seg_start[segid])
        idx16[pos] = (s % CHUNK).astype(np.int16)
        slot[pos] = (d & 255).astype(np.float32)
        idxw_l.append(np.ascontiguousarray(
            np.tile(idx16.reshape(-1, 16).T, (8, 1))))
        slot_l.append(np.ascontiguousarray(slot.reshape(-1, P).T))

    return {
        "S": S,
        "call_sizes": call_sizes,
        "idxw": idxw_l,
        "slot": slot_l,
        "dinv": dinv,
    }


# --------------------------------------------------------- bass builders
def _bass_mods():
    import concourse.bass as bass
    import concourse.bacc as bacc
    import concourse.tile as tile
    import concourse.mybir as mybir
    from concourse import library_config
    from concourse.masks import make_identity

    return bass, bacc, tile, mybir, library_config, make_identity


def _build_k1():
    """From xT (bf16) produce: ut rows [padn, H] bf16 (gather table
    shard), utT [H, padn] fp32 (self-loop term, already dinv-scaled)."""
    bass, bacc, tile, mybir, libcfg, make_identity = _bass_mods()
    ncn, padn, nwin, npair, nchunk, groups = _derived()
    f32, bf16 = mybir.dt.float32, mybir.dt.bfloat16

    nc = bacc.Bacc("TRN2", target_bir_lowering=False, debug=False,
                   num_devices=NCORE)
    xT = nc.dram_tensor("xT", [F_IN, padn], f32, kind="ExternalInput").ap()
    w1 = nc.dram_tensor("w1", [F_IN, H], f32, kind="ExternalInput").ap()
    dTd = nc.dram_tensor("dinvT", [H, padn], f32, kind="ExternalInput").ap()
    utd = nc.dram_tensor("ut", [padn, P], bf16, kind="ExternalOutput").ap()
    utTd = nc.dram_tensor("utT", [H, padn], f32, kind="ExternalOutput").ap()

    kf = F_IN // P          # 2
    SW = 4                  # windows per stripe (psum bank = 512 fp32)
    stripes = []
    w0 = 0
    while w0 < nwin:
        stripes.append((w0, min(SW, nwin - w0)))
        w0 += SW

    with tile.TileContext(nc) as tc:
        with (
            tc.tile_pool(name="const", bufs=1) as constp,
            tc.tile_pool(name="xin", bufs=3) as xp,
            tc.tile_pool(name="ps", bufs=2, space="PSUM") as psump,
            tc.tile_pool(name="wk", bufs=3) as wp,
        ):
            w1_s = constp.tile([P, kf * H], f32)
            for k in range(kf):
                nc.sync.dma_start(w1_s[:, k * H:(k + 1) * H],
                                  w1[k * P:(k + 1) * P, :])
            identH = constp.tile([H, H], bf16)
            make_identity(nc, identH[:])

            for (ws, sw) in stripes:
                c0 = ws * P
                SC = sw * P
                up = psump.tile([H, SW * P], f32, tag="up", bufs=2)
                for k in range(kf):
                    xt = xp.tile([P, SW * P], f32, tag="xt")
                    nc.sync.dma_start(xt[:, :SC], xT[k * P:(k + 1) * P,
                                                     c0:c0 + SC])
                    nc.tensor.matmul(up[:, :SC],
                                     lhsT=w1_s[:, k * H:(k + 1) * H],
                                     rhs=xt[:, :SC], start=(k == 0),
                                     stop=(k == kf - 1))
                dT = xp.tile([H, SW * P], f32, tag="dT")
                nc.sync.dma_start(dT[:, :SC], dTd[:, c0:c0 + SC])
                uT = wp.tile([H, SW * P], f32, tag="uT")
                nc.vector.tensor_tensor(uT[:, :SC], up[:, :SC], dT[:, :SC],
                                        op=mybir.AluOpType.mult)
                nc.sync.dma_start(utTd[:, c0:c0 + SC], uT[:, :SC])
                uTb = wp.tile([H, SW * P], bf16, tag="uTb")
                nc.vector.tensor_copy(uTb[:, :SC], uT[:, :SC])
                uTr = wp.tile([H, SW * P], f32, tag="uTr")
                nc.vector.tensor_tensor(uTr[:, :SC], uT[:, :SC],
                                        uTb[:, :SC],
                                        op=mybir.AluOpType.subtract)
                uTrb = wp.tile([H, SW * P], bf16, tag="uTrb")
                nc.vector.tensor_copy(uTrb[:, :SC], uTr[:, :SC])
                stage = wp.tile([P, SW, P], bf16, tag="stage")
                for w in range(sw):
                    tp = psump.tile([P, H], bf16, tag="tp", bufs=2)
                    nc.tensor.transpose(tp[:], uTb[:, w * P:(w + 1) * P],
                                        identH[:])
                    nc.vector.tensor_copy(stage[:, w, 0:H], tp[:])
                    tr = psump.tile([P, H], bf16, tag="tr", bufs=2)
                    nc.tensor.transpose(tr[:], uTrb[:, w * P:(w + 1) * P],
                                        identH[:])
                    nc.vector.tensor_copy(stage[:, w, H:P], tr[:])
                nc.sync.dma_start(
                    utd[c0:c0 + SC, :].rearrange("(b a) h -> a b h", b=sw),
                    stage[:, :sw, :])
    nc.compile()
    return nc


def _agg(nc, tc, mybir, pools, table, idx_s, slot_s, iota_s, S, call_sizes,
         feats, drain_fn, lh_slices=None):
    """Batched gather + transposed one-hot scatter.

    For each (group, chunk) call: one dma_gather of call_sizes[i]
    descriptors; per 128-desc block one bf16 matmul accumulating into
    the pair's PSUM tile [feats, 256].  drain_fn(p, ps) after a pair's
    last chunk."""
    f32, bf16 = mybir.dt.float32, mybir.dt.bfloat16
    ncn, padn, nwin, npair, nchunk, groups = _derived()
    if lh_slices is None:
        lh_slices = [(0, feats)]
    nsl = len(lh_slices)
    gatp, selp, psump = pools
    off16 = 0
    mmcol = 0
    ci = 0
    accp = psump.parent_pool if False else None
    for g in groups:
        acc = gatp.tile([feats, GPAIR, 2 * P], f32, tag="acc", bufs=1,
                        name="acc")
        for ch in range(nchunk):
            sz = call_sizes[ci]
            ci += 1
            gat = gatp.tile([P, sz // P, P], bf16, tag="gat", name="gat")
            nc.gpsimd.dma_gather(
                gat[:],
                table[ch * CHUNK:min(N, (ch + 1) * CHUNK), :],
                idx_s[:, off16:off16 + sz // 16],
                sz, sz, P, elem_step=P, single_packet=False,
                queue_num=ci % 4,
            )
            off16 += sz // 16
            bb = 0
            for gi, p in enumerate(g):
                nb = int(S[p * nchunk + ch]) // P
                nmm = nb * nsl
                pseg = psump.tile([feats, 2 * P], f32, tag="pseg", bufs=4,
                                  name="pseg")
                done = 0
                for _ in range(nb):
                    sel = selp.tile([P, 2 * P], bf16, tag="sel", name="sel")
                    nc.vector.tensor_tensor(
                        out=sel[:],
                        in0=slot_s[:, mmcol:mmcol + 1].to_broadcast(
                            [P, 2 * P]),
                        in1=iota_s[:],
                        op=mybir.AluOpType.is_equal,
                    )
                    for (a, b) in lh_slices:
                        nc.tensor.matmul(
                            pseg[:], lhsT=gat[:, bb, a:b], rhs=sel[:],
                            start=(done == 0), stop=(done == nmm - 1),
                        )
                        done += 1
                    mmcol += 1
                    bb += 1
                if ch == 0:
                    nc.vector.tensor_copy(acc[:, gi, :], pseg[:])
                else:
                    nc.vector.tensor_tensor(acc[:, gi, :], acc[:, gi, :],
                                            pseg[:],
                                            op=mybir.AluOpType.add)
        drain_fn(g, acc)


def _build_k2(idx_cols, slot_cols, S, call_sizes):
    """Aggregate ut -> htT -> gt rows (bf16) + gtT (fp32)."""
    bass, bacc, tile, mybir, libcfg, make_identity = _bass_mods()
    ncn, padn, nwin, npair, nchunk, groups = _derived()
    f32, bf16 = mybir.dt.float32, mybir.dt.bfloat16

    nc = bacc.Bacc("TRN2", target_bir_lowering=False, debug=False,
                   num_devices=NCORE, dynamic_dma_scratch_size=49152,
                   num_swdge_queues=4)
    table = nc.dram_tensor("table", [N, P], bf16, kind="ExternalInput").ap()
    idxd = nc.dram_tensor("idx", [P, idx_cols], mybir.dt.int16,
                          kind="ExternalInput").ap()
    slotd = nc.dram_tensor("slot", [P, slot_cols], bf16,
                           kind="ExternalInput").ap()
    iotad = nc.dram_tensor("iota", [P, 2 * P], bf16,
                           kind="ExternalInput").ap()
    utTd = nc.dram_tensor("utT", [H, padn], f32, kind="ExternalInput").ap()
    dT64d = nc.dram_tensor("dinvT64", [H, padn], f32,
                           kind="ExternalInput").ap()
    b1d = nc.dram_tensor("b1col", [H, 1], f32, kind="ExternalInput").ap()
    w2d = nc.dram_tensor("w2", [H, C], bf16, kind="ExternalInput").ap()
    gtd = nc.dram_tensor("gt", [padn, C], bf16, kind="ExternalOutput").ap()
    gtTd = nc.dram_tensor("gtT", [C, padn], f32, kind="ExternalOutput").ap()

    with tile.TileContext(nc) as tc:
        with (
            tc.tile_pool(name="const", bufs=1) as constp,
            tc.tile_pool(name="gat", bufs=4) as gatp,
            tc.tile_pool(name="sel", bufs=4) as selp,
            tc.tile_pool(name="ps", bufs=1, space="PSUM") as psump,
            tc.tile_pool(name="dr", bufs=2) as drp,
            tc.tile_pool(name="st", bufs=2) as stp,
        ):
            with tc.tile_critical():
                nc.gpsimd.load_library(libcfg.mlp)
            idx_s = constp.tile([P, idx_cols], mybir.dt.int16)
            nc.sync.dma_start(idx_s[:], idxd[:, :])
            slot_s = constp.tile([P, slot_cols], bf16)
            nc.sync.dma_start(slot_s[:], slotd[:, :])
            iota_s = constp.tile([P, 2 * P], bf16)
            nc.sync.dma_start(iota_s[:], iotad[:, :])
            b1_s = constp.tile([H, 1], f32)
            nc.sync.dma_start(b1_s[:], b1d[:, :])
            w2_s = constp.tile([H, C], bf16)
            nc.sync.dma_start(w2_s[:], w2d[:, :])
            identC = constp.tile([C, C], bf16)
            make_identity(nc, identC[:])

            # per-group drain: acc [H, GPAIR, 256] -> gt rows + gtT cols
            def drain(g, acc):
                ng = len(g)
                W = ng * 2 * P
                c0 = g[0] * 2 * P
                av = acc[:, 0:ng, :].rearrange("f g c -> f (g c)")
                uT = drp.tile([H, GPAIR * 2 * P], f32, tag="uTsl",
                              name="uTsl")
                nc.sync.dma_start(uT[:, :W], utTd[:, c0:c0 + W])
                dTt = drp.tile([H, GPAIR * 2 * P], f32, tag="dTsl",
                               name="dTsl")
                nc.sync.dma_start(dTt[:, :W], dT64d[:, c0:c0 + W])
                dT = dTt[:, :W]
                t1 = drp.tile([H, GPAIR * 2 * P], f32, tag="t1", name="t1")
                nc.vector.tensor_tensor(t1[:, :W], av, uT[:, :W],
                                        op=mybir.AluOpType.add)
                nc.vector.tensor_tensor(t1[:, :W], t1[:, :W], dT,
                                        op=mybir.AluOpType.mult)
                nc.vector.tensor_scalar_add(t1[:, :W], t1[:, :W], b1_s[:])
                nc.vector.tensor_scalar_max(t1[:, :W], t1[:, :W], 0.0)
                hb = drp.tile([H, GPAIR * 2 * P], bf16, tag="hb", name="hb")
                nc.vector.tensor_tensor(hb[:, :W], t1[:, :W], dT,
                                        op=mybir.AluOpType.mult)
                gps = psump.tile([C, GPAIR * 2 * P], f32, tag="gps", bufs=1,
                                 name="gps")
                for j in range(0, W, 512):
                    je = min(j + 512, W)
                    nc.tensor.matmul(gps[:, j:je], lhsT=w2_s[:],
                                     rhs=hb[:, j:je], start=True, stop=True)
                gT = drp.tile([C, GPAIR * 2 * P], f32, tag="gT", name="gT")
                nc.vector.tensor_copy(gT[:, :W], gps[:, :W])
                nc.sync.dma_start(gtTd[:, c0:c0 + W], gT[:, :W])
                gTb = drp.tile([C, GPAIR * 2 * P], bf16, tag="gTb",
                               name="gTb")
                nc.vector.tensor_copy(gTb[:, :W], gT[:, :W])
                stage = stp.tile([P, 2 * GPAIR, C], bf16, tag="gstage",
                                 name="gstage")
                for w in range(2 * ng):
                    tp = psump.tile([P, C], bf16, tag="gtp", bufs=2,
                                    name="gtp")
                    nc.tensor.transpose(tp[:], gTb[:, w * P:(w + 1) * P],
                                        identC[:])
                    nc.vector.tensor_copy(stage[:, w, :], tp[:])
                nc.sync.dma_start(
                    gtd[c0:c0 + W, :].rearrange("(b a) h -> a b h",
                                                b=2 * ng),
                    stage[:, :2 * ng, :])

            _agg(nc, tc, mybir, (gatp, selp, psump), table, idx_s, slot_s,
                 iota_s, S, call_sizes, H, drain,
                 lh_slices=[(0, H)])
    nc.compile()
    return nc


def _build_k3(idx_cols, slot_cols, S, call_sizes):
    """Aggregate gt -> log_softmax out rows [padn, C] fp32."""
    bass, bacc, tile, mybir, libcfg, make_identity = _bass_mods()
    ncn, padn, nwin, npair, nchunk, groups = _derived()
    f32, bf16 = mybir.dt.float32, mybir.dt.bfloat16

    nc = bacc.Bacc("TRN2", target_bir_lowering=False, debug=False,
                   num_devices=NCORE, dynamic_dma_scratch_size=49152,
                   num_swdge_queues=4)
    table = nc.dram_tensor("table", [N, P], bf16, kind="ExternalInput").ap()
    idxd = nc.dram_tensor("idx", [P, idx_cols], mybir.dt.int16,
                          kind="ExternalInput").ap()
    slotd = nc.dram_tensor("slot", [P, slot_cols], bf16,
                           kind="ExternalInput").ap()
    iotad = nc.dram_tensor("iota", [P, 2 * P], bf16,
                           kind="ExternalInput").ap()
    gtTd = nc.dram_tensor("gtT", [C, padn], f32, kind="ExternalInput").ap()
    dT16d = nc.dram_tensor("dinvT16", [C, padn], f32,
                           kind="ExternalInput").ap()
    b2d = nc.dram_tensor("b2rep", [P, C], f32, kind="ExternalInput").ap()
    outd = nc.dram_tensor("out", [padn, C], f32, kind="ExternalOutput").ap()

    with tile.TileContext(nc) as tc:
        with (
            tc.tile_pool(name="const", bufs=1) as constp,
            tc.tile_pool(name="gat", bufs=4) as gatp,
            tc.tile_pool(name="sel", bufs=4) as selp,
            tc.tile_pool(name="ps", bufs=1, space="PSUM") as psump,
            tc.tile_pool(name="dr", bufs=2) as drp,
            tc.tile_pool(name="st", bufs=2) as stp,
        ):
            with tc.tile_critical():
                nc.gpsimd.load_library(libcfg.mlp)
            idx_s = constp.tile([P, idx_cols], mybir.dt.int16)
            nc.sync.dma_start(idx_s[:], idxd[:, :])
            slot_s = constp.tile([P, slot_cols], bf16)
            nc.sync.dma_start(slot_s[:], slotd[:, :])
            iota_s = constp.tile([P, 2 * P], bf16)
            nc.sync.dma_start(iota_s[:], iotad[:, :])
            b2_s = constp.tile([P, C], f32)
            nc.sync.dma_start(b2_s[:], b2d[:, :])
            identC = constp.tile([C, C], bf16)
            make_identity(nc, identC[:])

            def drain(g, acc):
                ng = len(g)
                W = ng * 2 * P
                c0 = g[0] * 2 * P
                av = acc[:, 0:ng, :].rearrange("f g c -> f (g c)")
                gT = drp.tile([C, GPAIR * 2 * P], f32, tag="gTsl",
                              name="gTsl")
                nc.sync.dma_start(gT[:, :W], gtTd[:, c0:c0 + W])
                dTt = drp.tile([C, GPAIR * 2 * P], f32, tag="dTsl",
                               name="dTsl")
                nc.sync.dma_start(dTt[:, :W], dT16d[:, c0:c0 + W])
                dT = dTt[:, :W]
                t0 = drp.tile([C, GPAIR * 2 * P], f32, tag="t0", name="t0")
                nc.vector.tensor_tensor(t0[:, :W], av, gT[:, :W],
                                        op=mybir.AluOpType.add)
                ob = drp.tile([C, GPAIR * 2 * P], bf16, tag="ob", name="ob")
                nc.vector.tensor_tensor(ob[:, :W], t0[:, :W], dT,
                                        op=mybir.AluOpType.mult)
                stage = stp.tile([P, 2 * GPAIR, C], f32, tag="ostage",
                                 name="ostage")
                zz = drp.tile([P, 2 * GPAIR, C], f32, tag="zz", name="zz")
                nm = drp.tile([P, 2 * GPAIR, 1], f32, tag="nm", name="nm")
                for w in range(2 * ng):
                    tp = psump.tile([P, C], bf16, tag="otp", bufs=2,
                                    name="otp")
                    nc.tensor.transpose(tp[:], ob[:, w * P:(w + 1) * P],
                                        identC[:])
                    nc.vector.tensor_tensor(zz[:, w, :], tp[:], b2_s[:],
                                            op=mybir.AluOpType.add)
                nc.vector.tensor_reduce(
                    nm[:, 0:2 * ng, :], zz[:, 0:2 * ng, :],
                    axis=mybir.AxisListType.X,
                    op=mybir.AluOpType.max, negate=True)
                for w in range(2 * ng):
                    e = drp.tile([P, C], f32, tag="e", name="e")
                    sa = drp.tile([P, 1], f32, tag="sa", name="sa")
                    nc.scalar.activation(
                        e[:], zz[:, w, :], mybir.ActivationFunctionType.Exp,
                        bias=nm[:, w, :], accum_out=sa[:])
                    lns = drp.tile([P, 1], f32, tag="lns", name="lns")
                    nc.scalar.activation(
                        lns[:], sa[:], mybir.ActivationFunctionType.Ln)
                    nc.vector.tensor_scalar(
                        out=stage[:, w, :], in0=zz[:, w, :],
                        scalar1=nm[:, w, :],
                        scalar2=lns[:], op0=mybir.AluOpType.add,
                        op1=mybir.AluOpType.subtract)
                nc.sync.dma_start(
                    outd[c0:c0 + W, :].rearrange("(b a) h -> a b h",
                                                 b=2 * ng),
                    stage[:, :2 * ng, :])

            _agg(nc, tc, mybir, (gatp, selp, psump), table, idx_s, slot_s,
                 iota_s, S, call_sizes, C, drain)
    nc.compile()
    return nc


def _run(nc, in_maps):
    if os.environ.get("BASS_GCN_SIM"):
        from concourse.bass_interp import MultiCoreSim

        sim = MultiCoreSim(nc, num_cores=NCORE, trace=False)
        for c in range(NCORE):
            for k, v in in_maps[c].items():
                sim.cores[c].tensor(k)[:] = v
        sim.simulate()
        outs = []
        for c in range(NCORE):
            names = [
                a.memorylocations[0].name
                for a in nc.m.functions[0].allocations
                if getattr(a, "kind", None) == "ExternalOutput"
            ]
            outs.append({n: np.array(sim.cores[c].tensor(n)) for n in names})
        return outs

    from concourse.bass_utils import run_bass_kernel_spmd

    trace = TRACE and _install_ntff_shim()
    res = run_bass_kernel_spmd(nc, in_maps, core_ids=list(range(NCORE)),
                               trace=trace)
    if res.exec_time_ns:
        LAST_EXEC_NS.append(res.exec_time_ns)
    return res.results


# ---------------------------------------------------------------- kernel
def kernel(x, edge_index, W1, b1, W2, b2):
    ncn, padn, nwin, npair, nchunk, groups = _derived()
    LAST_EXEC_NS.clear()

    x = np.asarray(x, np.float32)
    edge_index = np.asarray(edge_index)
    W1 = np.asarray(W1, np.float32)
    b1 = np.asarray(b1, np.float32)
    W2 = np.asarray(W2, np.float32)
    b2 = np.asarray(b2, np.float32)

    plan = _build_plan(edge_index)
    S, call_sizes, dinv = plan["S"], plan["call_sizes"], plan["dinv"]
    idx_cols = plan["idxw"][0].shape[1]
    slot_cols = plan["slot"][0].shape[1]

    iota2 = np.tile(np.arange(2 * P, dtype=np.float32)[None, :], (P, 1))
    b2rep = np.tile(b2[None, :], (P, 1)).astype(np.float32)

    # ---- K1
    nc1 = _build_k1()
    in1 = []
    for c in range(NCORE):
        xc = np.zeros((padn, F_IN), np.float32)
        xc[:ncn] = x[c * ncn:(c + 1) * ncn]
        dv = np.zeros(padn, np.float32)
        dv[:ncn] = dinv[c * ncn:(c + 1) * ncn]
        in1.append({
            "xT": np.ascontiguousarray(xc.T),
            "w1": W1,
            "dinvT": np.ascontiguousarray(
                np.broadcast_to(dv[None, :], (H, padn))),
        })
    r1 = _run(nc1, in1)
    table1 = np.ascontiguousarray(np.concatenate(
        [r1[c]["ut"][:ncn] for c in range(NCORE)], axis=0))

    # ---- K2
    nc2 = _build_k2(idx_cols, slot_cols, S, call_sizes)
    in2 = []
    for c in range(NCORE):
        dv = np.zeros(padn, np.float32)
        dv[:ncn] = dinv[c * ncn:(c + 1) * ncn]
        in2.append({
            "table": table1,
            "idx": plan["idxw"][c],
            "slot": plan["slot"][c].astype(ml_dtypes.bfloat16),
            "iota": iota2.astype(ml_dtypes.bfloat16),
            "utT": r1[c]["utT"],
            "dinvT64": np.ascontiguousarray(
                np.broadcast_to(dv[None, :], (H, padn))),
            "b1col": b1[:, None],
            "w2": W2.astype(ml_dtypes.bfloat16),
        })
    r2 = _run(nc2, in2)
    gt_full = np.concatenate([r2[c]["gt"][:ncn] for c in range(NCORE)],
                             axis=0)
    table2 = np.zeros((N, P), ml_dtypes.bfloat16)
    for rep in range(P // C):
        table2[:, rep * C:(rep + 1) * C] = gt_full

    # ---- K3
    nc3 = _build_k3(idx_cols, slot_cols, S, call_sizes)
    in3 = []
    for c in range(NCORE):
        dv = np.zeros(padn, np.float32)
        dv[:ncn] = dinv[c * ncn:(c + 1) * ncn]
        in3.append({
            "table": table2,
            "idx": plan["idxw"][c],
            "slot": plan["slot"][c].astype(ml_dtypes.bfloat16),
            "iota": iota2.astype(ml_dtypes.bfloat16),
            "gtT": r2[c]["gtT"],
            "dinvT16": np.ascontiguousarray(
                np.broadcast_to(dv[None, :], (C, padn))),
            "b2rep": b2rep,
        })
    r3 = _run(nc3, in3)
    global _dbg
    _dbg = {"r1": r1, "r2": r2, "r3": r3}
    out = np.concatenate([r3[c]["out"][:ncn] for c in range(NCORE)], axis=0)
    return np.ascontiguousarray(out.astype(np.float32))


# revision 21
# speedup vs baseline: 2.1930x; 1.0631x over previous
"""GCN (2-layer, symmetric-normalized, self-loops) on 8 TRN2 NeuronCores.

Math (reference):
    A_hat = D^-1/2 (A + I) D^-1/2        (deg over dst incl. self-loops)
    h1    = relu(A_hat @ (x @ W1) + b1)
    out   = log_softmax(A_hat @ h1 @ W2 + b2)

Decomposition (nodes sharded by dst range across 8 cores, 3 launches):
    K1: ut   = dinv * (x @ W1)                       [per-core shard]
        writes ut rows (bf16) for the K2 gather table and utT (fp32)
        for the dense self-loop term.
    K2: htT  = dinv * relu(dinv * (A @ ut + ut_self) + b1)
        gt   = htT @ W2   (so layer 2 aggregates 16-wide)
        writes gt rows (bf16) for the K3 gather table and gtT (fp32).
    K3: out  = log_softmax(dinv * (A @ gt + dinv*gt_self) + b2)

Aggregation per core: edges (no self-loops) grouped by (dst-256-pair,
src chunk) and batched into large dma_gather calls (~11K descriptors)
to amortize the ~10.5us fixed cost per SWDGE gather call.  Scatter into
transposed PSUM accumulators [F, 256] via one bf16 matmul per 128-edge
block: lhsT = gathered rows (stationary), rhs = one-hot slot matrix.
Tables are bf16 with rows duplicated to 256B to satisfy the gather's
minimum element size.
"""

import os
import sys
import types

import numpy as np
import ml_dtypes

# ---------------------------------------------------------------- sizes
N = 100000
E = 1600000
F_IN = 256
H = 64
C = 16
NCORE = 8
P = 128
CHUNK = 25000            # int16-addressable source chunk
GPAIR = 4                # dst-pairs per gather call group
TRACE = bool(int(os.environ.get("BASS_GCN_TRACE", "0")))
SMALL = bool(int(os.environ.get("BASS_GCN_SMALL", "0")))
if SMALL:
    N, E, CHUNK, GPAIR = 12800, 96000, 3200, 3

LAST_EXEC_NS = []
_dbg = None


def _derived():
    ncn = N // NCORE
    padn = ((ncn + 255) // 256) * 256
    nwin = padn // P
    npair = nwin // 2
    nchunk = (N + CHUNK - 1) // CHUNK
    groups = [list(range(g, min(g + GPAIR, npair)))
              for g in range(0, npair, GPAIR)]
    return ncn, padn, nwin, npair, nchunk, groups


# ------------------------------------------------------- ntff shim (opt)
def _install_ntff_shim():
    try:
        if "antenv.axon_hooks" in sys.modules:
            return True
        sys.path.insert(0, "/root/.axon_site/trn_agent_boot")
        from trn_boot import _ntff_profile_via_ctypes  # type: ignore

        mod = types.ModuleType("antenv.axon_hooks")
        holder = [None]
        mod.set_axon_ntff_profile_hook = lambda h: holder.__setitem__(0, h)
        mod.get_axon_ntff_profile_hook = lambda: holder[0]
        sys.modules["antenv.axon_hooks"] = mod
        import antenv

        antenv.axon_hooks = mod
        mod.set_axon_ntff_profile_hook(
            _ntff_profile_via_ctypes("/opt/axon/libaxon_pjrt.so")
        )
        return True
    except Exception:
        return False


# ------------------------------------------------------------ host plan
def _build_plan(edge_index):
    """Edge index structures (functions of edge_index only).

    Per core: edges (no self-loops) with dst in the core's range are
    grouped by (pair = dst>>8, chunk = src//CHUNK) and laid out call by
    call: for each (group of GPAIR pairs, chunk), the member (pair,
    chunk) segments are padded to multiples of 128 descriptors and
    concatenated.  Segment sizes are made uniform across cores (max)
    so one SPMD program fits all.
    """
    ncn, padn, nwin, npair, nchunk, groups = _derived()
    nseg = npair * nchunk

    src_a = np.asarray(edge_index[0], np.int64)
    dst_a = np.asarray(edge_index[1], np.int64)
    deg = np.bincount(dst_a, minlength=N).astype(np.float64) + 1.0
    dinv = (1.0 / np.sqrt(deg)).astype(np.float32)

    per_core = []
    cnts = np.zeros((NCORE, nseg), np.int64)
    for c in range(NCORE):
        lo = c * ncn
        m = (dst_a >= lo) & (dst_a < lo + ncn)
        s = src_a[m]
        d = dst_a[m] - lo
        pair = d >> 8
        chunk = s // CHUNK
        segid = pair * nchunk + chunk
        order = np.argsort(segid, kind="stable")
        s, d, segid = s[order], d[order], segid[order]
        cnts[c] = np.bincount(segid, minlength=nseg)
        per_core.append((s, d, segid))

    S = 128 * ((cnts.max(axis=0) + 127) // 128)          # [nseg] uniform
    S = np.maximum(S, 128)
    # call layout: for each (group g, chunk ch): segments (p in g, ch)
    seg_order = []
    call_sizes = []
    for g in groups:
        for ch in range(nchunk):
            segs = [p * nchunk + ch for p in g]
            seg_order.extend(segs)
            call_sizes.append(int(sum(S[q] for q in segs)))
    assert max(call_sizes) <= 16000, call_sizes
    off = np.zeros(nseg + 1, np.int64)
    tot = 0
    seg_off = {}
    for q in seg_order:
        seg_off[q] = tot
        tot += int(S[q])
    total = tot

    idxw_l, slot_l = [], []
    for c in range(NCORE):
        s, d, segid = per_core[c]
        idx16 = np.zeros(total, np.int16)
        slot = np.full(total, 999.0, np.float32)
        seg_start = np.searchsorted(segid, np.arange(nseg))
        base = np.array([seg_off[q] for q in range(nseg)], np.int64)
        pos = base[segid] + (np.arange(len(s)) - seg_start[segid])
        idx16[pos] = (s % CHUNK).astype(np.int16)
        slot[pos] = (d & 255).astype(np.float32)
        idxw_l.append(np.ascontiguousarray(
            np.tile(idx16.reshape(-1, 16).T, (8, 1))))
        slot_l.append(np.ascontiguousarray(slot.reshape(-1, P).T))

    return {
        "S": S,
        "call_sizes": call_sizes,
        "idxw": idxw_l,
        "slot": slot_l,
        "dinv": dinv,
    }


# --------------------------------------------------------- bass builders
def _bass_mods():
    import concourse.bass as bass
    import concourse.bacc as bacc
    import concourse.tile as tile
    import concourse.mybir as mybir
    from concourse import library_config
    from concourse.masks import make_identity

    return bass, bacc, tile, mybir, library_config, make_identity


def _build_k1():
    """From xT (bf16) produce: ut rows [padn, H] bf16 (gather table
    shard), utT [H, padn] fp32 (self-loop term, already dinv-scaled)."""
    bass, bacc, tile, mybir, libcfg, make_identity = _bass_mods()
    ncn, padn, nwin, npair, nchunk, groups = _derived()
    f32, bf16 = mybir.dt.float32, mybir.dt.bfloat16

    nc = bacc.Bacc("TRN2", target_bir_lowering=False, debug=False,
                   num_devices=NCORE)
    xT = nc.dram_tensor("xT", [F_IN, padn], f32, kind="ExternalInput").ap()
    w1 = nc.dram_tensor("w1", [F_IN, H], f32, kind="ExternalInput").ap()
    dTd = nc.dram_tensor("dinvT", [H, padn], f32, kind="ExternalInput").ap()
    utd = nc.dram_tensor("ut", [padn, P], bf16, kind="ExternalOutput").ap()
    utTd = nc.dram_tensor("utT", [H, padn], f32, kind="ExternalOutput").ap()

    kf = F_IN // P          # 2
    SW = 4                  # windows per stripe (psum bank = 512 fp32)
    stripes = []
    w0 = 0
    while w0 < nwin:
        stripes.append((w0, min(SW, nwin - w0)))
        w0 += SW

    with tile.TileContext(nc) as tc:
        with (
            tc.tile_pool(name="const", bufs=1) as constp,
            tc.tile_pool(name="xin", bufs=3) as xp,
            tc.tile_pool(name="ps", bufs=2, space="PSUM") as psump,
            tc.tile_pool(name="wk", bufs=3) as wp,
        ):
            w1_s = constp.tile([P, kf * H], f32)
            for k in range(kf):
                nc.sync.dma_start(w1_s[:, k * H:(k + 1) * H],
                                  w1[k * P:(k + 1) * P, :])
            identH = constp.tile([H, H], bf16)
            make_identity(nc, identH[:])

            for (ws, sw) in stripes:
                c0 = ws * P
                SC = sw * P
                up = psump.tile([H, SW * P], f32, tag="up", bufs=2)
                for k in range(kf):
                    xt = xp.tile([P, SW * P], f32, tag="xt")
                    nc.sync.dma_start(xt[:, :SC], xT[k * P:(k + 1) * P,
                                                     c0:c0 + SC])
                    nc.tensor.matmul(up[:, :SC],
                                     lhsT=w1_s[:, k * H:(k + 1) * H],
                                     rhs=xt[:, :SC], start=(k == 0),
                                     stop=(k == kf - 1))
                dT = xp.tile([H, SW * P], f32, tag="dT")
                nc.sync.dma_start(dT[:, :SC], dTd[:, c0:c0 + SC])
                uT = wp.tile([H, SW * P], f32, tag="uT")
                nc.vector.tensor_tensor(uT[:, :SC], up[:, :SC], dT[:, :SC],
                                        op=mybir.AluOpType.mult)
                nc.sync.dma_start(utTd[:, c0:c0 + SC], uT[:, :SC])
                uTb = wp.tile([H, SW * P], bf16, tag="uTb")
                nc.vector.tensor_copy(uTb[:, :SC], uT[:, :SC])
                uTr = wp.tile([H, SW * P], f32, tag="uTr")
                nc.vector.tensor_tensor(uTr[:, :SC], uT[:, :SC],
                                        uTb[:, :SC],
                                        op=mybir.AluOpType.subtract)
                uTrb = wp.tile([H, SW * P], bf16, tag="uTrb")
                nc.vector.tensor_copy(uTrb[:, :SC], uTr[:, :SC])
                stage = wp.tile([P, SW, P], bf16, tag="stage")
                for w in range(sw):
                    tp = psump.tile([P, H], bf16, tag="tp", bufs=2)
                    nc.tensor.transpose(tp[:], uTb[:, w * P:(w + 1) * P],
                                        identH[:])
                    nc.vector.tensor_copy(stage[:, w, 0:H], tp[:])
                    tr = psump.tile([P, H], bf16, tag="tr", bufs=2)
                    nc.tensor.transpose(tr[:], uTrb[:, w * P:(w + 1) * P],
                                        identH[:])
                    nc.vector.tensor_copy(stage[:, w, H:P], tr[:])
                nc.sync.dma_start(
                    utd[c0:c0 + SC, :].rearrange("(b a) h -> a b h", b=sw),
                    stage[:, :sw, :])
    nc.compile()
    return nc


def _agg(nc, tc, mybir, pools, table, idx_s, slot_s, iota_s, S, call_sizes,
         feats, drain_fn, lh_slices=None):
    """Batched gather + transposed one-hot scatter.

    For each (group, chunk) call: one dma_gather of call_sizes[i]
    descriptors; per 128-desc block one bf16 matmul accumulating into
    the pair's PSUM tile [feats, 256].  drain_fn(p, ps) after a pair's
    last chunk."""
    f32, bf16 = mybir.dt.float32, mybir.dt.bfloat16
    ncn, padn, nwin, npair, nchunk, groups = _derived()
    if lh_slices is None:
        lh_slices = [(0, feats)]
    nsl = len(lh_slices)
    gatp, selp, psump = pools
    off16 = 0
    mmcol = 0
    ci = 0
    accp = psump.parent_pool if False else None
    for g in groups:
        acc = gatp.tile([feats, GPAIR, 2 * P], f32, tag="acc", bufs=1,
                        name="acc")
        for ch in range(nchunk):
            sz = call_sizes[ci]
            ci += 1
            gat = gatp.tile([P, sz // P, P], bf16, tag="gat", name="gat")
            nc.gpsimd.dma_gather(
                gat[:],
                table[ch * CHUNK:min(N, (ch + 1) * CHUNK), :],
                idx_s[:, off16:off16 + sz // 16],
                sz, sz, P, elem_step=P, single_packet=False,
                queue_num=ci % 4,
            )
            off16 += sz // 16
            nblk = sz // P
            selc = selp.tile([P, nblk, 2 * P], bf16, tag="selc",
                             name="selc")
            nc.vector.tensor_tensor(
                out=selc[:],
                in0=slot_s[:, mmcol:mmcol + nblk].rearrange(
                    "p (n o) -> p n o", o=1).to_broadcast([P, nblk, 2 * P]),
                in1=iota_s[:].rearrange("p (o c) -> p o c", o=1).to_broadcast(
                    [P, nblk, 2 * P]),
                op=mybir.AluOpType.is_equal,
            )
            mmcol += nblk
            bb = 0
            for gi, p in enumerate(g):
                nb = int(S[p * nchunk + ch]) // P
                nmm = nb * nsl
                pseg = psump.tile([feats, 2 * P], f32, tag="pseg", bufs=4,
                                  name="pseg")
                done = 0
                for _ in range(nb):
                    for (a, b) in lh_slices:
                        nc.tensor.matmul(
                            pseg[:], lhsT=gat[:, bb, a:b], rhs=selc[:, bb, :],
                            start=(done == 0), stop=(done == nmm - 1),
                        )
                        done += 1
                    bb += 1
                if ch == 0:
                    nc.vector.tensor_copy(acc[:, gi, :], pseg[:])
                else:
                    nc.vector.tensor_tensor(acc[:, gi, :], acc[:, gi, :],
                                            pseg[:],
                                            op=mybir.AluOpType.add)
        drain_fn(g, acc)


def _build_k2(idx_cols, slot_cols, S, call_sizes):
    """Aggregate ut -> htT -> gt rows (bf16) + gtT (fp32)."""
    bass, bacc, tile, mybir, libcfg, make_identity = _bass_mods()
    ncn, padn, nwin, npair, nchunk, groups = _derived()
    f32, bf16 = mybir.dt.float32, mybir.dt.bfloat16

    nc = bacc.Bacc("TRN2", target_bir_lowering=False, debug=False,
                   num_devices=NCORE, dynamic_dma_scratch_size=49152,
                   num_swdge_queues=4)
    table = nc.dram_tensor("table", [N, P], bf16, kind="ExternalInput").ap()
    idxd = nc.dram_tensor("idx", [P, idx_cols], mybir.dt.int16,
                          kind="ExternalInput").ap()
    slotd = nc.dram_tensor("slot", [P, slot_cols], bf16,
                           kind="ExternalInput").ap()
    iotad = nc.dram_tensor("iota", [P, 2 * P], bf16,
                           kind="ExternalInput").ap()
    utTd = nc.dram_tensor("utT", [H, padn], f32, kind="ExternalInput").ap()
    dT64d = nc.dram_tensor("dinvT64", [H, padn], f32,
                           kind="ExternalInput").ap()
    b1d = nc.dram_tensor("b1col", [H, 1], f32, kind="ExternalInput").ap()
    w2d = nc.dram_tensor("w2", [H, C], bf16, kind="ExternalInput").ap()
    gtd = nc.dram_tensor("gt", [padn, C], bf16, kind="ExternalOutput").ap()
    gtTd = nc.dram_tensor("gtT", [C, padn], f32, kind="ExternalOutput").ap()

    with tile.TileContext(nc) as tc:
        with (
            tc.tile_pool(name="const", bufs=1) as constp,
            tc.tile_pool(name="gat", bufs=4) as gatp,
            tc.tile_pool(name="sel", bufs=2) as selp,
            tc.tile_pool(name="ps", bufs=1, space="PSUM") as psump,
            tc.tile_pool(name="dr", bufs=2) as drp,
            tc.tile_pool(name="st", bufs=2) as stp,
        ):
            with tc.tile_critical():
                nc.gpsimd.load_library(libcfg.mlp)
            idx_s = constp.tile([P, idx_cols], mybir.dt.int16)
            nc.sync.dma_start(idx_s[:], idxd[:, :])
            slot_s = constp.tile([P, slot_cols], bf16)
            nc.sync.dma_start(slot_s[:], slotd[:, :])
            iota_s = constp.tile([P, 2 * P], bf16)
            nc.sync.dma_start(iota_s[:], iotad[:, :])
            b1_s = constp.tile([H, 1], f32)
            nc.sync.dma_start(b1_s[:], b1d[:, :])
            w2_s = constp.tile([H, C], bf16)
            nc.sync.dma_start(w2_s[:], w2d[:, :])
            identC = constp.tile([C, C], bf16)
            make_identity(nc, identC[:])

            # per-group drain: acc [H, GPAIR, 256] -> gt rows + gtT cols
            def drain(g, acc):
                ng = len(g)
                W = ng * 2 * P
                c0 = g[0] * 2 * P
                av = acc[:, 0:ng, :].rearrange("f g c -> f (g c)")
                uT = drp.tile([H, GPAIR * 2 * P], f32, tag="uTsl",
                              name="uTsl")
                nc.sync.dma_start(uT[:, :W], utTd[:, c0:c0 + W])
                dTt = drp.tile([H, GPAIR * 2 * P], f32, tag="dTsl",
                               name="dTsl")
                nc.sync.dma_start(dTt[:, :W], dT64d[:, c0:c0 + W])
                dT = dTt[:, :W]
                t1 = drp.tile([H, GPAIR * 2 * P], f32, tag="t1", name="t1")
                nc.vector.tensor_tensor(t1[:, :W], av, uT[:, :W],
                                        op=mybir.AluOpType.add)
                nc.vector.tensor_tensor(t1[:, :W], t1[:, :W], dT,
                                        op=mybir.AluOpType.mult)
                nc.vector.tensor_scalar_add(t1[:, :W], t1[:, :W], b1_s[:])
                nc.vector.tensor_scalar_max(t1[:, :W], t1[:, :W], 0.0)
                hb = drp.tile([H, GPAIR * 2 * P], bf16, tag="hb", name="hb")
                nc.vector.tensor_tensor(hb[:, :W], t1[:, :W], dT,
                                        op=mybir.AluOpType.mult)
                gps = psump.tile([C, GPAIR * 2 * P], f32, tag="gps", bufs=1,
                                 name="gps")
                for j in range(0, W, 512):
                    je = min(j + 512, W)
                    nc.tensor.matmul(gps[:, j:je], lhsT=w2_s[:],
                                     rhs=hb[:, j:je], start=True, stop=True)
                gT = drp.tile([C, GPAIR * 2 * P], f32, tag="gT", name="gT")
                nc.vector.tensor_copy(gT[:, :W], gps[:, :W])
                nc.sync.dma_start(gtTd[:, c0:c0 + W], gT[:, :W])
                gTb = drp.tile([C, GPAIR * 2 * P], bf16, tag="gTb",
                               name="gTb")
                nc.vector.tensor_copy(gTb[:, :W], gT[:, :W])
                stage = stp.tile([P, 2 * GPAIR, C], bf16, tag="gstage",
                                 name="gstage")
                for w in range(2 * ng):
                    tp = psump.tile([P, C], bf16, tag="gtp", bufs=2,
                                    name="gtp")
                    nc.tensor.transpose(tp[:], gTb[:, w * P:(w + 1) * P],
                                        identC[:])
                    nc.vector.tensor_copy(stage[:, w, :], tp[:])
                nc.sync.dma_start(
                    gtd[c0:c0 + W, :].rearrange("(b a) h -> a b h",
                                                b=2 * ng),
                    stage[:, :2 * ng, :])

            _agg(nc, tc, mybir, (gatp, selp, psump), table, idx_s, slot_s,
                 iota_s, S, call_sizes, H, drain,
                 lh_slices=[(0, H)])
    nc.compile()
    return nc


def _build_k3(idx_cols, slot_cols, S, call_sizes):
    """Aggregate gt -> log_softmax out rows [padn, C] fp32."""
    bass, bacc, tile, mybir, libcfg, make_identity = _bass_mods()
    ncn, padn, nwin, npair, nchunk, groups = _derived()
    f32, bf16 = mybir.dt.float32, mybir.dt.bfloat16

    nc = bacc.Bacc("TRN2", target_bir_lowering=False, debug=False,
                   num_devices=NCORE, dynamic_dma_scratch_size=49152,
                   num_swdge_queues=4)
    table = nc.dram_tensor("table", [N, P], bf16, kind="ExternalInput").ap()
    idxd = nc.dram_tensor("idx", [P, idx_cols], mybir.dt.int16,
                          kind="ExternalInput").ap()
    slotd = nc.dram_tensor("slot", [P, slot_cols], bf16,
                           kind="ExternalInput").ap()
    iotad = nc.dram_tensor("iota", [P, 2 * P], bf16,
                           kind="ExternalInput").ap()
    gtTd = nc.dram_tensor("gtT", [C, padn], f32, kind="ExternalInput").ap()
    dT16d = nc.dram_tensor("dinvT16", [C, padn], f32,
                           kind="ExternalInput").ap()
    b2d = nc.dram_tensor("b2rep", [P, C], f32, kind="ExternalInput").ap()
    outd = nc.dram_tensor("out", [padn, C], f32, kind="ExternalOutput").ap()

    with tile.TileContext(nc) as tc:
        with (
            tc.tile_pool(name="const", bufs=1) as constp,
            tc.tile_pool(name="gat", bufs=4) as gatp,
            tc.tile_pool(name="sel", bufs=2) as selp,
            tc.tile_pool(name="ps", bufs=1, space="PSUM") as psump,
            tc.tile_pool(name="dr", bufs=2) as drp,
            tc.tile_pool(name="st", bufs=2) as stp,
        ):
            with tc.tile_critical():
                nc.gpsimd.load_library(libcfg.mlp)
            idx_s = constp.tile([P, idx_cols], mybir.dt.int16)
            nc.sync.dma_start(idx_s[:], idxd[:, :])
            slot_s = constp.tile([P, slot_cols], bf16)
            nc.sync.dma_start(slot_s[:], slotd[:, :])
            iota_s = constp.tile([P, 2 * P], bf16)
            nc.sync.dma_start(iota_s[:], iotad[:, :])
            b2_s = constp.tile([P, C], f32)
            nc.sync.dma_start(b2_s[:], b2d[:, :])
            identC = constp.tile([C, C], bf16)
            make_identity(nc, identC[:])

            def drain(g, acc):
                ng = len(g)
                W = ng * 2 * P
                c0 = g[0] * 2 * P
                av = acc[:, 0:ng, :].rearrange("f g c -> f (g c)")
                gT = drp.tile([C, GPAIR * 2 * P], f32, tag="gTsl",
                              name="gTsl")
                nc.sync.dma_start(gT[:, :W], gtTd[:, c0:c0 + W])
                dTt = drp.tile([C, GPAIR * 2 * P], f32, tag="dTsl",
                               name="dTsl")
                nc.sync.dma_start(dTt[:, :W], dT16d[:, c0:c0 + W])
                dT = dTt[:, :W]
                t0 = drp.tile([C, GPAIR * 2 * P], f32, tag="t0", name="t0")
                nc.vector.tensor_tensor(t0[:, :W], av, gT[:, :W],
                                        op=mybir.AluOpType.add)
                ob = drp.tile([C, GPAIR * 2 * P], bf16, tag="ob", name="ob")
                nc.vector.tensor_tensor(ob[:, :W], t0[:, :W], dT,
                                        op=mybir.AluOpType.mult)
                stage = stp.tile([P, 2 * GPAIR, C], f32, tag="ostage",
                                 name="ostage")
                zz = drp.tile([P, 2 * GPAIR, C], f32, tag="zz", name="zz")
                nm = drp.tile([P, 2 * GPAIR, 1], f32, tag="nm", name="nm")
                for w in range(2 * ng):
                    tp = psump.tile([P, C], bf16, tag="otp", bufs=2,
                                    name="otp")
                    nc.tensor.transpose(tp[:], ob[:, w * P:(w + 1) * P],
                                        identC[:])
                    nc.vector.tensor_tensor(zz[:, w, :], tp[:], b2_s[:],
                                            op=mybir.AluOpType.add)
                nc.vector.tensor_reduce(
                    nm[:, 0:2 * ng, :], zz[:, 0:2 * ng, :],
                    axis=mybir.AxisListType.X,
                    op=mybir.AluOpType.max, negate=True)
                for w in range(2 * ng):
                    e = drp.tile([P, C], f32, tag="e", name="e")
                    sa = drp.tile([P, 1], f32, tag="sa", name="sa")
                    nc.scalar.activation(
                        e[:], zz[:, w, :], mybir.ActivationFunctionType.Exp,
                        bias=nm[:, w, :], accum_out=sa[:])
                    lns = drp.tile([P, 1], f32, tag="lns", name="lns")
                    nc.scalar.activation(
                        lns[:], sa[:], mybir.ActivationFunctionType.Ln)
                    nc.vector.tensor_scalar(
                        out=stage[:, w, :], in0=zz[:, w, :],
                        scalar1=nm[:, w, :],
                        scalar2=lns[:], op0=mybir.AluOpType.add,
                        op1=mybir.AluOpType.subtract)
                nc.sync.dma_start(
                    outd[c0:c0 + W, :].rearrange("(b a) h -> a b h",
                                                 b=2 * ng),
                    stage[:, :2 * ng, :])

            _agg(nc, tc, mybir, (gatp, selp, psump), table, idx_s, slot_s,
                 iota_s, S, call_sizes, C, drain)
    nc.compile()
    return nc


def _run(nc, in_maps):
    if os.environ.get("BASS_GCN_SIM"):
        from concourse.bass_interp import MultiCoreSim

        sim = MultiCoreSim(nc, num_cores=NCORE, trace=False)
        for c in range(NCORE):
            for k, v in in_maps[c].items():
                sim.cores[c].tensor(k)[:] = v
        sim.simulate()
        outs = []
        for c in range(NCORE):
            names = [
                a.memorylocations[0].name
                for a in nc.m.functions[0].allocations
                if getattr(a, "kind", None) == "ExternalOutput"
            ]
            outs.append({n: np.array(sim.cores[c].tensor(n)) for n in names})
        return outs

    from concourse.bass_utils import run_bass_kernel_spmd

    trace = TRACE and _install_ntff_shim()
    res = run_bass_kernel_spmd(nc, in_maps, core_ids=list(range(NCORE)),
                               trace=trace)
    if res.exec_time_ns:
        LAST_EXEC_NS.append(res.exec_time_ns)
    return res.results


# ---------------------------------------------------------------- kernel
def kernel(x, edge_index, W1, b1, W2, b2):
    ncn, padn, nwin, npair, nchunk, groups = _derived()
    LAST_EXEC_NS.clear()

    x = np.asarray(x, np.float32)
    edge_index = np.asarray(edge_index)
    W1 = np.asarray(W1, np.float32)
    b1 = np.asarray(b1, np.float32)
    W2 = np.asarray(W2, np.float32)
    b2 = np.asarray(b2, np.float32)

    plan = _build_plan(edge_index)
    S, call_sizes, dinv = plan["S"], plan["call_sizes"], plan["dinv"]
    idx_cols = plan["idxw"][0].shape[1]
    slot_cols = plan["slot"][0].shape[1]

    iota2 = np.tile(np.arange(2 * P, dtype=np.float32)[None, :], (P, 1))
    b2rep = np.tile(b2[None, :], (P, 1)).astype(np.float32)

    # ---- K1
    nc1 = _build_k1()
    in1 = []
    for c in range(NCORE):
        xc = np.zeros((padn, F_IN), np.float32)
        xc[:ncn] = x[c * ncn:(c + 1) * ncn]
        dv = np.zeros(padn, np.float32)
        dv[:ncn] = dinv[c * ncn:(c + 1) * ncn]
        in1.append({
            "xT": np.ascontiguousarray(xc.T),
            "w1": W1,
            "dinvT": np.ascontiguousarray(
                np.broadcast_to(dv[None, :], (H, padn))),
        })
    r1 = _run(nc1, in1)
    table1 = np.ascontiguousarray(np.concatenate(
        [r1[c]["ut"][:ncn] for c in range(NCORE)], axis=0))

    # ---- K2
    nc2 = _build_k2(idx_cols, slot_cols, S, call_sizes)
    in2 = []
    for c in range(NCORE):
        dv = np.zeros(padn, np.float32)
        dv[:ncn] = dinv[c * ncn:(c + 1) * ncn]
        in2.append({
            "table": table1,
            "idx": plan["idxw"][c],
            "slot": plan["slot"][c].astype(ml_dtypes.bfloat16),
            "iota": iota2.astype(ml_dtypes.bfloat16),
            "utT": r1[c]["utT"],
            "dinvT64": np.ascontiguousarray(
                np.broadcast_to(dv[None, :], (H, padn))),
            "b1col": b1[:, None],
            "w2": W2.astype(ml_dtypes.bfloat16),
        })
    r2 = _run(nc2, in2)
    gt_full = np.concatenate([r2[c]["gt"][:ncn] for c in range(NCORE)],
                             axis=0)
    table2 = np.zeros((N, P), ml_dtypes.bfloat16)
    for rep in range(P // C):
        table2[:, rep * C:(rep + 1) * C] = gt_full

    # ---- K3
    nc3 = _build_k3(idx_cols, slot_cols, S, call_sizes)
    in3 = []
    for c in range(NCORE):
        dv = np.zeros(padn, np.float32)
        dv[:ncn] = dinv[c * ncn:(c + 1) * ncn]
        in3.append({
            "table": table2,
            "idx": plan["idxw"][c],
            "slot": plan["slot"][c].astype(ml_dtypes.bfloat16),
            "iota": iota2.astype(ml_dtypes.bfloat16),
            "gtT": r2[c]["gtT"],
            "dinvT16": np.ascontiguousarray(
                np.broadcast_to(dv[None, :], (C, padn))),
            "b2rep": b2rep,
        })
    r3 = _run(nc3, in3)
    global _dbg
    _dbg = {"r1": r1, "r2": r2, "r3": r3}
    out = np.concatenate([r3[c]["out"][:ncn] for c in range(NCORE)], axis=0)
    return np.ascontiguousarray(out.astype(np.float32))
